# revision 1
# baseline (speedup 1.0000x reference)
"""Two-layer GAT on 8 Trainium2 NeuronCores (Bass/Tile).

Host (numpy): append self-loops, degree-sort nodes (desc), pad node count to
VPAD (multiple of 8*128) and assign sorted nodes round-robin at 128-node
block granularity to the 8 cores (sorted-rank s -> block g=s//128,
lane=s%128 -> core c=g%8, local block j=g//8, table row = c*PC+j*128+lane).
Per block-rank j the chunk schedule is shared by all cores (SPMD: one
program, per-core tensor data).  Each dst node's edges occupy "slots"; a
chunk is slot k of all 128 lanes of a block.  Edge slots are split into a
lo-region (src table row < VPAD/2) and hi-region so every chunk's gather
indices fit in int16 after rebasing (dma_gather is int16-indexed).
Attention vectors are folded into the feature matmul (W_aug = [W | W@a_src |
W@a_dst]) and the bias into the gather table (h+b), which adds b exactly
after softmax normalization.

Device, per core (Tile): h-phase matmuls build the gather table
[h+b | alpha_src] and the local alpha_dst column; AllGather replicates the
table; aggregation gathers source rows with multi-chunk dma_gather (4 SWDGE
queues), computes w = exp(prelu(asrc+adst, 0.2)) * pad_mask on ACT/DVE,
builds V = [w*h | w] on DVE, and segment-sums via identity-stationary
matmuls accumulating in PSUM; epilogue fuses out = act(num * (1/den)) on ACT
(scale = per-lane reciprocal), with the layer-2 h-phase inline per block.
The final sigmoid rows are written per block; the host undoes the row
permutation.
"""

import numpy as np

NCORES = 8
F_IN = 128
HID = 64
HEADS = 2
OUT = 64
NEG_SLOPE = 0.2

TROW1 = 192  # L1 table row floats: [h+b1 (128) | asrc (2) | pad] (768B)
TROW2 = 128  # L2 table row floats: [h2+b2 (64) | asrc2 (1) | pad] (512B)
V1C = 130    # L1 V cols: [w0*h0 | w1*h1 | w0 | w1]
V2C = 65     # L2 V cols: [w*h2 | w]
GBATCH = 32  # max chunks per dma_gather

DEBUG = False
TRACE = False
_cache = {}


def _build_schedule(edge_index, n_nodes):
    ei = np.asarray(edge_index).astype(np.int64)
    src = np.concatenate([ei[0], np.arange(n_nodes, dtype=np.int64)])
    dst = np.concatenate([ei[1], np.arange(n_nodes, dtype=np.int64)])
    deg = np.bincount(dst, minlength=n_nodes)

    stripe = NCORES * 128
    vpad = ((n_nodes + stripe - 1) // stripe) * stripe
    pc = vpad // NCORES
    nb = pc // 128
    half = vpad // 2
    assert half <= 32768

    degp = np.zeros(vpad, np.int64)
    degp[:n_nodes] = deg
    order = np.argsort(-degp, kind="stable")
    rank = np.empty(vpad, np.int64)
    rank[order] = np.arange(vpad)

    s = np.arange(vpad)
    g = s // 128
    lane = s % 128
    row_of_rank = (g % NCORES) * pc + (g // NCORES) * 128 + lane
    row_of_node = row_of_rank[rank[:n_nodes]]

    e_dstrow = row_of_node[dst]
    e_srcrow = row_of_node[src]
    is_hi = e_srcrow >= half

    # per-(dstrow, lo/hi) counts
    lo_cnt = np.bincount(e_dstrow[~is_hi], minlength=vpad)
    hi_cnt = np.bincount(e_dstrow[is_hi], minlength=vpad)

    # per-block-rank shared chunk counts: max over the 8 cores' j-th blocks
    # block of table row r: j = (r % pc) // 128
    jj = (np.arange(vpad) % pc) // 128
    K_lo = np.zeros(nb, np.int64)
    K_hi = np.zeros(nb, np.int64)
    np.maximum.at(K_lo, jj, lo_cnt)
    np.maximum.at(K_hi, jj, hi_cnt)
    K = K_lo + K_hi
    bump = K == 0
    K_lo[bump] += 1
    K[bump] += 1
    nch = int(K.sum())
    chunk_base = np.concatenate([[0], np.cumsum(K)])[:-1]

    # slot assignment: edges of a dst grouped, lo first then hi
    key = e_dstrow * 2 + is_hi
    ord_e = np.argsort(key, kind="stable")
    ds = e_dstrow[ord_e]
    hs = is_hi[ord_e]
    first = np.r_[True, ds[1:] != ds[:-1]]
    grp_start = np.flatnonzero(first)
    grp_id = np.cumsum(first) - 1
    slot = np.arange(ds.shape[0]) - grp_start[grp_id]  # 0.. within dst (lo+hi)
    c = ds // pc
    j = (ds % pc) // 128
    ln = ds % 128
    pos = chunk_base[j] + np.where(hs, K_lo[j] + slot - lo_cnt[ds], slot)
    assert (pos >= chunk_base[j]).all() and (pos < chunk_base[j] + K[j]).all()

    idx_val = np.where(hs, e_srcrow[ord_e] - half, e_srcrow[ord_e])
    idx_stream = np.zeros((NCORES, 128, nch), np.int16)
    mask_stream = np.zeros((NCORES, 128, nch), np.float32)
    idx_stream[c, ln, pos] = idx_val.astype(np.int16)
    mask_stream[c, ln, pos] = 1.0

    # wrapped int16 layout for dma_gather: chunk k -> columns 8k:8k+8 of
    # [128, 8*nch]; within a chunk the 128 lane-indices are wrapped as
    # flat[i] -> [i % 16, i // 16] and replicated over the 8 16-partition
    # groups.
    iw = idx_stream.transpose(0, 2, 1).reshape(NCORES, nch, 8, 16)
    iw = iw.transpose(0, 3, 1, 2).reshape(NCORES, 16, nch * 8)
    idx_wrapped = np.tile(iw, (1, 8, 1))  # [NCORES, 128, nch*8]

    return dict(vpad=vpad, pc=pc, nb=nb, half=half, K=K, K_lo=K_lo, K_hi=K_hi,
                nch=nch, chunk_base=chunk_base, row_of_node=row_of_node,
                idx_wrapped=np.ascontiguousarray(idx_wrapped),
                mask_stream=mask_stream)


def _build_program(vpad, pc, nb, half, K, K_lo, K_hi, nch, chunk_base):
    import concourse.bacc as bacc
    import concourse.bass as bass
    import concourse.mybir as mybir
    import concourse.tile as tile
    from concourse.masks import make_identity

    F32 = mybir.dt.float32
    I16 = mybir.dt.int16
    ACTF = mybir.ActivationFunctionType
    ALU = mybir.AluOpType

    nc = bacc.Bacc("TRN2", target_bir_lowering=False, debug=False,
                   num_devices=NCORES, num_swdge_queues=4)

    xt_d = nc.dram_tensor("xt", [128, pc], F32, kind="ExternalInput")
    idx_d = nc.dram_tensor("idx", [128, nch * 8], I16, kind="ExternalInput")
    msk_d = nc.dram_tensor("msk", [128, nch], F32, kind="ExternalInput")
    w1_d = nc.dram_tensor("w1aug", [128, 132], F32, kind="ExternalInput")
    w2_d = nc.dram_tensor("w2aug", [128, 66], F32, kind="ExternalInput")
    b1_d = nc.dram_tensor("b1rep", [128, 128], F32, kind="ExternalInput")
    b2_d = nc.dram_tensor("b2rep", [128, 64], F32, kind="ExternalInput")
    out_d = nc.dram_tensor("out", [pc, OUT], F32, kind="ExternalOutput")
    if DEBUG:
        dbg_h = nc.dram_tensor("dbg_h", [pc, TROW1], F32, kind="ExternalOutput")
        dbg_a1 = nc.dram_tensor("dbg_a1", [pc, 2], F32, kind="ExternalOutput")
        dbg_h2 = nc.dram_tensor("dbg_h2", [pc, TROW2], F32, kind="ExternalOutput")
        dbg_ps = nc.dram_tensor("dbg_ps", [pc, V1C], F32, kind="ExternalOutput")
        dbg_g = nc.dram_tensor("dbg_g", [128, GBATCH * TROW1], F32, kind="ExternalOutput")

    qn = [0]

    with tile.TileContext(nc) as tc:
        with (
            tc.tile_pool(name="const", bufs=1) as cp,
            tc.tile_pool(name="dram", bufs=1, space="DRAM") as dp,
            tc.tile_pool(name="xs", bufs=3) as xp,
            tc.tile_pool(name="hrow", bufs=3) as hp,
            tc.tile_pool(name="psh", bufs=2, space="PSUM") as psh,
            tc.tile_pool(name="g", bufs=4) as gp,
            tc.tile_pool(name="v", bufs=3) as vp,
            tc.tile_pool(name="wz", bufs=4) as wp,
            tc.tile_pool(name="psa", bufs=5, space="PSUM") as psa,
            tc.tile_pool(name="pst", bufs=1, space="PSUM") as pst,
            tc.tile_pool(name="epi", bufs=3) as ep,
        ):
            ident = cp.tile([128, 128], F32)
            make_identity(nc, ident[:])
            w1_sb = cp.tile([128, 132], F32)
            w2_sb = cp.tile([128, 66], F32)
            b1_sb = cp.tile([128, 128], F32)
            b2_sb = cp.tile([128, 64], F32)
            idx_t = cp.tile([128, nch * 8], I16)
            msk_t = cp.tile([128, nch], F32)
            for t, d in ((w1_sb, w1_d), (w2_sb, w2_d), (b1_sb, b1_d),
                         (b2_sb, b2_d), (idx_t, idx_d), (msk_t, msk_d)):
                nc.sync.dma_start(out=t[:], in_=d[:])

            h_loc = dp.tile([pc, TROW1], F32)
            h_full = dp.tile([vpad, TROW1], F32)
            a1_loc = dp.tile([pc, 2], F32)
            h2_loc = dp.tile([pc, TROW2], F32)
            h2_full = dp.tile([vpad, TROW2], F32)
            a2_loc = dp.tile([pc, 1], F32)

            # ---- Phase 1: L1 h-phase ----
            for j in range(nb):
                xt_sb = xp.tile([128, 128], F32, tag="xt")
                nc.sync.dma_start(out=xt_sb[:], in_=xt_d[:, j * 128:(j + 1) * 128])
                ps = psh.tile([128, 132], F32, tag="psh")
                nc.tensor.matmul(ps[:], lhsT=xt_sb[:], rhs=w1_sb[:],
                                 start=True, stop=True)
                hrow = hp.tile([128, TROW1], F32, tag="hrow")
                nc.vector.tensor_tensor(out=hrow[:, 0:128], in0=ps[:, 0:128],
                                        in1=b1_sb[:], op=ALU.add)
                nc.scalar.activation(hrow[:, 128:130], ps[:, 128:130], ACTF.Copy)
                arow = hp.tile([128, 2], F32, tag="arow")
                nc.scalar.activation(arow[:], ps[:, 130:132], ACTF.Copy)
                nc.sync.dma_start(out=h_loc[j * 128:(j + 1) * 128, 0:130],
                                  in_=hrow[:, 0:130])
                nc.sync.dma_start(out=a1_loc[j * 128:(j + 1) * 128, :], in_=arow[:])
                if DEBUG:
                    nc.sync.dma_start(out=dbg_h[j * 128:(j + 1) * 128, 0:130],
                                      in_=hrow[:, 0:130])
                    nc.sync.dma_start(out=dbg_a1[j * 128:(j + 1) * 128, :],
                                      in_=arow[:])

            # ---- Phase 2: AllGather L1 table ----
            nc.gpsimd.collective_compute(
                "AllGather", mybir.AluOpType.bypass,
                replica_groups=[list(range(NCORES))],
                ins=[h_loc[:]], outs=[h_full[:]],
            )

            def agg_layer(layer):
                if layer == 1:
                    table, arows, grow, vcols, heads = h_full, a1_loc, TROW1, V1C, 2
                else:
                    table, arows, grow, vcols, heads = h2_full, a2_loc, TROW2, V2C, 1
                hdim = (vcols - heads) // heads
                for j in range(nb):
                    adw = wp.tile([128, heads], F32, tag="adw")
                    nc.sync.dma_start(out=adw[:],
                                      in_=arows[j * 128:(j + 1) * 128, :])
                    psum = psa.tile([128, vcols], F32, tag="psa")
                    kj = int(K[j])
                    cb = int(chunk_base[j])
                    # batches: lo region then hi region, each split to <=GBATCH
                    batches = []
                    done = 0
                    for rl in (int(K_lo[j]), int(K_hi[j])):
                        r0 = done
                        while done < r0 + rl:
                            gl = min(GBATCH, r0 + rl - done)
                            batches.append((done, gl, done >= int(K_lo[j])))
                            done += gl
                    for (b0, gl, in_hi) in batches:
                        k0 = cb + b0
                        gt = gp.tile([128, GBATCH * grow], F32, tag="g")
                        tab_ap = table[half:vpad, :] if in_hi else table[0:half, :]
                        nc.gpsimd.dma_gather(
                            gt[:, 0:gl * grow].rearrange("p (k c) -> p k c", c=grow),
                            tab_ap,
                            idx_t[:, k0 * 8:(k0 + gl) * 8],
                            gl * 128, gl * 128, grow,
                            single_packet=False, queue_num=qn[0],
                        )
                        qn[0] = 0  # multi-SWDGE-queue rotation crashes under Tile; keep queue 0
                        if DEBUG and layer == 1 and j == 0 and b0 == 0:
                            nc.sync.dma_start(out=dbg_g[:, 0:gl * grow],
                                              in_=gt[:, 0:gl * grow])
                        gv = gt[:, 0:gl * grow].rearrange("p (k c) -> p k c", c=grow)
                        z = wp.tile([128, GBATCH * heads], F32, tag="z")
                        zv = z[:, 0:gl * heads].rearrange(
                            "p (k h) -> p k h", h=heads)
                        vt = vp.tile([128, GBATCH * vcols], F32, tag="v")
                        vv = vt[:, 0:gl * vcols].rearrange(
                            "p (k c) -> p k c", c=vcols)
                        for h in range(heads):
                            nc.scalar.activation(
                                zv[:, :, h], gv[:, :, heads * hdim + h],
                                ACTF.Prelu, bias=adw[:, h:h + 1],
                                alpha=NEG_SLOPE)
                            nc.scalar.activation(zv[:, :, h], zv[:, :, h],
                                                 ACTF.Exp)
                        nc.vector.tensor_tensor(
                            out=vv[:, :, heads * hdim:vcols],
                            in0=zv[:, :, :],
                            in1=msk_t[:, k0:k0 + gl].to_broadcast(
                                [128, gl, heads]),
                            op=ALU.mult)
                        for h in range(heads):
                            nc.vector.tensor_tensor(
                                out=vv[:, :, h * hdim:(h + 1) * hdim],
                                in0=gv[:, :, h * hdim:(h + 1) * hdim],
                                in1=vv[:, :, heads * hdim + h:heads * hdim + h + 1]
                                    .to_broadcast([128, gl, hdim]),
                                op=ALU.mult)
                        for k in range(gl):
                            nc.tensor.matmul(
                                psum[:], lhsT=ident[:],
                                rhs=vt[:, k * vcols:(k + 1) * vcols],
                                start=(b0 + k == 0),
                                stop=(b0 + k == kj - 1))

                    # epilogue
                    if DEBUG and layer == 1:
                        pscp = ep.tile([128, V1C], F32, tag="pscp")
                        nc.scalar.activation(pscp[:], psum[:], ACTF.Copy)
                        nc.sync.dma_start(
                            out=dbg_ps[j * 128:(j + 1) * 128, :], in_=pscp[:])
                    # +1e-30 is exact for any real den (>=0.3) but keeps
                    # all-padding (dummy) lanes finite: 0 -> 1e30 -> 0*1e30=0,
                    # so no NaN ever enters a later matmul (0*NaN = NaN would
                    # poison every lane of the PSUM accumulation).
                    dsafe = wp.tile([128, heads], F32, tag="dsafe")
                    nc.vector.tensor_scalar_add(dsafe[:],
                                                psum[:, heads * hdim:vcols],
                                                1e-30)
                    rden = wp.tile([128, heads], F32, tag="rden")
                    nc.vector.reciprocal(rden[:], dsafe[:])
                    if layer == 1:
                        h2pre = ep.tile([128, 128], F32, tag="h2pre")
                        for h in range(heads):
                            nc.scalar.activation(
                                h2pre[:, h * hdim:(h + 1) * hdim],
                                psum[:, h * hdim:(h + 1) * hdim],
                                ACTF.Relu, scale=rden[:, h:h + 1])
                        tps = pst.tile([128, 128], F32, tag="tps")
                        nc.tensor.transpose(out=tps[:], in_=h2pre[:],
                                            identity=ident[:])
                        h2t = ep.tile([128, 128], F32, tag="h2t")
                        nc.scalar.activation(h2t[:], tps[:], ACTF.Copy)
                        ps3 = psh.tile([128, 66], F32, tag="psh")
                        nc.tensor.matmul(ps3[:], lhsT=h2t[:], rhs=w2_sb[:],
                                         start=True, stop=True)
                        h2row = hp.tile([128, TROW2], F32, tag="hrow")
                        nc.vector.tensor_tensor(out=h2row[:, 0:64],
                                                in0=ps3[:, 0:64], in1=b2_sb[:],
                                                op=ALU.add)
                        nc.scalar.activation(h2row[:, 64:65], ps3[:, 64:65],
                                             ACTF.Copy)
                        a2row = hp.tile([128, 1], F32, tag="arow")
                        nc.scalar.activation(a2row[:], ps3[:, 65:66], ACTF.Copy)
                        nc.sync.dma_start(
                            out=h2_loc[j * 128:(j + 1) * 128, 0:65],
                            in_=h2row[:, 0:65])
                        if DEBUG:
                            nc.sync.dma_start(
                                out=dbg_h2[j * 128:(j + 1) * 128, 0:65],
                                in_=h2row[:, 0:65])
                        nc.sync.dma_start(out=a2_loc[j * 128:(j + 1) * 128, :],
                                          in_=a2row[:])
                    else:
                        ob = ep.tile([128, OUT], F32, tag="ob")
                        nc.scalar.activation(ob[:], psum[:, 0:OUT],
                                             ACTF.Sigmoid, scale=rden[:, 0:1])
                        nc.sync.dma_start(out=out_d[j * 128:(j + 1) * 128, :],
                                          in_=ob[:])

            agg_layer(1)
            nc.gpsimd.collective_compute(
                "AllGather", mybir.AluOpType.bypass,
                replica_groups=[list(range(NCORES))],
                ins=[h2_loc[:]], outs=[h2_full[:]],
            )
            agg_layer(2)

    nc.finalize()
    return nc


def kernel(x, edge_index, W1, att_src1, att_dst1, b1, W2, att_src2, att_dst2,
           b2):
    from concourse import bass_utils

    x = np.asarray(x, np.float32)
    W1 = np.asarray(W1, np.float32)
    W2 = np.asarray(W2, np.float32)
    att_src1 = np.asarray(att_src1, np.float32)
    att_dst1 = np.asarray(att_dst1, np.float32)
    att_src2 = np.asarray(att_src2, np.float32)
    att_dst2 = np.asarray(att_dst2, np.float32)
    b1 = np.asarray(b1, np.float32)
    b2 = np.asarray(b2, np.float32)
    n_nodes = x.shape[0]

    sch = _build_schedule(edge_index, n_nodes)
    vpad, pc = sch["vpad"], sch["pc"]

    W1r = W1.reshape(F_IN, HEADS, HID)
    w1_aug = np.zeros((F_IN, 132), np.float32)
    w1_aug[:, 0:HEADS * HID] = W1
    for h in range(HEADS):
        w1_aug[:, HEADS * HID + h] = W1r[:, h, :] @ att_src1[h]
        w1_aug[:, HEADS * HID + HEADS + h] = W1r[:, h, :] @ att_dst1[h]
    w2_aug = np.zeros((HEADS * HID, 66), np.float32)
    w2_aug[:, 0:OUT] = W2
    w2_aug[:, OUT] = W2 @ att_src2[0]
    w2_aug[:, OUT + 1] = W2 @ att_dst2[0]
    b1_rep = np.broadcast_to(b1, (128, HEADS * HID)).copy()
    b2_rep = np.broadcast_to(b2, (128, OUT)).copy()

    x_rho = np.zeros((vpad, F_IN), np.float32)
    x_rho[sch["row_of_node"]] = x

    key = (vpad, sch["nch"], tuple(sch["K"].tolist()),
           tuple(sch["K_lo"].tolist()), DEBUG)
    if key not in _cache:
        _cache[key] = _build_program(vpad, pc, sch["nb"], sch["half"],
                                     sch["K"], sch["K_lo"], sch["K_hi"],
                                     sch["nch"], sch["chunk_base"])
    nc = _cache[key]

    in_maps = []
    for c in range(NCORES):
        in_maps.append({
            "xt": np.ascontiguousarray(x_rho[c * pc:(c + 1) * pc].T),
            "idx": sch["idx_wrapped"][c],
            "msk": sch["mask_stream"][c],
            "w1aug": w1_aug,
            "w2aug": w2_aug,
            "b1rep": b1_rep,
            "b2rep": b2_rep,
        })
    res = bass_utils.run_bass_kernel_spmd(nc, in_maps,
                                          core_ids=list(range(NCORES)),
                                          trace=TRACE)
    kernel.last_exec_ns = res.exec_time_ns
    kernel.last_mean_ns = res.mean_exec_time_ns
    out_all = np.concatenate([res.results[c]["out"] for c in range(NCORES)], 0)
    if DEBUG:
        kernel.dbg = {
            k: np.concatenate([res.results[c][k] for c in range(NCORES)], 0)
            for k in ("dbg_h", "dbg_a1", "dbg_h2", "dbg_ps", "dbg_g")}
        kernel.sch = sch
    return out_all[sch["row_of_node"][:n_nodes]]



# revision 2
# speedup vs baseline: 1.0611x; 1.0611x over previous
"""Two-layer GAT on 8 Trainium2 NeuronCores (Bass/Tile).

Host (numpy): append self-loops, degree-sort nodes (desc), pad node count to
VPAD (multiple of 8*128) and assign sorted nodes round-robin at 128-node
block granularity to the 8 cores (sorted-rank s -> block g=s//128,
lane=s%128 -> core c=g%8, local block j=g//8, table row = c*PC+j*128+lane).
Per block-rank j the chunk schedule is shared by all cores (SPMD: one
program, per-core tensor data).  Each dst node's edges occupy "slots"; a
chunk is slot k of all 128 lanes of a block.  Edge slots are split into a
lo-region (src table row < VPAD/2) and hi-region so every chunk's gather
indices fit in int16 after rebasing (dma_gather is int16-indexed).
Attention vectors are folded into the feature matmul (W_aug = [W | W@a_src |
W@a_dst]) and the bias into the gather table (h+b), which adds b exactly
after softmax normalization.

Device, per core (Tile): h-phase matmuls build the gather table
[h+b | alpha_src] and the local alpha_dst column; AllGather replicates the
table; aggregation gathers source rows with multi-chunk dma_gather (4 SWDGE
queues), computes w = exp(prelu(asrc+adst, 0.2)) * pad_mask on ACT/DVE,
builds V = [w*h | w] on DVE, and segment-sums via identity-stationary
matmuls accumulating in PSUM; epilogue fuses out = act(num * (1/den)) on ACT
(scale = per-lane reciprocal), with the layer-2 h-phase inline per block.
The final sigmoid rows are written per block; the host undoes the row
permutation.
"""

import numpy as np

NCORES = 8
F_IN = 128
HID = 64
HEADS = 2
OUT = 64
NEG_SLOPE = 0.2

TROW1 = 192  # L1 table row floats: [h+b1 (128) | asrc (2) | pad] (768B)
TROW2 = 128  # L2 table row floats: [h2+b2 (64) | asrc2 (1) | pad] (512B)
V1C = 130    # L1 V cols: [w0*h0 | w1*h1 | w0 | w1]
V2C = 65     # L2 V cols: [w*h2 | w]
GBATCH = 32  # max chunks per dma_gather

DEBUG = False
TRACE = False
_cache = {}


def _build_schedule(edge_index, n_nodes):
    ei = np.asarray(edge_index).astype(np.int64)
    src = np.concatenate([ei[0], np.arange(n_nodes, dtype=np.int64)])
    dst = np.concatenate([ei[1], np.arange(n_nodes, dtype=np.int64)])
    deg = np.bincount(dst, minlength=n_nodes)

    stripe = NCORES * 128
    vpad = ((n_nodes + stripe - 1) // stripe) * stripe
    pc = vpad // NCORES
    nb = pc // 128
    half = vpad // 2
    assert half <= 32768

    degp = np.zeros(vpad, np.int64)
    degp[:n_nodes] = deg
    order = np.argsort(-degp, kind="stable")
    rank = np.empty(vpad, np.int64)
    rank[order] = np.arange(vpad)

    s = np.arange(vpad)
    g = s // 128
    lane = s % 128
    row_of_rank = (g % NCORES) * pc + (g // NCORES) * 128 + lane
    row_of_node = row_of_rank[rank[:n_nodes]]

    e_dstrow = row_of_node[dst]
    e_srcrow = row_of_node[src]
    is_hi = e_srcrow >= half

    # per-(dstrow, lo/hi) counts
    lo_cnt = np.bincount(e_dstrow[~is_hi], minlength=vpad)
    hi_cnt = np.bincount(e_dstrow[is_hi], minlength=vpad)

    # per-block-rank shared chunk counts: max over the 8 cores' j-th blocks
    # block of table row r: j = (r % pc) // 128
    jj = (np.arange(vpad) % pc) // 128
    K_lo = np.zeros(nb, np.int64)
    K_hi = np.zeros(nb, np.int64)
    np.maximum.at(K_lo, jj, lo_cnt)
    np.maximum.at(K_hi, jj, hi_cnt)
    K = K_lo + K_hi
    bump = K == 0
    K_lo[bump] += 1
    K[bump] += 1
    nch = int(K.sum())
    chunk_base = np.concatenate([[0], np.cumsum(K)])[:-1]

    # slot assignment: edges of a dst grouped, lo first then hi
    key = e_dstrow * 2 + is_hi
    ord_e = np.argsort(key, kind="stable")
    ds = e_dstrow[ord_e]
    hs = is_hi[ord_e]
    first = np.r_[True, ds[1:] != ds[:-1]]
    grp_start = np.flatnonzero(first)
    grp_id = np.cumsum(first) - 1
    slot = np.arange(ds.shape[0]) - grp_start[grp_id]  # 0.. within dst (lo+hi)
    c = ds // pc
    j = (ds % pc) // 128
    ln = ds % 128
    pos = chunk_base[j] + np.where(hs, K_lo[j] + slot - lo_cnt[ds], slot)
    assert (pos >= chunk_base[j]).all() and (pos < chunk_base[j] + K[j]).all()

    idx_val = np.where(hs, e_srcrow[ord_e] - half, e_srcrow[ord_e])
    idx_stream = np.zeros((NCORES, 128, nch), np.int16)
    mask_stream = np.zeros((NCORES, 128, nch), np.float32)
    idx_stream[c, ln, pos] = idx_val.astype(np.int16)
    mask_stream[c, ln, pos] = 1.0

    # wrapped int16 layout for dma_gather: chunk k -> columns 8k:8k+8 of
    # [128, 8*nch]; within a chunk the 128 lane-indices are wrapped as
    # flat[i] -> [i % 16, i // 16] and replicated over the 8 16-partition
    # groups.
    iw = idx_stream.transpose(0, 2, 1).reshape(NCORES, nch, 8, 16)
    iw = iw.transpose(0, 3, 1, 2).reshape(NCORES, 16, nch * 8)
    idx_wrapped = np.tile(iw, (1, 8, 1))  # [NCORES, 128, nch*8]

    return dict(vpad=vpad, pc=pc, nb=nb, half=half, K=K, K_lo=K_lo, K_hi=K_hi,
                nch=nch, chunk_base=chunk_base, row_of_node=row_of_node,
                idx_wrapped=np.ascontiguousarray(idx_wrapped),
                mask_stream=mask_stream)


def _build_program(vpad, pc, nb, half, K, K_lo, K_hi, nch, chunk_base):
    import concourse.bacc as bacc
    import concourse.bass as bass
    import concourse.mybir as mybir
    import concourse.tile as tile
    from concourse.masks import make_identity

    F32 = mybir.dt.float32
    I16 = mybir.dt.int16
    ACTF = mybir.ActivationFunctionType
    ALU = mybir.AluOpType

    nc = bacc.Bacc("TRN2", target_bir_lowering=False, debug=False,
                   num_devices=NCORES, num_swdge_queues=4)

    xt_d = nc.dram_tensor("xt", [128, pc], F32, kind="ExternalInput")
    idx_d = nc.dram_tensor("idx", [128, nch * 8], I16, kind="ExternalInput")
    msk_d = nc.dram_tensor("msk", [128, nch], F32, kind="ExternalInput")
    w1_d = nc.dram_tensor("w1aug", [128, 132], F32, kind="ExternalInput")
    w2_d = nc.dram_tensor("w2aug", [128, 66], F32, kind="ExternalInput")
    b1_d = nc.dram_tensor("b1rep", [128, 128], F32, kind="ExternalInput")
    b2_d = nc.dram_tensor("b2rep", [128, 64], F32, kind="ExternalInput")
    out_d = nc.dram_tensor("out", [pc, OUT], F32, kind="ExternalOutput")
    if DEBUG:
        dbg_h = nc.dram_tensor("dbg_h", [pc, TROW1], F32, kind="ExternalOutput")
        dbg_a1 = nc.dram_tensor("dbg_a1", [pc, 2], F32, kind="ExternalOutput")
        dbg_h2 = nc.dram_tensor("dbg_h2", [pc, TROW2], F32, kind="ExternalOutput")
        dbg_ps = nc.dram_tensor("dbg_ps", [pc, V1C], F32, kind="ExternalOutput")
        dbg_g = nc.dram_tensor("dbg_g", [128, GBATCH * TROW1], F32, kind="ExternalOutput")

    qn = [0]

    with tile.TileContext(nc) as tc:
        with (
            tc.tile_pool(name="const", bufs=1) as cp,
            tc.tile_pool(name="dram", bufs=1, space="DRAM") as dp,
            tc.tile_pool(name="xs", bufs=3) as xp,
            tc.tile_pool(name="hrow", bufs=3) as hp,
            tc.tile_pool(name="psh", bufs=2, space="PSUM") as psh,
            tc.tile_pool(name="g", bufs=4) as gp,
            tc.tile_pool(name="v", bufs=3) as vp,
            tc.tile_pool(name="wz", bufs=4) as wp,
            tc.tile_pool(name="psa", bufs=5, space="PSUM") as psa,
            tc.tile_pool(name="pst", bufs=1, space="PSUM") as pst,
            tc.tile_pool(name="epi", bufs=3) as ep,
        ):
            ident = cp.tile([128, 128], F32)
            make_identity(nc, ident[:])
            w1_sb = cp.tile([128, 132], F32)
            w2_sb = cp.tile([128, 66], F32)
            b1_sb = cp.tile([128, 128], F32)
            b2_sb = cp.tile([128, 64], F32)
            idx_t = cp.tile([128, nch * 8], I16)
            msk_t = cp.tile([128, nch], F32)
            for t, d in ((w1_sb, w1_d), (w2_sb, w2_d), (b1_sb, b1_d),
                         (b2_sb, b2_d), (idx_t, idx_d), (msk_t, msk_d)):
                nc.sync.dma_start(out=t[:], in_=d[:])

            h_loc = dp.tile([pc, TROW1], F32)
            h_full = dp.tile([vpad, TROW1], F32)
            a1_loc = dp.tile([pc, 2], F32)
            h2_loc = dp.tile([pc, TROW2], F32)
            h2_full = dp.tile([vpad, TROW2], F32)
            a2_loc = dp.tile([pc, 1], F32)

            # ---- Phase 1: L1 h-phase ----
            for j in range(nb):
                xt_sb = xp.tile([128, 128], F32, tag="xt")
                nc.sync.dma_start(out=xt_sb[:], in_=xt_d[:, j * 128:(j + 1) * 128])
                ps = psh.tile([128, 132], F32, tag="psh")
                nc.tensor.matmul(ps[:], lhsT=xt_sb[:], rhs=w1_sb[:],
                                 start=True, stop=True)
                hrow = hp.tile([128, TROW1], F32, tag="hrow")
                nc.vector.tensor_tensor(out=hrow[:, 0:128], in0=ps[:, 0:128],
                                        in1=b1_sb[:], op=ALU.add)
                nc.scalar.activation(hrow[:, 128:130], ps[:, 128:130], ACTF.Copy)
                arow = hp.tile([128, 2], F32, tag="arow")
                nc.scalar.activation(arow[:], ps[:, 130:132], ACTF.Copy)
                nc.sync.dma_start(out=h_loc[j * 128:(j + 1) * 128, 0:130],
                                  in_=hrow[:, 0:130])
                nc.sync.dma_start(out=a1_loc[j * 128:(j + 1) * 128, :], in_=arow[:])
                if DEBUG:
                    nc.sync.dma_start(out=dbg_h[j * 128:(j + 1) * 128, 0:130],
                                      in_=hrow[:, 0:130])
                    nc.sync.dma_start(out=dbg_a1[j * 128:(j + 1) * 128, :],
                                      in_=arow[:])

            # ---- Phase 2: AllGather L1 table ----
            nc.gpsimd.collective_compute(
                "AllGather", mybir.AluOpType.bypass,
                replica_groups=[list(range(NCORES))],
                ins=[h_loc[:]], outs=[h_full[:]],
            )

            def agg_layer(layer):
                if layer == 1:
                    table, arows, grow, vcols, heads = h_full, a1_loc, TROW1, V1C, 2
                else:
                    table, arows, grow, vcols, heads = h2_full, a2_loc, TROW2, V2C, 1
                hdim = (vcols - heads) // heads
                for j in range(nb):
                    adw = wp.tile([128, heads], F32, tag="adw")
                    nc.sync.dma_start(out=adw[:],
                                      in_=arows[j * 128:(j + 1) * 128, :])
                    psum = psa.tile([128, vcols], F32, tag="psa")
                    kj = int(K[j])
                    cb = int(chunk_base[j])
                    # batches: lo region then hi region, each split to <=GBATCH
                    batches = []
                    done = 0
                    for rl in (int(K_lo[j]), int(K_hi[j])):
                        r0 = done
                        while done < r0 + rl:
                            gl = min(GBATCH, r0 + rl - done)
                            batches.append((done, gl, done >= int(K_lo[j])))
                            done += gl
                    for (b0, gl, in_hi) in batches:
                        k0 = cb + b0
                        gt = gp.tile([128, GBATCH * grow], F32, tag="g")
                        tab_ap = table[half:vpad, :] if in_hi else table[0:half, :]
                        nc.gpsimd.dma_gather(
                            gt[:, 0:gl * grow].rearrange("p (k c) -> p k c", c=grow),
                            tab_ap,
                            idx_t[:, k0 * 8:(k0 + gl) * 8],
                            gl * 128, gl * 128, grow,
                            single_packet=False, queue_num=qn[0],
                        )
                        qn[0] = (qn[0] + 1) % 4
                        if DEBUG and layer == 1 and j == 0 and b0 == 0:
                            nc.sync.dma_start(out=dbg_g[:, 0:gl * grow],
                                              in_=gt[:, 0:gl * grow])
                        gv = gt[:, 0:gl * grow].rearrange("p (k c) -> p k c", c=grow)
                        z = wp.tile([128, GBATCH * heads], F32, tag="z")
                        zv = z[:, 0:gl * heads].rearrange(
                            "p (k h) -> p k h", h=heads)
                        vt = vp.tile([128, GBATCH * vcols], F32, tag="v")
                        vv = vt[:, 0:gl * vcols].rearrange(
                            "p (k c) -> p k c", c=vcols)
                        for h in range(heads):
                            nc.scalar.activation(
                                zv[:, :, h], gv[:, :, heads * hdim + h],
                                ACTF.Prelu, bias=adw[:, h:h + 1],
                                alpha=NEG_SLOPE)
                            nc.scalar.activation(zv[:, :, h], zv[:, :, h],
                                                 ACTF.Exp)
                        nc.vector.tensor_tensor(
                            out=vv[:, :, heads * hdim:vcols],
                            in0=zv[:, :, :],
                            in1=msk_t[:, k0:k0 + gl].to_broadcast(
                                [128, gl, heads]),
                            op=ALU.mult)
                        for h in range(heads):
                            nc.vector.tensor_tensor(
                                out=vv[:, :, h * hdim:(h + 1) * hdim],
                                in0=gv[:, :, h * hdim:(h + 1) * hdim],
                                in1=vv[:, :, heads * hdim + h:heads * hdim + h + 1]
                                    .to_broadcast([128, gl, hdim]),
                                op=ALU.mult)
                        for k in range(gl):
                            nc.tensor.matmul(
                                psum[:], lhsT=ident[:],
                                rhs=vt[:, k * vcols:(k + 1) * vcols],
                                start=(b0 + k == 0),
                                stop=(b0 + k == kj - 1))

                    # epilogue
                    if DEBUG and layer == 1:
                        pscp = ep.tile([128, V1C], F32, tag="pscp")
                        nc.scalar.activation(pscp[:], psum[:], ACTF.Copy)
                        nc.sync.dma_start(
                            out=dbg_ps[j * 128:(j + 1) * 128, :], in_=pscp[:])
                    # +1e-30 is exact for any real den (>=0.3) but keeps
                    # all-padding (dummy) lanes finite: 0 -> 1e30 -> 0*1e30=0,
                    # so no NaN ever enters a later matmul (0*NaN = NaN would
                    # poison every lane of the PSUM accumulation).
                    dsafe = wp.tile([128, heads], F32, tag="dsafe")
                    nc.vector.tensor_scalar_add(dsafe[:],
                                                psum[:, heads * hdim:vcols],
                                                1e-30)
                    rden = wp.tile([128, heads], F32, tag="rden")
                    nc.vector.reciprocal(rden[:], dsafe[:])
                    if layer == 1:
                        h2pre = ep.tile([128, 128], F32, tag="h2pre")
                        for h in range(heads):
                            nc.scalar.activation(
                                h2pre[:, h * hdim:(h + 1) * hdim],
                                psum[:, h * hdim:(h + 1) * hdim],
                                ACTF.Relu, scale=rden[:, h:h + 1])
                        tps = pst.tile([128, 128], F32, tag="tps")
                        nc.tensor.transpose(out=tps[:], in_=h2pre[:],
                                            identity=ident[:])
                        h2t = ep.tile([128, 128], F32, tag="h2t")
                        nc.scalar.activation(h2t[:], tps[:], ACTF.Copy)
                        ps3 = psh.tile([128, 66], F32, tag="psh")
                        nc.tensor.matmul(ps3[:], lhsT=h2t[:], rhs=w2_sb[:],
                                         start=True, stop=True)
                        h2row = hp.tile([128, TROW2], F32, tag="hrow")
                        nc.vector.tensor_tensor(out=h2row[:, 0:64],
                                                in0=ps3[:, 0:64], in1=b2_sb[:],
                                                op=ALU.add)
                        nc.scalar.activation(h2row[:, 64:65], ps3[:, 64:65],
                                             ACTF.Copy)
                        a2row = hp.tile([128, 1], F32, tag="arow")
                        nc.scalar.activation(a2row[:], ps3[:, 65:66], ACTF.Copy)
                        nc.sync.dma_start(
                            out=h2_loc[j * 128:(j + 1) * 128, 0:65],
                            in_=h2row[:, 0:65])
                        if DEBUG:
                            nc.sync.dma_start(
                                out=dbg_h2[j * 128:(j + 1) * 128, 0:65],
                                in_=h2row[:, 0:65])
                        nc.sync.dma_start(out=a2_loc[j * 128:(j + 1) * 128, :],
                                          in_=a2row[:])
                    else:
                        ob = ep.tile([128, OUT], F32, tag="ob")
                        nc.scalar.activation(ob[:], psum[:, 0:OUT],
                                             ACTF.Sigmoid, scale=rden[:, 0:1])
                        nc.sync.dma_start(out=out_d[j * 128:(j + 1) * 128, :],
                                          in_=ob[:])

            agg_layer(1)
            nc.gpsimd.collective_compute(
                "AllGather", mybir.AluOpType.bypass,
                replica_groups=[list(range(NCORES))],
                ins=[h2_loc[:]], outs=[h2_full[:]],
            )
            agg_layer(2)

    nc.finalize()
    return nc


def kernel(x, edge_index, W1, att_src1, att_dst1, b1, W2, att_src2, att_dst2,
           b2):
    from concourse import bass_utils

    x = np.asarray(x, np.float32)
    W1 = np.asarray(W1, np.float32)
    W2 = np.asarray(W2, np.float32)
    att_src1 = np.asarray(att_src1, np.float32)
    att_dst1 = np.asarray(att_dst1, np.float32)
    att_src2 = np.asarray(att_src2, np.float32)
    att_dst2 = np.asarray(att_dst2, np.float32)
    b1 = np.asarray(b1, np.float32)
    b2 = np.asarray(b2, np.float32)
    n_nodes = x.shape[0]

    sch = _build_schedule(edge_index, n_nodes)
    vpad, pc = sch["vpad"], sch["pc"]

    W1r = W1.reshape(F_IN, HEADS, HID)
    w1_aug = np.zeros((F_IN, 132), np.float32)
    w1_aug[:, 0:HEADS * HID] = W1
    for h in range(HEADS):
        w1_aug[:, HEADS * HID + h] = W1r[:, h, :] @ att_src1[h]
        w1_aug[:, HEADS * HID + HEADS + h] = W1r[:, h, :] @ att_dst1[h]
    w2_aug = np.zeros((HEADS * HID, 66), np.float32)
    w2_aug[:, 0:OUT] = W2
    w2_aug[:, OUT] = W2 @ att_src2[0]
    w2_aug[:, OUT + 1] = W2 @ att_dst2[0]
    b1_rep = np.broadcast_to(b1, (128, HEADS * HID)).copy()
    b2_rep = np.broadcast_to(b2, (128, OUT)).copy()

    x_rho = np.zeros((vpad, F_IN), np.float32)
    x_rho[sch["row_of_node"]] = x

    key = (vpad, sch["nch"], tuple(sch["K"].tolist()),
           tuple(sch["K_lo"].tolist()), DEBUG)
    if key not in _cache:
        _cache[key] = _build_program(vpad, pc, sch["nb"], sch["half"],
                                     sch["K"], sch["K_lo"], sch["K_hi"],
                                     sch["nch"], sch["chunk_base"])
    nc = _cache[key]

    in_maps = []
    for c in range(NCORES):
        in_maps.append({
            "xt": np.ascontiguousarray(x_rho[c * pc:(c + 1) * pc].T),
            "idx": sch["idx_wrapped"][c],
            "msk": sch["mask_stream"][c],
            "w1aug": w1_aug,
            "w2aug": w2_aug,
            "b1rep": b1_rep,
            "b2rep": b2_rep,
        })
    res = bass_utils.run_bass_kernel_spmd(nc, in_maps,
                                          core_ids=list(range(NCORES)),
                                          trace=TRACE)
    kernel.last_exec_ns = res.exec_time_ns
    kernel.last_mean_ns = res.mean_exec_time_ns
    out_all = np.concatenate([res.results[c]["out"] for c in range(NCORES)], 0)
    if DEBUG:
        kernel.dbg = {
            k: np.concatenate([res.results[c][k] for c in range(NCORES)], 0)
            for k in ("dbg_h", "dbg_a1", "dbg_h2", "dbg_ps", "dbg_g")}
        kernel.sch = sch
    return out_all[sch["row_of_node"][:n_nodes]]



# revision 6
# speedup vs baseline: 1.3620x; 1.2836x over previous
"""Two-layer GAT on 8 Trainium2 NeuronCores (Bass/Tile) — v3.

The per-edge gather tables live in SBUF as bf16 rows of 264B
([h (128) | alpha_src (1-2) | pad], 132 bf16) and are read with
non-transpose SBUF-source dma_gather on 4 rotating SWDGE queues: plain
SBUF->SBUF 264B descriptor pairs (no HBM latency, no XBAR), with descriptor
generation spread over all 8 Q7 cores.  The bass wrapper only allows
transpose=True for SBUF sources, so the instruction is constructed directly
(the non-transpose SBUF path exists in the ucode and is exact).  L1 table
rows are [x@W1 | x@W1@a_src] computed per-core from the replicated x (no
AllGather); L2 rows are [relu(out1) | relu(out1)@W2@a_src2] (W2 is applied
AFTER aggregation by linearity), so the only collective is a 1.7MB/core bf16
AllGather.  The slot-chunk schedule, lane-major V build, softmax by
reciprocal and identity-stationary segment-sum matmuls are from v1.
"""

import numpy as np

NCORES = 8
F_IN = 128
HID = 64
HEADS = 2
OUT = 64
NEG_SLOPE = 0.2
GBATCH = 8    # chunks per dma_gather call
TROW = 132    # table row: 132 bf16 = 264B

DEBUG = False
TRACE = False
_cache = {}


def _bf16(x):
    u = np.asarray(x, np.float32).view(np.uint32)
    r = ((u >> 16) & 1) + 0x7FFF
    return ((u + r) >> 16).astype(np.uint16)


def _build_schedule(edge_index, n_nodes):
    ei = np.asarray(edge_index).astype(np.int64)
    src = np.concatenate([ei[0], np.arange(n_nodes, dtype=np.int64)])
    dst = np.concatenate([ei[1], np.arange(n_nodes, dtype=np.int64)])
    deg = np.bincount(dst, minlength=n_nodes)

    stripe = NCORES * 128
    vpad = ((n_nodes + stripe - 1) // stripe) * stripe
    pc = vpad // NCORES
    nb = pc // 128
    half = vpad // 2
    assert half <= 32768

    degp = np.zeros(vpad, np.int64)
    degp[:n_nodes] = deg
    order = np.argsort(-degp, kind="stable")
    rank = np.empty(vpad, np.int64)
    rank[order] = np.arange(vpad)

    s = np.arange(vpad)
    g = s // 128
    lane = s % 128
    row_of_rank = (g % NCORES) * pc + (g // NCORES) * 128 + lane
    row_of_node = row_of_rank[rank[:n_nodes]]

    e_dstrow = row_of_node[dst]
    e_srcrow = row_of_node[src]
    is_hi = e_srcrow >= half

    lo_cnt = np.bincount(e_dstrow[~is_hi], minlength=vpad)
    hi_cnt = np.bincount(e_dstrow[is_hi], minlength=vpad)

    jj = (np.arange(vpad) % pc) // 128
    K_lo = np.zeros(nb, np.int64)
    K_hi = np.zeros(nb, np.int64)
    np.maximum.at(K_lo, jj, lo_cnt)
    np.maximum.at(K_hi, jj, hi_cnt)
    K = K_lo + K_hi
    bump = K == 0
    K_lo[bump] += 1
    K[bump] += 1
    nch = int(K.sum())
    chunk_base = np.concatenate([[0], np.cumsum(K)])[:-1]

    key = e_dstrow * 2 + is_hi
    ord_e = np.argsort(key, kind="stable")
    ds = e_dstrow[ord_e]
    hs = is_hi[ord_e]
    first = np.r_[True, ds[1:] != ds[:-1]]
    grp_start = np.flatnonzero(first)
    grp_id = np.cumsum(first) - 1
    slot = np.arange(ds.shape[0]) - grp_start[grp_id]
    c = ds // pc
    j = (ds % pc) // 128
    ln = ds % 128
    pos = chunk_base[j] + np.where(hs, K_lo[j] + slot - lo_cnt[ds], slot)
    assert (pos >= chunk_base[j]).all() and (pos < chunk_base[j] + K[j]).all()

    idx_val = np.where(hs, e_srcrow[ord_e] - half, e_srcrow[ord_e])
    idx_stream = np.zeros((NCORES, 128, nch), np.int16)
    mask_stream = np.zeros((NCORES, 128, nch), np.float32)
    idx_stream[c, ln, pos] = idx_val.astype(np.int16)
    mask_stream[c, ln, pos] = 1.0

    # wrapped int16 layout: chunk k -> columns 8k:8k+8; within a chunk the
    # 128 lane-indices wrap as flat[i] -> [i % 16, i // 16], replicated over
    # the 8 16-partition groups (each SWDGE queue's core-pair reads its own).
    iw = idx_stream.transpose(0, 2, 1).reshape(NCORES, nch, 8, 16)
    iw = iw.transpose(0, 3, 1, 2).reshape(NCORES, 16, nch * 8)
    idx_wrapped = np.tile(iw, (1, 8, 1))

    return dict(vpad=vpad, pc=pc, nb=nb, half=half, K=K, K_lo=K_lo, K_hi=K_hi,
                nch=nch, chunk_base=chunk_base, row_of_node=row_of_node,
                idx_wrapped=np.ascontiguousarray(idx_wrapped),
                mask_stream=mask_stream)


def _sbuf_gather_rows(nc, out_ap, in_ap, idxs_ap, num_idxs, queue_num):
    """Non-transpose SBUF-source dma_gather (row layout, 264B elements).

    The bass wrapper restricts SBUF sources to transpose=True; the ucode's
    non-transpose src_is_sbuf path is complete and exact, so build the
    instruction directly.  Row r of the table sits at partition r%128,
    byte offset (r//128)*2*TROW of in_ap.
    """
    import concourse.mybir as mybir
    gp = nc.gpsimd
    return gp.add_instruction(
        mybir.InstDMAGatherAnt(
            name=nc.get_next_instruction_name(),
            ins=[
                gp.lower_ap(in_ap),
                gp.lower_ap(idxs_ap),
                gp.lower_val_access(gp.to_reg(num_idxs)),
            ],
            outs=[gp.lower_ap(out_ap)],
            transpose=False,
            num_idxs=num_idxs,
            elem_size=TROW,
            stride_bytes_256=0,
            gen_mode=0,
            single_packet=False,
            queue_num=queue_num,
            sbuf_tokens_per_rank=128,
            sbuf_free_dim_per_rank=2 * TROW,
            sbuf_free_dim_pad_per_rank=0,
            sbuf_byte_offset=0,
        )
    )


def _build_program(vpad, pc, nb, half, K, K_lo, K_hi, nch, chunk_base):
    import concourse.bacc as bacc
    import concourse.mybir as mybir
    import concourse.tile as tile

    F32 = mybir.dt.float32
    BF16 = mybir.dt.bfloat16
    I16 = mybir.dt.int16
    ACTF = mybir.ActivationFunctionType
    ALU = mybir.AluOpType
    AXL = mybir.AxisListType

    nstr = vpad // 128
    hstr = half // 128

    nc = bacc.Bacc("TRN2", target_bir_lowering=False, debug=False,
                   num_devices=NCORES, num_swdge_queues=4)

    xt_d = nc.dram_tensor("xt", [128, vpad], BF16, kind="ExternalInput")
    xtl_d = nc.dram_tensor("xtl", [128, pc], BF16, kind="ExternalInput")
    idx_d = nc.dram_tensor("idx", [128, nch * 8], I16, kind="ExternalInput")
    msk_d = nc.dram_tensor("msk", [128, nch], BF16, kind="ExternalInput")
    id_d = nc.dram_tensor("ident", [128, 128], BF16, kind="ExternalInput")
    w1_d = nc.dram_tensor("w1aug", [128, TROW], BF16, kind="ExternalInput")
    wad1_d = nc.dram_tensor("wad1", [128, 2], BF16, kind="ExternalInput")
    w2_d = nc.dram_tensor("w2", [128, 64], BF16, kind="ExternalInput")
    ws2_d = nc.dram_tensor("ws2", [128, 128], BF16, kind="ExternalInput")
    wd2_d = nc.dram_tensor("wd2", [128, 128], BF16, kind="ExternalInput")
    b1_d = nc.dram_tensor("b1rep", [128, 128], F32, kind="ExternalInput")
    b2_d = nc.dram_tensor("b2rep", [128, 64], F32, kind="ExternalInput")
    out_d = nc.dram_tensor("out", [pc, OUT], F32, kind="ExternalOutput")

    qn = [0]

    with tile.TileContext(nc) as tc:
        with (
            tc.tile_pool(name="const", bufs=1) as cp,
            tc.tile_pool(name="dram", bufs=1, space="DRAM") as dp,
            tc.tile_pool(name="xs", bufs=3) as xp,
            tc.tile_pool(name="pso", bufs=2, space="PSUM") as pso_pool,
            tc.tile_pool(name="pst", bufs=3, space="PSUM") as pst_pool,
            tc.tile_pool(name="g", bufs=4) as gp_pool,
            tc.tile_pool(name="v", bufs=3) as vp,
            tc.tile_pool(name="wz", bufs=4) as wp,
            tc.tile_pool(name="epi", bufs=3) as ep,
        ):
            table = cp.tile([128, nstr * TROW], BF16)
            idx_t = cp.tile([128, nch * 8], I16)
            msk_t = cp.tile([128, nch], BF16)
            ident = cp.tile([128, 128], BF16)
            w1_sb = cp.tile([128, TROW], BF16)
            wad1 = cp.tile([128, 2], BF16)
            w2_sb = cp.tile([128, 64], BF16)
            ws2 = cp.tile([128, 128], BF16)
            wd2 = cp.tile([128, 128], BF16)
            b1_sb = cp.tile([128, 128], F32)
            b2_sb = cp.tile([128, 64], F32)
            xtl = cp.tile([128, pc], BF16)
            a1_loc = cp.tile([128, nb * 2], F32)
            a2_loc = cp.tile([128, nb], F32)
            for t, d in ((idx_t, idx_d), (msk_t, msk_d), (ident, id_d),
                         (w1_sb, w1_d), (wad1, wad1_d), (w2_sb, w2_d),
                         (ws2, ws2_d), (wd2, wd2_d), (b1_sb, b1_d),
                         (b2_sb, b2_d), (xtl, xtl_d)):
                nc.sync.dma_start(out=t[:], in_=d[:])

            loc_dram = dp.tile([128, nb * TROW], BF16)
            ag_out = dp.tile([128 * NCORES, nb * TROW], BF16)

            # ---- adst1 for local blocks ----
            for j in range(nb):
                psa = pst_pool.tile([128, 2], F32, tag="pst")
                nc.tensor.matmul(psa[:], lhsT=xtl[:, j * 128:(j + 1) * 128],
                                 rhs=wad1[:], start=True, stop=True)
                nc.scalar.activation(a1_loc[:, j * 2:(j + 1) * 2], psa[:],
                                     ACTF.Copy)

            # ---- L1 table build: [x@W1 | x@W1@a_src] (full, replicated) ----
            for s in range(nstr):
                xpan = xp.tile([128, 128], BF16, tag="xt")
                nc.sync.dma_start(out=xpan[:], in_=xt_d[:, s * 128:(s + 1) * 128])
                psh = pst_pool.tile([128, TROW], F32, tag="pst")
                nc.tensor.matmul(psh[:], lhsT=xpan[:], rhs=w1_sb[:],
                                 start=True, stop=True)
                nc.scalar.activation(table[:, s * TROW:(s + 1) * TROW], psh[:],
                                     ACTF.Copy)

            def agg_layer(layer):
                if layer == 1:
                    aloc, H, vcols = a1_loc, 2, 130
                else:
                    aloc, H, vcols = a2_loc, 1, 129
                hd = 128 // H
                tab_lo = table[:, 0:hstr * TROW]
                tab_hi = table[:, hstr * TROW:nstr * TROW]
                for j in range(nb):
                    kj = int(K[j])
                    cb = int(chunk_base[j])
                    o1 = pso_pool.tile([128, vcols], F32, tag="pso")
                    batches = []
                    done = 0
                    for rl in (int(K_lo[j]), int(K_hi[j])):
                        r0 = done
                        while done < r0 + rl:
                            gl = min(GBATCH, r0 + rl - done)
                            batches.append((done, gl, done >= int(K_lo[j])))
                            done += gl
                    for (b0, gl, in_hi) in batches:
                        k0 = cb + b0
                        gt = gp_pool.tile([128, GBATCH * TROW], BF16, tag="g")
                        _sbuf_gather_rows(
                            nc,
                            gt[:, 0:gl * TROW].rearrange(
                                "p (k c) -> p k c", c=TROW),
                            tab_hi if in_hi else tab_lo,
                            idx_t[:, k0 * 8:(k0 + gl) * 8],
                            gl * 128, qn[0])
                        qn[0] = (qn[0] + 1) % 4
                        gv = gt[:, 0:gl * TROW].rearrange(
                            "p (k c) -> p k c", c=TROW)
                        vt = vp.tile([128, GBATCH * vcols], BF16, tag="v")
                        vv = vt[:, 0:gl * vcols].rearrange(
                            "p (k c) -> p k c", c=vcols)
                        zt = wp.tile([128, GBATCH * H], F32, tag="z")
                        zv = zt[:, 0:gl * H].rearrange("p (k h) -> p k h", h=H)
                        for h in range(H):
                            nc.scalar.activation(
                                zv[:, :, h], gv[:, :, 128 + h],
                                ACTF.Prelu,
                                bias=aloc[:, j * H + h:j * H + h + 1],
                                alpha=NEG_SLOPE)
                        nc.scalar.activation(zv[:, :, :], zv[:, :, :],
                                             ACTF.Exp)
                        nc.vector.tensor_tensor(
                            out=vv[:, :, H * hd:vcols], in0=zv[:, :, :],
                            in1=msk_t[:, k0:k0 + gl].to_broadcast(
                                [128, gl, H]),
                            op=ALU.mult)
                        for h in range(H):
                            nc.vector.tensor_tensor(
                                out=vv[:, :, h * hd:(h + 1) * hd],
                                in0=gv[:, :, h * hd:(h + 1) * hd],
                                in1=vv[:, :, H * hd + h:H * hd + h + 1]
                                    .to_broadcast([128, gl, hd]),
                                op=ALU.mult)
                        for i in range(gl):
                            nc.tensor.matmul(
                                o1[:], lhsT=ident[:],
                                rhs=vt[:, i * vcols:(i + 1) * vcols],
                                start=(b0 + i == 0), stop=(b0 + i == kj - 1))

                    # ---- epilogue ----
                    dsafe = wp.tile([128, H], F32, tag="ds")
                    nc.vector.tensor_scalar_add(dsafe[:], o1[:, H * hd:vcols],
                                                1e-30)
                    rden = wp.tile([128, H], F32, tag="rd")
                    nc.vector.reciprocal(rden[:], dsafe[:])
                    if layer == 1:
                        pre = ep.tile([128, 128], F32, tag="pre")
                        for h in range(H):
                            nc.scalar.activation(
                                pre[:, h * hd:(h + 1) * hd],
                                o1[:, h * hd:(h + 1) * hd],
                                ACTF.Copy, scale=rden[:, h:h + 1])
                        nc.vector.tensor_tensor(out=pre[:], in0=pre[:],
                                                in1=b1_sb[:], op=ALU.add)
                        row = ep.tile([128, TROW], BF16, tag="row")
                        nc.scalar.activation(row[:, 0:128], pre[:], ACTF.Relu)
                        # alpha_src2 / alpha_dst2 from relu(out1)
                        tmp2 = ep.tile([128, 128], F32, tag="tmp2")
                        red = wp.tile([128, 2], F32, tag="red")
                        nc.vector.tensor_tensor(out=tmp2[:], in0=row[:, 0:128],
                                                in1=ws2[:], op=ALU.mult)
                        nc.vector.tensor_reduce(
                            red[:, 0:1],
                            tmp2[:].rearrange("p (o c) -> p o c", o=1),
                            axis=AXL.X, op=ALU.add)
                        nc.vector.tensor_tensor(out=tmp2[:], in0=row[:, 0:128],
                                                in1=wd2[:], op=ALU.mult)
                        nc.vector.tensor_reduce(
                            red[:, 1:2],
                            tmp2[:].rearrange("p (o c) -> p o c", o=1),
                            axis=AXL.X, op=ALU.add)
                        nc.scalar.activation(row[:, 128:129], red[:, 0:1],
                                             ACTF.Copy)
                        nc.scalar.activation(a2_loc[:, j:j + 1], red[:, 1:2],
                                             ACTF.Copy)
                        nc.scalar.activation(row[:, 129:132], red[:, 0:1]
                                             .to_broadcast([128, 3]),
                                             ACTF.Copy)
                        nc.sync.dma_start(
                            out=loc_dram[:, j * TROW:(j + 1) * TROW],
                            in_=row[:])
                    else:
                        preb = ep.tile([128, 128], BF16, tag="preb")
                        nc.scalar.activation(preb[:], o1[:, 0:128], ACTF.Copy,
                                             scale=rden[:, 0:1])
                        pstr = pst_pool.tile([128, 128], F32, tag="pst")
                        nc.tensor.matmul(pstr[:], lhsT=preb[:], rhs=ident[:],
                                         start=True, stop=True)
                        preT = ep.tile([128, 128], BF16, tag="preT")
                        nc.scalar.activation(preT[:], pstr[:], ACTF.Copy)
                        ps2 = pst_pool.tile([128, 64], F32, tag="pst")
                        nc.tensor.matmul(ps2[:], lhsT=preT[:], rhs=w2_sb[:],
                                         start=True, stop=True)
                        ob = ep.tile([128, OUT], F32, tag="ob")
                        nc.vector.tensor_tensor(out=ob[:], in0=ps2[:],
                                                in1=b2_sb[:], op=ALU.add)
                        nc.scalar.activation(ob[:], ob[:], ACTF.Sigmoid)
                        nc.sync.dma_start(out=out_d[j * 128:(j + 1) * 128, :],
                                          in_=ob[:])

            agg_layer(1)

            # ---- AllGather [relu(out1) | asrc2] rows -> L2 table ----
            nc.gpsimd.collective_compute(
                "AllGather", mybir.AluOpType.bypass,
                replica_groups=[list(range(NCORES))],
                ins=[loc_dram[:]], outs=[ag_out[:]],
            )
            for cc in range(NCORES):
                nc.sync.dma_start(
                    out=table[:, cc * nb * TROW:(cc + 1) * nb * TROW],
                    in_=ag_out[cc * 128:(cc + 1) * 128, :])

            agg_layer(2)

    nc.finalize()
    return nc


def kernel(x, edge_index, W1, att_src1, att_dst1, b1, W2, att_src2, att_dst2,
           b2):
    from concourse import bass_utils

    x = np.asarray(x, np.float32)
    W1 = np.asarray(W1, np.float32)
    W2 = np.asarray(W2, np.float32)
    att_src1 = np.asarray(att_src1, np.float32)
    att_dst1 = np.asarray(att_dst1, np.float32)
    att_src2 = np.asarray(att_src2, np.float32)
    att_dst2 = np.asarray(att_dst2, np.float32)
    b1 = np.asarray(b1, np.float32)
    b2 = np.asarray(b2, np.float32)
    n_nodes = x.shape[0]

    sch = _build_schedule(edge_index, n_nodes)
    vpad, pc = sch["vpad"], sch["pc"]

    W1r = W1.reshape(F_IN, HEADS, HID)
    w1aug = np.zeros((128, TROW), np.float32)
    w1aug[:, 0:128] = W1
    w1aug[:, 128] = W1r[:, 0, :] @ att_src1[0]
    w1aug[:, 129] = W1r[:, 1, :] @ att_src1[1]
    wad1 = np.stack([W1r[:, h, :] @ att_dst1[h] for h in range(HEADS)], 1)
    ws2_t = np.broadcast_to(W2 @ att_src2[0], (128, 128)).copy()
    wd2_t = np.broadcast_to(W2 @ att_dst2[0], (128, 128)).copy()
    b1_rep = np.broadcast_to(b1, (128, HEADS * HID)).astype(np.float32).copy()
    b2_rep = np.broadcast_to(b2, (128, OUT)).astype(np.float32).copy()

    x_rho = np.zeros((vpad, F_IN), np.float32)
    x_rho[sch["row_of_node"]] = x
    xt_full = _bf16(np.ascontiguousarray(x_rho.T))          # [128, vpad]

    key = (vpad, sch["nch"], tuple(sch["K"].tolist()),
           tuple(sch["K_lo"].tolist()), DEBUG)
    if key not in _cache:
        _cache[key] = _build_program(vpad, pc, sch["nb"], sch["half"],
                                     sch["K"], sch["K_lo"], sch["K_hi"],
                                     sch["nch"], sch["chunk_base"])
    nc = _cache[key]

    in_maps = []
    for c in range(NCORES):
        in_maps.append({
            "xt": xt_full,
            "xtl": np.ascontiguousarray(xt_full[:, c * pc:(c + 1) * pc]),
            "idx": sch["idx_wrapped"][c],
            "msk": _bf16(sch["mask_stream"][c]),
            "ident": _bf16(np.eye(128, dtype=np.float32)),
            "w1aug": _bf16(w1aug),
            "wad1": _bf16(wad1),
            "w2": _bf16(W2),
            "ws2": _bf16(ws2_t),
            "wd2": _bf16(wd2_t),
            "b1rep": b1_rep,
            "b2rep": b2_rep,
        })
    res = bass_utils.run_bass_kernel_spmd(nc, in_maps,
                                          core_ids=list(range(NCORES)),
                                          trace=TRACE)
    kernel.last_exec_ns = res.exec_time_ns
    kernel.last_mean_ns = res.mean_exec_time_ns
    out_all = np.concatenate([res.results[c]["out"] for c in range(NCORES)], 0)
    return out_all[sch["row_of_node"][:n_nodes]]


# revision 7
# speedup vs baseline: 1.3842x; 1.0163x over previous
"""Two-layer GAT on 8 Trainium2 NeuronCores (Bass/Tile) — v3.

The per-edge gather tables live in SBUF as bf16 rows of 264B
([h (128) | alpha_src (1-2) | pad], 132 bf16) and are read with
non-transpose SBUF-source dma_gather on 4 rotating SWDGE queues: plain
SBUF->SBUF 264B descriptor pairs (no HBM latency, no XBAR), with descriptor
generation spread over all 8 Q7 cores.  The bass wrapper only allows
transpose=True for SBUF sources, so the instruction is constructed directly
(the non-transpose SBUF path exists in the ucode and is exact).  L1 table
rows are [x@W1 | x@W1@a_src] computed per-core from the replicated x (no
AllGather); L2 rows are [relu(out1) | relu(out1)@W2@a_src2] (W2 is applied
AFTER aggregation by linearity), so the only collective is a 1.7MB/core bf16
AllGather.  The slot-chunk schedule, lane-major V build, softmax by
reciprocal and identity-stationary segment-sum matmuls are from v1.
"""

import numpy as np

NCORES = 8
F_IN = 128
HID = 64
HEADS = 2
OUT = 64
NEG_SLOPE = 0.2
GBATCH = 16   # chunks per dma_gather call
TROW = 132    # table row: 132 bf16 = 264B

DEBUG = False
TRACE = False
_cache = {}


def _bf16(x):
    u = np.asarray(x, np.float32).view(np.uint32)
    r = ((u >> 16) & 1) + 0x7FFF
    return ((u + r) >> 16).astype(np.uint16)


def _build_schedule(edge_index, n_nodes):
    ei = np.asarray(edge_index).astype(np.int64)
    src = np.concatenate([ei[0], np.arange(n_nodes, dtype=np.int64)])
    dst = np.concatenate([ei[1], np.arange(n_nodes, dtype=np.int64)])
    deg = np.bincount(dst, minlength=n_nodes)

    stripe = NCORES * 128
    vpad = ((n_nodes + stripe - 1) // stripe) * stripe
    pc = vpad // NCORES
    nb = pc // 128
    half = vpad // 2
    assert half <= 32768

    degp = np.zeros(vpad, np.int64)
    degp[:n_nodes] = deg
    order = np.argsort(-degp, kind="stable")
    rank = np.empty(vpad, np.int64)
    rank[order] = np.arange(vpad)

    s = np.arange(vpad)
    g = s // 128
    lane = s % 128
    row_of_rank = (g % NCORES) * pc + (g // NCORES) * 128 + lane
    row_of_node = row_of_rank[rank[:n_nodes]]

    e_dstrow = row_of_node[dst]
    e_srcrow = row_of_node[src]
    is_hi = e_srcrow >= half

    lo_cnt = np.bincount(e_dstrow[~is_hi], minlength=vpad)
    hi_cnt = np.bincount(e_dstrow[is_hi], minlength=vpad)

    jj = (np.arange(vpad) % pc) // 128
    K_lo = np.zeros(nb, np.int64)
    K_hi = np.zeros(nb, np.int64)
    np.maximum.at(K_lo, jj, lo_cnt)
    np.maximum.at(K_hi, jj, hi_cnt)
    K = K_lo + K_hi
    bump = K == 0
    K_lo[bump] += 1
    K[bump] += 1
    nch = int(K.sum())
    chunk_base = np.concatenate([[0], np.cumsum(K)])[:-1]

    key = e_dstrow * 2 + is_hi
    ord_e = np.argsort(key, kind="stable")
    ds = e_dstrow[ord_e]
    hs = is_hi[ord_e]
    first = np.r_[True, ds[1:] != ds[:-1]]
    grp_start = np.flatnonzero(first)
    grp_id = np.cumsum(first) - 1
    slot = np.arange(ds.shape[0]) - grp_start[grp_id]
    c = ds // pc
    j = (ds % pc) // 128
    ln = ds % 128
    pos = chunk_base[j] + np.where(hs, K_lo[j] + slot - lo_cnt[ds], slot)
    assert (pos >= chunk_base[j]).all() and (pos < chunk_base[j] + K[j]).all()

    idx_val = np.where(hs, e_srcrow[ord_e] - half, e_srcrow[ord_e])
    idx_stream = np.zeros((NCORES, 128, nch), np.int16)
    mask_stream = np.zeros((NCORES, 128, nch), np.float32)
    idx_stream[c, ln, pos] = idx_val.astype(np.int16)
    mask_stream[c, ln, pos] = 1.0

    # wrapped int16 layout: chunk k -> columns 8k:8k+8; within a chunk the
    # 128 lane-indices wrap as flat[i] -> [i % 16, i // 16], replicated over
    # the 8 16-partition groups (each SWDGE queue's core-pair reads its own).
    iw = idx_stream.transpose(0, 2, 1).reshape(NCORES, nch, 8, 16)
    iw = iw.transpose(0, 3, 1, 2).reshape(NCORES, 16, nch * 8)
    idx_wrapped = np.tile(iw, (1, 8, 1))

    return dict(vpad=vpad, pc=pc, nb=nb, half=half, K=K, K_lo=K_lo, K_hi=K_hi,
                nch=nch, chunk_base=chunk_base, row_of_node=row_of_node,
                idx_wrapped=np.ascontiguousarray(idx_wrapped),
                mask_stream=mask_stream)


def _sbuf_gather_rows(nc, out_ap, in_ap, idxs_ap, num_idxs, queue_num):
    """Non-transpose SBUF-source dma_gather (row layout, 264B elements).

    The bass wrapper restricts SBUF sources to transpose=True; the ucode's
    non-transpose src_is_sbuf path is complete and exact, so build the
    instruction directly.  Row r of the table sits at partition r%128,
    byte offset (r//128)*2*TROW of in_ap.
    """
    import concourse.mybir as mybir
    gp = nc.gpsimd
    return gp.add_instruction(
        mybir.InstDMAGatherAnt(
            name=nc.get_next_instruction_name(),
            ins=[
                gp.lower_ap(in_ap),
                gp.lower_ap(idxs_ap),
                gp.lower_val_access(gp.to_reg(num_idxs)),
            ],
            outs=[gp.lower_ap(out_ap)],
            transpose=False,
            num_idxs=num_idxs,
            elem_size=TROW,
            stride_bytes_256=0,
            gen_mode=0,
            single_packet=False,
            queue_num=queue_num,
            sbuf_tokens_per_rank=128,
            sbuf_free_dim_per_rank=2 * TROW,
            sbuf_free_dim_pad_per_rank=0,
            sbuf_byte_offset=0,
        )
    )


def _build_program(vpad, pc, nb, half, K, K_lo, K_hi, nch, chunk_base):
    import concourse.bacc as bacc
    import concourse.mybir as mybir
    import concourse.tile as tile

    F32 = mybir.dt.float32
    BF16 = mybir.dt.bfloat16
    I16 = mybir.dt.int16
    ACTF = mybir.ActivationFunctionType
    ALU = mybir.AluOpType
    AXL = mybir.AxisListType

    nstr = vpad // 128
    hstr = half // 128

    nc = bacc.Bacc("TRN2", target_bir_lowering=False, debug=False,
                   num_devices=NCORES, num_swdge_queues=4)

    xt_d = nc.dram_tensor("xt", [128, vpad], BF16, kind="ExternalInput")
    xtl_d = nc.dram_tensor("xtl", [128, pc], BF16, kind="ExternalInput")
    idx_d = nc.dram_tensor("idx", [128, nch * 8], I16, kind="ExternalInput")
    msk_d = nc.dram_tensor("msk", [128, nch], BF16, kind="ExternalInput")
    id_d = nc.dram_tensor("ident", [128, 128], BF16, kind="ExternalInput")
    w1_d = nc.dram_tensor("w1aug", [128, TROW], BF16, kind="ExternalInput")
    wad1_d = nc.dram_tensor("wad1", [128, 2], BF16, kind="ExternalInput")
    w2_d = nc.dram_tensor("w2", [128, 64], BF16, kind="ExternalInput")
    ws2_d = nc.dram_tensor("ws2", [128, 128], BF16, kind="ExternalInput")
    wd2_d = nc.dram_tensor("wd2", [128, 128], BF16, kind="ExternalInput")
    b1_d = nc.dram_tensor("b1rep", [128, 128], F32, kind="ExternalInput")
    b2_d = nc.dram_tensor("b2rep", [128, 64], F32, kind="ExternalInput")
    out_d = nc.dram_tensor("out", [pc, OUT], F32, kind="ExternalOutput")

    qn = [0]

    with tile.TileContext(nc) as tc:
        with (
            tc.tile_pool(name="const", bufs=1) as cp,
            tc.tile_pool(name="dram", bufs=1, space="DRAM") as dp,
            tc.tile_pool(name="xs", bufs=3) as xp,
            tc.tile_pool(name="pso", bufs=3, space="PSUM") as pso_pool,
            tc.tile_pool(name="pst", bufs=3, space="PSUM") as pst_pool,
            tc.tile_pool(name="g", bufs=3) as gp_pool,
            tc.tile_pool(name="v", bufs=3) as vp,
            tc.tile_pool(name="wz", bufs=4) as wp,
            tc.tile_pool(name="epi", bufs=3) as ep,
        ):
            table = cp.tile([128, nstr * TROW], BF16)
            idx_t = cp.tile([128, nch * 8], I16)
            msk_t = cp.tile([128, nch], BF16)
            ident = cp.tile([128, 128], BF16)
            w1_sb = cp.tile([128, TROW], BF16)
            wad1 = cp.tile([128, 2], BF16)
            w2_sb = cp.tile([128, 64], BF16)
            ws2 = cp.tile([128, 128], BF16)
            wd2 = cp.tile([128, 128], BF16)
            b1_sb = cp.tile([128, 128], F32)
            b2_sb = cp.tile([128, 64], F32)
            xtl = cp.tile([128, pc], BF16)
            a1_loc = cp.tile([128, nb * 2], F32)
            a2_loc = cp.tile([128, nb], F32)
            for t, d in ((idx_t, idx_d), (msk_t, msk_d), (ident, id_d),
                         (w1_sb, w1_d), (wad1, wad1_d), (w2_sb, w2_d),
                         (ws2, ws2_d), (wd2, wd2_d), (b1_sb, b1_d),
                         (b2_sb, b2_d), (xtl, xtl_d)):
                nc.sync.dma_start(out=t[:], in_=d[:])

            loc_dram = dp.tile([128, nb * TROW], BF16)
            ag_out = dp.tile([128 * NCORES, nb * TROW], BF16)

            # ---- adst1 for local blocks ----
            for j in range(nb):
                psa = pst_pool.tile([128, 2], F32, tag="pst")
                nc.tensor.matmul(psa[:], lhsT=xtl[:, j * 128:(j + 1) * 128],
                                 rhs=wad1[:], start=True, stop=True)
                nc.scalar.activation(a1_loc[:, j * 2:(j + 1) * 2], psa[:],
                                     ACTF.Copy)

            # ---- L1 table build: [x@W1 | x@W1@a_src] (full, replicated) ----
            for s in range(nstr):
                xpan = xp.tile([128, 128], BF16, tag="xt")
                nc.sync.dma_start(out=xpan[:], in_=xt_d[:, s * 128:(s + 1) * 128])
                psh = pst_pool.tile([128, TROW], F32, tag="pst")
                nc.tensor.matmul(psh[:], lhsT=xpan[:], rhs=w1_sb[:],
                                 start=True, stop=True)
                nc.scalar.activation(table[:, s * TROW:(s + 1) * TROW], psh[:],
                                     ACTF.Copy)

            def agg_layer(layer):
                if layer == 1:
                    aloc, H, vcols = a1_loc, 2, 130
                else:
                    aloc, H, vcols = a2_loc, 1, 129
                hd = 128 // H
                tab_lo = table[:, 0:hstr * TROW]
                tab_hi = table[:, hstr * TROW:nstr * TROW]
                for j in range(nb):
                    kj = int(K[j])
                    cb = int(chunk_base[j])
                    o1 = pso_pool.tile([128, vcols], F32, tag="pso")
                    batches = []
                    done = 0
                    for rl in (int(K_lo[j]), int(K_hi[j])):
                        r0 = done
                        while done < r0 + rl:
                            gl = min(GBATCH, r0 + rl - done)
                            batches.append((done, gl, done >= int(K_lo[j])))
                            done += gl
                    for (b0, gl, in_hi) in batches:
                        k0 = cb + b0
                        gt = gp_pool.tile([128, GBATCH * TROW], BF16, tag="g")
                        _sbuf_gather_rows(
                            nc,
                            gt[:, 0:gl * TROW].rearrange(
                                "p (k c) -> p k c", c=TROW),
                            tab_hi if in_hi else tab_lo,
                            idx_t[:, k0 * 8:(k0 + gl) * 8],
                            gl * 128, qn[0])
                        qn[0] = (qn[0] + 1) % 4
                        gv = gt[:, 0:gl * TROW].rearrange(
                            "p (k c) -> p k c", c=TROW)
                        vt = vp.tile([128, GBATCH * vcols], BF16, tag="v")
                        vv = vt[:, 0:gl * vcols].rearrange(
                            "p (k c) -> p k c", c=vcols)
                        zt = wp.tile([128, GBATCH * H], F32, tag="z")
                        zv = zt[:, 0:gl * H].rearrange("p (k h) -> p k h", h=H)
                        for h in range(H):
                            nc.scalar.activation(
                                zv[:, :, h], gv[:, :, 128 + h],
                                ACTF.Prelu,
                                bias=aloc[:, j * H + h:j * H + h + 1],
                                alpha=NEG_SLOPE)
                        nc.scalar.activation(zv[:, :, :], zv[:, :, :],
                                             ACTF.Exp)
                        nc.vector.tensor_tensor(
                            out=vv[:, :, H * hd:vcols], in0=zv[:, :, :],
                            in1=msk_t[:, k0:k0 + gl].to_broadcast(
                                [128, gl, H]),
                            op=ALU.mult)
                        for h in range(H):
                            nc.vector.tensor_tensor(
                                out=vv[:, :, h * hd:(h + 1) * hd],
                                in0=gv[:, :, h * hd:(h + 1) * hd],
                                in1=vv[:, :, H * hd + h:H * hd + h + 1]
                                    .to_broadcast([128, gl, hd]),
                                op=ALU.mult)
                        for i in range(gl):
                            nc.tensor.matmul(
                                o1[:], lhsT=ident[:],
                                rhs=vt[:, i * vcols:(i + 1) * vcols],
                                start=(b0 + i == 0), stop=(b0 + i == kj - 1))

                    # ---- epilogue ----
                    dsafe = wp.tile([128, H], F32, tag="ds")
                    nc.vector.tensor_scalar_add(dsafe[:], o1[:, H * hd:vcols],
                                                1e-30)
                    rden = wp.tile([128, H], F32, tag="rd")
                    nc.vector.reciprocal(rden[:], dsafe[:])
                    if layer == 1:
                        pre = ep.tile([128, 128], F32, tag="pre")
                        for h in range(H):
                            nc.scalar.activation(
                                pre[:, h * hd:(h + 1) * hd],
                                o1[:, h * hd:(h + 1) * hd],
                                ACTF.Copy, scale=rden[:, h:h + 1])
                        nc.vector.tensor_tensor(out=pre[:], in0=pre[:],
                                                in1=b1_sb[:], op=ALU.add)
                        row = ep.tile([128, TROW], BF16, tag="row")
                        nc.scalar.activation(row[:, 0:128], pre[:], ACTF.Relu)
                        # alpha_src2 / alpha_dst2 from relu(out1)
                        tmp2 = ep.tile([128, 128], F32, tag="tmp2")
                        red = wp.tile([128, 2], F32, tag="red")
                        nc.vector.tensor_tensor(out=tmp2[:], in0=row[:, 0:128],
                                                in1=ws2[:], op=ALU.mult)
                        nc.vector.tensor_reduce(
                            red[:, 0:1],
                            tmp2[:].rearrange("p (o c) -> p o c", o=1),
                            axis=AXL.X, op=ALU.add)
                        nc.vector.tensor_tensor(out=tmp2[:], in0=row[:, 0:128],
                                                in1=wd2[:], op=ALU.mult)
                        nc.vector.tensor_reduce(
                            red[:, 1:2],
                            tmp2[:].rearrange("p (o c) -> p o c", o=1),
                            axis=AXL.X, op=ALU.add)
                        nc.scalar.activation(row[:, 128:129], red[:, 0:1],
                                             ACTF.Copy)
                        nc.scalar.activation(a2_loc[:, j:j + 1], red[:, 1:2],
                                             ACTF.Copy)
                        nc.scalar.activation(row[:, 129:132], red[:, 0:1]
                                             .to_broadcast([128, 3]),
                                             ACTF.Copy)
                        nc.sync.dma_start(
                            out=loc_dram[:, j * TROW:(j + 1) * TROW],
                            in_=row[:])
                    else:
                        preb = ep.tile([128, 128], BF16, tag="preb")
                        nc.scalar.activation(preb[:], o1[:, 0:128], ACTF.Copy,
                                             scale=rden[:, 0:1])
                        pstr = pst_pool.tile([128, 128], F32, tag="pst")
                        nc.tensor.matmul(pstr[:], lhsT=preb[:], rhs=ident[:],
                                         start=True, stop=True)
                        preT = ep.tile([128, 128], BF16, tag="preT")
                        nc.scalar.activation(preT[:], pstr[:], ACTF.Copy)
                        ps2 = pst_pool.tile([128, 64], F32, tag="pst")
                        nc.tensor.matmul(ps2[:], lhsT=preT[:], rhs=w2_sb[:],
                                         start=True, stop=True)
                        ob = ep.tile([128, OUT], F32, tag="ob")
                        nc.vector.tensor_tensor(out=ob[:], in0=ps2[:],
                                                in1=b2_sb[:], op=ALU.add)
                        nc.scalar.activation(ob[:], ob[:], ACTF.Sigmoid)
                        nc.sync.dma_start(out=out_d[j * 128:(j + 1) * 128, :],
                                          in_=ob[:])

            agg_layer(1)

            # ---- AllGather [relu(out1) | asrc2] rows -> L2 table ----
            nc.gpsimd.collective_compute(
                "AllGather", mybir.AluOpType.bypass,
                replica_groups=[list(range(NCORES))],
                ins=[loc_dram[:]], outs=[ag_out[:]],
            )
            for cc in range(NCORES):
                nc.sync.dma_start(
                    out=table[:, cc * nb * TROW:(cc + 1) * nb * TROW],
                    in_=ag_out[cc * 128:(cc + 1) * 128, :])

            agg_layer(2)

    nc.finalize()
    return nc


def kernel(x, edge_index, W1, att_src1, att_dst1, b1, W2, att_src2, att_dst2,
           b2):
    from concourse import bass_utils

    x = np.asarray(x, np.float32)
    W1 = np.asarray(W1, np.float32)
    W2 = np.asarray(W2, np.float32)
    att_src1 = np.asarray(att_src1, np.float32)
    att_dst1 = np.asarray(att_dst1, np.float32)
    att_src2 = np.asarray(att_src2, np.float32)
    att_dst2 = np.asarray(att_dst2, np.float32)
    b1 = np.asarray(b1, np.float32)
    b2 = np.asarray(b2, np.float32)
    n_nodes = x.shape[0]

    sch = _build_schedule(edge_index, n_nodes)
    vpad, pc = sch["vpad"], sch["pc"]

    W1r = W1.reshape(F_IN, HEADS, HID)
    w1aug = np.zeros((128, TROW), np.float32)
    w1aug[:, 0:128] = W1
    w1aug[:, 128] = W1r[:, 0, :] @ att_src1[0]
    w1aug[:, 129] = W1r[:, 1, :] @ att_src1[1]
    wad1 = np.stack([W1r[:, h, :] @ att_dst1[h] for h in range(HEADS)], 1)
    ws2_t = np.broadcast_to(W2 @ att_src2[0], (128, 128)).copy()
    wd2_t = np.broadcast_to(W2 @ att_dst2[0], (128, 128)).copy()
    b1_rep = np.broadcast_to(b1, (128, HEADS * HID)).astype(np.float32).copy()
    b2_rep = np.broadcast_to(b2, (128, OUT)).astype(np.float32).copy()

    x_rho = np.zeros((vpad, F_IN), np.float32)
    x_rho[sch["row_of_node"]] = x
    xt_full = _bf16(np.ascontiguousarray(x_rho.T))          # [128, vpad]

    key = (vpad, sch["nch"], tuple(sch["K"].tolist()),
           tuple(sch["K_lo"].tolist()), DEBUG)
    if key not in _cache:
        _cache[key] = _build_program(vpad, pc, sch["nb"], sch["half"],
                                     sch["K"], sch["K_lo"], sch["K_hi"],
                                     sch["nch"], sch["chunk_base"])
    nc = _cache[key]

    in_maps = []
    for c in range(NCORES):
        in_maps.append({
            "xt": xt_full,
            "xtl": np.ascontiguousarray(xt_full[:, c * pc:(c + 1) * pc]),
            "idx": sch["idx_wrapped"][c],
            "msk": _bf16(sch["mask_stream"][c]),
            "ident": _bf16(np.eye(128, dtype=np.float32)),
            "w1aug": _bf16(w1aug),
            "wad1": _bf16(wad1),
            "w2": _bf16(W2),
            "ws2": _bf16(ws2_t),
            "wd2": _bf16(wd2_t),
            "b1rep": b1_rep,
            "b2rep": b2_rep,
        })
    res = bass_utils.run_bass_kernel_spmd(nc, in_maps,
                                          core_ids=list(range(NCORES)),
                                          trace=TRACE)
    kernel.last_exec_ns = res.exec_time_ns
    kernel.last_mean_ns = res.mean_exec_time_ns
    out_all = np.concatenate([res.results[c]["out"] for c in range(NCORES)], 0)
    return out_all[sch["row_of_node"][:n_nodes]]


# revision 8
# speedup vs baseline: 2.2268x; 1.6087x over previous
"""Two-layer GAT on 8 Trainium2 NeuronCores (Bass/Tile) — v3.

The per-edge gather tables live in SBUF as bf16 rows of 264B
([h (128) | alpha_src (1-2) | pad], 132 bf16) and are read with
non-transpose SBUF-source dma_gather on 4 rotating SWDGE queues: plain
SBUF->SBUF 264B descriptor pairs (no HBM latency, no XBAR), with descriptor
generation spread over all 8 Q7 cores.  The bass wrapper only allows
transpose=True for SBUF sources, so the instruction is constructed directly
(the non-transpose SBUF path exists in the ucode and is exact).  L1 table
rows are [x@W1 | x@W1@a_src] computed per-core from the replicated x (no
AllGather); L2 rows are [relu(out1) | relu(out1)@W2@a_src2] (W2 is applied
AFTER aggregation by linearity), so the only collective is a 1.7MB/core bf16
AllGather.  The slot-chunk schedule, lane-major V build, softmax by
reciprocal and identity-stationary segment-sum matmuls are from v1.
"""

import numpy as np

NCORES = 8
F_IN = 128
HID = 64
HEADS = 2
OUT = 64
NEG_SLOPE = 0.2
GBATCH = 16   # chunks per dma_gather call
TROW = 132    # table row: 132 bf16 = 264B

DEBUG = False
TRACE = False
_cache = {}


def _bf16(x):
    u = np.asarray(x, np.float32).view(np.uint32)
    r = ((u >> 16) & 1) + 0x7FFF
    return ((u + r) >> 16).astype(np.uint16)


def _build_schedule(edge_index, n_nodes):
    ei = np.asarray(edge_index).astype(np.int64)
    src = np.concatenate([ei[0], np.arange(n_nodes, dtype=np.int64)])
    dst = np.concatenate([ei[1], np.arange(n_nodes, dtype=np.int64)])
    deg = np.bincount(dst, minlength=n_nodes)

    stripe = NCORES * 128
    vpad = ((n_nodes + stripe - 1) // stripe) * stripe
    pc = vpad // NCORES
    nb = pc // 128
    half = vpad // 2
    assert half <= 32768

    degp = np.zeros(vpad, np.int64)
    degp[:n_nodes] = deg
    order = np.argsort(-degp, kind="stable")
    rank = np.empty(vpad, np.int64)
    rank[order] = np.arange(vpad)

    s = np.arange(vpad)
    g = s // 128
    lane = s % 128
    row_of_rank = (g % NCORES) * pc + (g // NCORES) * 128 + lane
    row_of_node = row_of_rank[rank[:n_nodes]]

    # Re-pack nodes within each core to minimize per-stripe slot maxima.
    # An edge's lo/hi class depends only on its source's CORE (cores 0-3 are
    # the lo half), which a within-core permutation preserves, so per-node
    # lo/hi counts are invariant under the re-packing.
    e_dstrow = row_of_node[dst]
    e_srcrow = row_of_node[src]
    is_hi = e_srcrow >= half
    lo_cnt = np.bincount(e_dstrow[~is_hi], minlength=vpad)
    hi_cnt = np.bincount(e_dstrow[is_hi], minlength=vpad)
    rfull = row_of_rank[rank]
    lo_n = lo_cnt[rfull]
    hi_n = hi_cnt[rfull]
    key = np.maximum(lo_n, hi_n) * 1000.0 + lo_n + hi_n
    core_of = rfull // pc
    new_row = np.empty(vpad, np.int64)
    for cc in range(NCORES):
        nodes = np.nonzero(core_of == cc)[0]
        o = nodes[np.argsort(-key[nodes], kind="stable")]
        new_row[o] = cc * pc + np.arange(pc)
    row_of_node = new_row[:n_nodes]

    e_dstrow = row_of_node[dst]
    e_srcrow = row_of_node[src]
    is_hi = e_srcrow >= half

    lo_cnt = np.bincount(e_dstrow[~is_hi], minlength=vpad)
    hi_cnt = np.bincount(e_dstrow[is_hi], minlength=vpad)

    jj = (np.arange(vpad) % pc) // 128
    K_lo = np.zeros(nb, np.int64)
    K_hi = np.zeros(nb, np.int64)
    np.maximum.at(K_lo, jj, lo_cnt)
    np.maximum.at(K_hi, jj, hi_cnt)
    K = K_lo + K_hi
    bump = K == 0
    K_lo[bump] += 1
    K[bump] += 1
    nch = int(K.sum())
    chunk_base = np.concatenate([[0], np.cumsum(K)])[:-1]

    key = e_dstrow * 2 + is_hi
    ord_e = np.argsort(key, kind="stable")
    ds = e_dstrow[ord_e]
    hs = is_hi[ord_e]
    first = np.r_[True, ds[1:] != ds[:-1]]
    grp_start = np.flatnonzero(first)
    grp_id = np.cumsum(first) - 1
    slot = np.arange(ds.shape[0]) - grp_start[grp_id]
    c = ds // pc
    j = (ds % pc) // 128
    ln = ds % 128
    pos = chunk_base[j] + np.where(hs, K_lo[j] + slot - lo_cnt[ds], slot)
    assert (pos >= chunk_base[j]).all() and (pos < chunk_base[j] + K[j]).all()

    idx_val = np.where(hs, e_srcrow[ord_e] - half, e_srcrow[ord_e])
    idx_stream = np.zeros((NCORES, 128, nch), np.int16)
    mask_stream = np.zeros((NCORES, 128, nch), np.float32)
    idx_stream[c, ln, pos] = idx_val.astype(np.int16)
    mask_stream[c, ln, pos] = 1.0

    # wrapped int16 layout: chunk k -> columns 8k:8k+8; within a chunk the
    # 128 lane-indices wrap as flat[i] -> [i % 16, i // 16], replicated over
    # the 8 16-partition groups (each SWDGE queue's core-pair reads its own).
    iw = idx_stream.transpose(0, 2, 1).reshape(NCORES, nch, 8, 16)
    iw = iw.transpose(0, 3, 1, 2).reshape(NCORES, 16, nch * 8)
    idx_wrapped = np.tile(iw, (1, 8, 1))

    return dict(vpad=vpad, pc=pc, nb=nb, half=half, K=K, K_lo=K_lo, K_hi=K_hi,
                nch=nch, chunk_base=chunk_base, row_of_node=row_of_node,
                idx_wrapped=np.ascontiguousarray(idx_wrapped),
                mask_stream=mask_stream)


def _sbuf_gather_rows(nc, out_ap, in_ap, idxs_ap, num_idxs, queue_num):
    """Non-transpose SBUF-source dma_gather (row layout, 264B elements).

    The bass wrapper restricts SBUF sources to transpose=True; the ucode's
    non-transpose src_is_sbuf path is complete and exact, so build the
    instruction directly.  Row r of the table sits at partition r%128,
    byte offset (r//128)*2*TROW of in_ap.
    """
    import concourse.mybir as mybir
    gp = nc.gpsimd
    return gp.add_instruction(
        mybir.InstDMAGatherAnt(
            name=nc.get_next_instruction_name(),
            ins=[
                gp.lower_ap(in_ap),
                gp.lower_ap(idxs_ap),
                gp.lower_val_access(gp.to_reg(num_idxs)),
            ],
            outs=[gp.lower_ap(out_ap)],
            transpose=False,
            num_idxs=num_idxs,
            elem_size=TROW,
            stride_bytes_256=0,
            gen_mode=0,
            single_packet=False,
            queue_num=queue_num,
            sbuf_tokens_per_rank=128,
            sbuf_free_dim_per_rank=2 * TROW,
            sbuf_free_dim_pad_per_rank=0,
            sbuf_byte_offset=0,
        )
    )


def _build_program(vpad, pc, nb, half, K, K_lo, K_hi, nch, chunk_base):
    import concourse.bacc as bacc
    import concourse.mybir as mybir
    import concourse.tile as tile

    F32 = mybir.dt.float32
    BF16 = mybir.dt.bfloat16
    I16 = mybir.dt.int16
    ACTF = mybir.ActivationFunctionType
    ALU = mybir.AluOpType
    AXL = mybir.AxisListType

    nstr = vpad // 128
    hstr = half // 128

    nc = bacc.Bacc("TRN2", target_bir_lowering=False, debug=False,
                   num_devices=NCORES, num_swdge_queues=4)

    xt_d = nc.dram_tensor("xt", [128, vpad], BF16, kind="ExternalInput")
    xtl_d = nc.dram_tensor("xtl", [128, pc], BF16, kind="ExternalInput")
    idx_d = nc.dram_tensor("idx", [128, nch * 8], I16, kind="ExternalInput")
    msk_d = nc.dram_tensor("msk", [128, nch], BF16, kind="ExternalInput")
    id_d = nc.dram_tensor("ident", [128, 128], BF16, kind="ExternalInput")
    w1_d = nc.dram_tensor("w1aug", [128, TROW], BF16, kind="ExternalInput")
    wad1_d = nc.dram_tensor("wad1", [128, 2], BF16, kind="ExternalInput")
    w2_d = nc.dram_tensor("w2", [128, 64], BF16, kind="ExternalInput")
    ws2_d = nc.dram_tensor("ws2", [128, 128], BF16, kind="ExternalInput")
    wd2_d = nc.dram_tensor("wd2", [128, 128], BF16, kind="ExternalInput")
    b1_d = nc.dram_tensor("b1rep", [128, 128], F32, kind="ExternalInput")
    b2_d = nc.dram_tensor("b2rep", [128, 64], F32, kind="ExternalInput")
    out_d = nc.dram_tensor("out", [pc, OUT], F32, kind="ExternalOutput")

    qn = [0]

    with tile.TileContext(nc) as tc:
        with (
            tc.tile_pool(name="const", bufs=1) as cp,
            tc.tile_pool(name="dram", bufs=1, space="DRAM") as dp,
            tc.tile_pool(name="xs", bufs=3) as xp,
            tc.tile_pool(name="pso", bufs=3, space="PSUM") as pso_pool,
            tc.tile_pool(name="pst", bufs=3, space="PSUM") as pst_pool,
            tc.tile_pool(name="g", bufs=3) as gp_pool,
            tc.tile_pool(name="v", bufs=3) as vp,
            tc.tile_pool(name="wz", bufs=4) as wp,
            tc.tile_pool(name="epi", bufs=3) as ep,
        ):
            table = cp.tile([128, nstr * TROW], BF16)
            idx_t = cp.tile([128, nch * 8], I16)
            msk_t = cp.tile([128, nch], BF16)
            ident = cp.tile([128, 128], BF16)
            w1_sb = cp.tile([128, TROW], BF16)
            wad1 = cp.tile([128, 2], BF16)
            w2_sb = cp.tile([128, 64], BF16)
            ws2 = cp.tile([128, 128], BF16)
            wd2 = cp.tile([128, 128], BF16)
            b1_sb = cp.tile([128, 128], F32)
            b2_sb = cp.tile([128, 64], F32)
            xtl = cp.tile([128, pc], BF16)
            a1_loc = cp.tile([128, nb * 2], F32)
            a2_loc = cp.tile([128, nb], F32)
            for t, d in ((idx_t, idx_d), (msk_t, msk_d), (ident, id_d),
                         (w1_sb, w1_d), (wad1, wad1_d), (w2_sb, w2_d),
                         (ws2, ws2_d), (wd2, wd2_d), (b1_sb, b1_d),
                         (b2_sb, b2_d), (xtl, xtl_d)):
                nc.sync.dma_start(out=t[:], in_=d[:])

            loc_dram = dp.tile([128, nb * TROW], BF16)
            ag_out = dp.tile([128 * NCORES, nb * TROW], BF16)

            # ---- adst1 for local blocks ----
            for j in range(nb):
                psa = pst_pool.tile([128, 2], F32, tag="pst")
                nc.tensor.matmul(psa[:], lhsT=xtl[:, j * 128:(j + 1) * 128],
                                 rhs=wad1[:], start=True, stop=True)
                nc.scalar.activation(a1_loc[:, j * 2:(j + 1) * 2], psa[:],
                                     ACTF.Copy)

            # ---- L1 table build: [x@W1 | x@W1@a_src] (full, replicated) ----
            for s in range(nstr):
                xpan = xp.tile([128, 128], BF16, tag="xt")
                nc.sync.dma_start(out=xpan[:], in_=xt_d[:, s * 128:(s + 1) * 128])
                psh = pst_pool.tile([128, TROW], F32, tag="pst")
                nc.tensor.matmul(psh[:], lhsT=xpan[:], rhs=w1_sb[:],
                                 start=True, stop=True)
                nc.scalar.activation(table[:, s * TROW:(s + 1) * TROW], psh[:],
                                     ACTF.Copy)

            def agg_layer(layer):
                if layer == 1:
                    aloc, H, vcols = a1_loc, 2, 130
                else:
                    aloc, H, vcols = a2_loc, 1, 129
                hd = 128 // H
                tab_lo = table[:, 0:hstr * TROW]
                tab_hi = table[:, hstr * TROW:nstr * TROW]
                for j in range(nb):
                    kj = int(K[j])
                    cb = int(chunk_base[j])
                    o1 = pso_pool.tile([128, vcols], F32, tag="pso")
                    batches = []
                    done = 0
                    for rl in (int(K_lo[j]), int(K_hi[j])):
                        r0 = done
                        while done < r0 + rl:
                            gl = min(GBATCH, r0 + rl - done)
                            batches.append((done, gl, done >= int(K_lo[j])))
                            done += gl
                    for (b0, gl, in_hi) in batches:
                        k0 = cb + b0
                        gt = gp_pool.tile([128, GBATCH * TROW], BF16, tag="g")
                        _sbuf_gather_rows(
                            nc,
                            gt[:, 0:gl * TROW].rearrange(
                                "p (k c) -> p k c", c=TROW),
                            tab_hi if in_hi else tab_lo,
                            idx_t[:, k0 * 8:(k0 + gl) * 8],
                            gl * 128, qn[0])
                        qn[0] = (qn[0] + 1) % 4
                        gv = gt[:, 0:gl * TROW].rearrange(
                            "p (k c) -> p k c", c=TROW)
                        vt = vp.tile([128, GBATCH * vcols], BF16, tag="v")
                        vv = vt[:, 0:gl * vcols].rearrange(
                            "p (k c) -> p k c", c=vcols)
                        zt = wp.tile([128, GBATCH * H], F32, tag="z")
                        zv = zt[:, 0:gl * H].rearrange("p (k h) -> p k h", h=H)
                        for h in range(H):
                            nc.scalar.activation(
                                zv[:, :, h], gv[:, :, 128 + h],
                                ACTF.Prelu,
                                bias=aloc[:, j * H + h:j * H + h + 1],
                                alpha=NEG_SLOPE)
                        nc.scalar.activation(zv[:, :, :], zv[:, :, :],
                                             ACTF.Exp)
                        nc.vector.tensor_tensor(
                            out=vv[:, :, H * hd:vcols], in0=zv[:, :, :],
                            in1=msk_t[:, k0:k0 + gl].to_broadcast(
                                [128, gl, H]),
                            op=ALU.mult)
                        for h in range(H):
                            nc.vector.tensor_tensor(
                                out=vv[:, :, h * hd:(h + 1) * hd],
                                in0=gv[:, :, h * hd:(h + 1) * hd],
                                in1=vv[:, :, H * hd + h:H * hd + h + 1]
                                    .to_broadcast([128, gl, hd]),
                                op=ALU.mult)
                        for i in range(gl):
                            nc.tensor.matmul(
                                o1[:], lhsT=ident[:],
                                rhs=vt[:, i * vcols:(i + 1) * vcols],
                                start=(b0 + i == 0), stop=(b0 + i == kj - 1))

                    # ---- epilogue ----
                    dsafe = wp.tile([128, H], F32, tag="ds")
                    nc.vector.tensor_scalar_add(dsafe[:], o1[:, H * hd:vcols],
                                                1e-30)
                    rden = wp.tile([128, H], F32, tag="rd")
                    nc.vector.reciprocal(rden[:], dsafe[:])
                    if layer == 1:
                        pre = ep.tile([128, 128], F32, tag="pre")
                        for h in range(H):
                            nc.scalar.activation(
                                pre[:, h * hd:(h + 1) * hd],
                                o1[:, h * hd:(h + 1) * hd],
                                ACTF.Copy, scale=rden[:, h:h + 1])
                        nc.vector.tensor_tensor(out=pre[:], in0=pre[:],
                                                in1=b1_sb[:], op=ALU.add)
                        row = ep.tile([128, TROW], BF16, tag="row")
                        nc.scalar.activation(row[:, 0:128], pre[:], ACTF.Relu)
                        # alpha_src2 / alpha_dst2 from relu(out1)
                        tmp2 = ep.tile([128, 128], F32, tag="tmp2")
                        red = wp.tile([128, 2], F32, tag="red")
                        nc.vector.tensor_tensor(out=tmp2[:], in0=row[:, 0:128],
                                                in1=ws2[:], op=ALU.mult)
                        nc.vector.tensor_reduce(
                            red[:, 0:1],
                            tmp2[:].rearrange("p (o c) -> p o c", o=1),
                            axis=AXL.X, op=ALU.add)
                        nc.vector.tensor_tensor(out=tmp2[:], in0=row[:, 0:128],
                                                in1=wd2[:], op=ALU.mult)
                        nc.vector.tensor_reduce(
                            red[:, 1:2],
                            tmp2[:].rearrange("p (o c) -> p o c", o=1),
                            axis=AXL.X, op=ALU.add)
                        nc.scalar.activation(row[:, 128:129], red[:, 0:1],
                                             ACTF.Copy)
                        nc.scalar.activation(a2_loc[:, j:j + 1], red[:, 1:2],
                                             ACTF.Copy)
                        nc.scalar.activation(row[:, 129:132], red[:, 0:1]
                                             .to_broadcast([128, 3]),
                                             ACTF.Copy)
                        nc.sync.dma_start(
                            out=loc_dram[:, j * TROW:(j + 1) * TROW],
                            in_=row[:])
                    else:
                        preb = ep.tile([128, 128], BF16, tag="preb")
                        nc.scalar.activation(preb[:], o1[:, 0:128], ACTF.Copy,
                                             scale=rden[:, 0:1])
                        pstr = pst_pool.tile([128, 128], F32, tag="pst")
                        nc.tensor.matmul(pstr[:], lhsT=preb[:], rhs=ident[:],
                                         start=True, stop=True)
                        preT = ep.tile([128, 128], BF16, tag="preT")
                        nc.scalar.activation(preT[:], pstr[:], ACTF.Copy)
                        ps2 = pst_pool.tile([128, 64], F32, tag="pst")
                        nc.tensor.matmul(ps2[:], lhsT=preT[:], rhs=w2_sb[:],
                                         start=True, stop=True)
                        ob = ep.tile([128, OUT], F32, tag="ob")
                        nc.vector.tensor_tensor(out=ob[:], in0=ps2[:],
                                                in1=b2_sb[:], op=ALU.add)
                        nc.scalar.activation(ob[:], ob[:], ACTF.Sigmoid)
                        nc.sync.dma_start(out=out_d[j * 128:(j + 1) * 128, :],
                                          in_=ob[:])

            agg_layer(1)

            # ---- AllGather [relu(out1) | asrc2] rows -> L2 table ----
            nc.gpsimd.collective_compute(
                "AllGather", mybir.AluOpType.bypass,
                replica_groups=[list(range(NCORES))],
                ins=[loc_dram[:]], outs=[ag_out[:]],
            )
            for cc in range(NCORES):
                nc.sync.dma_start(
                    out=table[:, cc * nb * TROW:(cc + 1) * nb * TROW],
                    in_=ag_out[cc * 128:(cc + 1) * 128, :])

            agg_layer(2)

    nc.finalize()
    return nc


def kernel(x, edge_index, W1, att_src1, att_dst1, b1, W2, att_src2, att_dst2,
           b2):
    from concourse import bass_utils

    x = np.asarray(x, np.float32)
    W1 = np.asarray(W1, np.float32)
    W2 = np.asarray(W2, np.float32)
    att_src1 = np.asarray(att_src1, np.float32)
    att_dst1 = np.asarray(att_dst1, np.float32)
    att_src2 = np.asarray(att_src2, np.float32)
    att_dst2 = np.asarray(att_dst2, np.float32)
    b1 = np.asarray(b1, np.float32)
    b2 = np.asarray(b2, np.float32)
    n_nodes = x.shape[0]

    sch = _build_schedule(edge_index, n_nodes)
    vpad, pc = sch["vpad"], sch["pc"]

    W1r = W1.reshape(F_IN, HEADS, HID)
    w1aug = np.zeros((128, TROW), np.float32)
    w1aug[:, 0:128] = W1
    w1aug[:, 128] = W1r[:, 0, :] @ att_src1[0]
    w1aug[:, 129] = W1r[:, 1, :] @ att_src1[1]
    wad1 = np.stack([W1r[:, h, :] @ att_dst1[h] for h in range(HEADS)], 1)
    ws2_t = np.broadcast_to(W2 @ att_src2[0], (128, 128)).copy()
    wd2_t = np.broadcast_to(W2 @ att_dst2[0], (128, 128)).copy()
    b1_rep = np.broadcast_to(b1, (128, HEADS * HID)).astype(np.float32).copy()
    b2_rep = np.broadcast_to(b2, (128, OUT)).astype(np.float32).copy()

    x_rho = np.zeros((vpad, F_IN), np.float32)
    x_rho[sch["row_of_node"]] = x
    xt_full = _bf16(np.ascontiguousarray(x_rho.T))          # [128, vpad]

    key = (vpad, sch["nch"], tuple(sch["K"].tolist()),
           tuple(sch["K_lo"].tolist()), DEBUG)
    if key not in _cache:
        _cache[key] = _build_program(vpad, pc, sch["nb"], sch["half"],
                                     sch["K"], sch["K_lo"], sch["K_hi"],
                                     sch["nch"], sch["chunk_base"])
    nc = _cache[key]

    in_maps = []
    for c in range(NCORES):
        in_maps.append({
            "xt": xt_full,
            "xtl": np.ascontiguousarray(xt_full[:, c * pc:(c + 1) * pc]),
            "idx": sch["idx_wrapped"][c],
            "msk": _bf16(sch["mask_stream"][c]),
            "ident": _bf16(np.eye(128, dtype=np.float32)),
            "w1aug": _bf16(w1aug),
            "wad1": _bf16(wad1),
            "w2": _bf16(W2),
            "ws2": _bf16(ws2_t),
            "wd2": _bf16(wd2_t),
            "b1rep": b1_rep,
            "b2rep": b2_rep,
        })
    res = bass_utils.run_bass_kernel_spmd(nc, in_maps,
                                          core_ids=list(range(NCORES)),
                                          trace=TRACE)
    kernel.last_exec_ns = res.exec_time_ns
    kernel.last_mean_ns = res.mean_exec_time_ns
    out_all = np.concatenate([res.results[c]["out"] for c in range(NCORES)], 0)
    return out_all[sch["row_of_node"][:n_nodes]]


# revision 10
# speedup vs baseline: 2.2534x; 1.0120x over previous
"""Two-layer GAT on 8 Trainium2 NeuronCores (Bass/Tile) — v3.

The per-edge gather tables live in SBUF as bf16 rows of 264B
([h (128) | alpha_src (1-2) | pad], 132 bf16) and are read with
non-transpose SBUF-source dma_gather on 4 rotating SWDGE queues: plain
SBUF->SBUF 264B descriptor pairs (no HBM latency, no XBAR), with descriptor
generation spread over all 8 Q7 cores.  The bass wrapper only allows
transpose=True for SBUF sources, so the instruction is constructed directly
(the non-transpose SBUF path exists in the ucode and is exact).  L1 table
rows are [x@W1 | x@W1@a_src] computed per-core from the replicated x (no
AllGather); L2 rows are [relu(out1) | relu(out1)@W2@a_src2] (W2 is applied
AFTER aggregation by linearity), so the only collective is a 1.7MB/core bf16
AllGather.  The slot-chunk schedule, lane-major V build, softmax by
reciprocal and identity-stationary segment-sum matmuls are from v1.
"""

import numpy as np

NCORES = 8
F_IN = 128
HID = 64
HEADS = 2
OUT = 64
NEG_SLOPE = 0.2
GBATCH = 8    # chunks per dma_gather call
TROW = 132    # table row: 132 bf16 = 264B

DEBUG = False
TRACE = False
_cache = {}


def _bf16(x):
    u = np.asarray(x, np.float32).view(np.uint32)
    r = ((u >> 16) & 1) + 0x7FFF
    return ((u + r) >> 16).astype(np.uint16)


def _build_schedule(edge_index, n_nodes):
    ei = np.asarray(edge_index).astype(np.int64)
    src = np.concatenate([ei[0], np.arange(n_nodes, dtype=np.int64)])
    dst = np.concatenate([ei[1], np.arange(n_nodes, dtype=np.int64)])
    deg = np.bincount(dst, minlength=n_nodes)

    stripe = NCORES * 128
    vpad = ((n_nodes + stripe - 1) // stripe) * stripe
    pc = vpad // NCORES
    nb = pc // 128
    half = vpad // 2
    assert half <= 32768

    degp = np.zeros(vpad, np.int64)
    degp[:n_nodes] = deg
    order = np.argsort(-degp, kind="stable")
    rank = np.empty(vpad, np.int64)
    rank[order] = np.arange(vpad)

    s = np.arange(vpad)
    g = s // 128
    lane = s % 128
    row_of_rank = (g % NCORES) * pc + (g // NCORES) * 128 + lane
    row_of_node = row_of_rank[rank[:n_nodes]]

    # Re-pack nodes within each core to minimize per-stripe slot maxima.
    # An edge's lo/hi class depends only on its source's CORE (cores 0-3 are
    # the lo half), which a within-core permutation preserves, so per-node
    # lo/hi counts are invariant under the re-packing.
    e_dstrow = row_of_node[dst]
    e_srcrow = row_of_node[src]
    is_hi = e_srcrow >= half
    lo_cnt = np.bincount(e_dstrow[~is_hi], minlength=vpad)
    hi_cnt = np.bincount(e_dstrow[is_hi], minlength=vpad)
    rfull = row_of_rank[rank]
    lo_n = lo_cnt[rfull]
    hi_n = hi_cnt[rfull]
    key = np.maximum(lo_n, hi_n) * 1000.0 + lo_n + hi_n
    core_of = rfull // pc
    new_row = np.empty(vpad, np.int64)
    for cc in range(NCORES):
        nodes = np.nonzero(core_of == cc)[0]
        o = nodes[np.argsort(-key[nodes], kind="stable")]
        new_row[o] = cc * pc + np.arange(pc)
    row_of_node = new_row[:n_nodes]

    e_dstrow = row_of_node[dst]
    e_srcrow = row_of_node[src]
    is_hi = e_srcrow >= half

    lo_cnt = np.bincount(e_dstrow[~is_hi], minlength=vpad)
    hi_cnt = np.bincount(e_dstrow[is_hi], minlength=vpad)

    jj = (np.arange(vpad) % pc) // 128
    K_lo = np.zeros(nb, np.int64)
    K_hi = np.zeros(nb, np.int64)
    np.maximum.at(K_lo, jj, lo_cnt)
    np.maximum.at(K_hi, jj, hi_cnt)
    K = K_lo + K_hi
    bump = K == 0
    K_lo[bump] += 1
    K[bump] += 1
    nch = int(K.sum())
    chunk_base = np.concatenate([[0], np.cumsum(K)])[:-1]

    key = e_dstrow * 2 + is_hi
    ord_e = np.argsort(key, kind="stable")
    ds = e_dstrow[ord_e]
    hs = is_hi[ord_e]
    first = np.r_[True, ds[1:] != ds[:-1]]
    grp_start = np.flatnonzero(first)
    grp_id = np.cumsum(first) - 1
    slot = np.arange(ds.shape[0]) - grp_start[grp_id]
    c = ds // pc
    j = (ds % pc) // 128
    ln = ds % 128
    pos = chunk_base[j] + np.where(hs, K_lo[j] + slot - lo_cnt[ds], slot)
    assert (pos >= chunk_base[j]).all() and (pos < chunk_base[j] + K[j]).all()

    idx_val = np.where(hs, e_srcrow[ord_e] - half, e_srcrow[ord_e])
    idx_stream = np.zeros((NCORES, 128, nch), np.int16)
    mask_stream = np.zeros((NCORES, 128, nch), np.float32)
    idx_stream[c, ln, pos] = idx_val.astype(np.int16)
    mask_stream[c, ln, pos] = 1.0

    # wrapped int16 layout: chunk k -> columns 8k:8k+8; within a chunk the
    # 128 lane-indices wrap as flat[i] -> [i % 16, i // 16], replicated over
    # the 8 16-partition groups (each SWDGE queue's core-pair reads its own).
    iw = idx_stream.transpose(0, 2, 1).reshape(NCORES, nch, 8, 16)
    iw = iw.transpose(0, 3, 1, 2).reshape(NCORES, 16, nch * 8)
    idx_wrapped = np.tile(iw, (1, 8, 1))

    return dict(vpad=vpad, pc=pc, nb=nb, half=half, K=K, K_lo=K_lo, K_hi=K_hi,
                nch=nch, chunk_base=chunk_base, row_of_node=row_of_node,
                idx_wrapped=np.ascontiguousarray(idx_wrapped),
                mask_stream=mask_stream)


def _sbuf_gather_rows(nc, out_ap, in_ap, idxs_ap, num_idxs, queue_num):
    """Non-transpose SBUF-source dma_gather (row layout, 264B elements).

    The bass wrapper restricts SBUF sources to transpose=True; the ucode's
    non-transpose src_is_sbuf path is complete and exact, so build the
    instruction directly.  Row r of the table sits at partition r%128,
    byte offset (r//128)*2*TROW of in_ap.
    """
    import concourse.mybir as mybir
    gp = nc.gpsimd
    return gp.add_instruction(
        mybir.InstDMAGatherAnt(
            name=nc.get_next_instruction_name(),
            ins=[
                gp.lower_ap(in_ap),
                gp.lower_ap(idxs_ap),
                gp.lower_val_access(gp.to_reg(num_idxs)),
            ],
            outs=[gp.lower_ap(out_ap)],
            transpose=False,
            num_idxs=num_idxs,
            elem_size=TROW,
            stride_bytes_256=0,
            gen_mode=0,
            single_packet=False,
            queue_num=queue_num,
            sbuf_tokens_per_rank=128,
            sbuf_free_dim_per_rank=2 * TROW,
            sbuf_free_dim_pad_per_rank=0,
            sbuf_byte_offset=0,
        )
    )


def _build_program(vpad, pc, nb, half, K, K_lo, K_hi, nch, chunk_base):
    import concourse.bacc as bacc
    import concourse.mybir as mybir
    import concourse.tile as tile

    F32 = mybir.dt.float32
    BF16 = mybir.dt.bfloat16
    I16 = mybir.dt.int16
    ACTF = mybir.ActivationFunctionType
    ALU = mybir.AluOpType
    AXL = mybir.AxisListType

    nstr = vpad // 128
    hstr = half // 128

    nc = bacc.Bacc("TRN2", target_bir_lowering=False, debug=False,
                   num_devices=NCORES, num_swdge_queues=4)

    xt_d = nc.dram_tensor("xt", [128, vpad], BF16, kind="ExternalInput")
    xtl_d = nc.dram_tensor("xtl", [128, pc], BF16, kind="ExternalInput")
    idx_d = nc.dram_tensor("idx", [128, nch * 8], I16, kind="ExternalInput")
    msk_d = nc.dram_tensor("msk", [128, nch], BF16, kind="ExternalInput")
    id_d = nc.dram_tensor("ident", [128, 128], BF16, kind="ExternalInput")
    w1_d = nc.dram_tensor("w1aug", [128, TROW], BF16, kind="ExternalInput")
    wad1_d = nc.dram_tensor("wad1", [128, 2], BF16, kind="ExternalInput")
    w2_d = nc.dram_tensor("w2", [128, 64], BF16, kind="ExternalInput")
    ws2_d = nc.dram_tensor("ws2", [128, 128], BF16, kind="ExternalInput")
    wd2_d = nc.dram_tensor("wd2", [128, 128], BF16, kind="ExternalInput")
    b1_d = nc.dram_tensor("b1rep", [128, 128], F32, kind="ExternalInput")
    b2_d = nc.dram_tensor("b2rep", [128, 64], F32, kind="ExternalInput")
    out_d = nc.dram_tensor("out", [pc, OUT], F32, kind="ExternalOutput")

    qn = [0]

    with tile.TileContext(nc) as tc:
        with (
            tc.tile_pool(name="const", bufs=1) as cp,
            tc.tile_pool(name="dram", bufs=1, space="DRAM") as dp,
            tc.tile_pool(name="xs", bufs=3) as xp,
            tc.tile_pool(name="pso", bufs=3, space="PSUM") as pso_pool,
            tc.tile_pool(name="pst", bufs=3, space="PSUM") as pst_pool,
            tc.tile_pool(name="g", bufs=3) as gp_pool,
            tc.tile_pool(name="v", bufs=3) as vp,
            tc.tile_pool(name="wz", bufs=4) as wp,
            tc.tile_pool(name="epi", bufs=3) as ep,
        ):
            table = cp.tile([128, nstr * TROW], BF16)
            idx_t = cp.tile([128, nch * 8], I16)
            msk_t = cp.tile([128, nch], BF16)
            ident = cp.tile([128, 128], BF16)
            w1_sb = cp.tile([128, TROW], BF16)
            wad1 = cp.tile([128, 2], BF16)
            w2_sb = cp.tile([128, 64], BF16)
            ws2 = cp.tile([128, 128], BF16)
            wd2 = cp.tile([128, 128], BF16)
            b1_sb = cp.tile([128, 128], F32)
            b2_sb = cp.tile([128, 64], F32)
            xtl = cp.tile([128, pc], BF16)
            a1_loc = cp.tile([128, nb * 2], F32)
            a2_loc = cp.tile([128, nb], F32)
            for t, d in ((idx_t, idx_d), (msk_t, msk_d), (ident, id_d),
                         (w1_sb, w1_d), (wad1, wad1_d), (w2_sb, w2_d),
                         (ws2, ws2_d), (wd2, wd2_d), (b1_sb, b1_d),
                         (b2_sb, b2_d), (xtl, xtl_d)):
                nc.sync.dma_start(out=t[:], in_=d[:])

            nbh = nb // 2 + 1          # 25 blocks in the first AG half
            loc1 = dp.tile([128, nbh * TROW], BF16)
            loc2 = dp.tile([128, (nb - nbh) * TROW], BF16)
            ag_out1 = dp.tile([128 * NCORES, nbh * TROW], BF16)
            ag_out2 = dp.tile([128 * NCORES, (nb - nbh) * TROW], BF16)

            # ---- adst1 for local blocks ----
            for j in range(nb):
                psa = pst_pool.tile([128, 2], F32, tag="pst")
                nc.tensor.matmul(psa[:], lhsT=xtl[:, j * 128:(j + 1) * 128],
                                 rhs=wad1[:], start=True, stop=True)
                nc.scalar.activation(a1_loc[:, j * 2:(j + 1) * 2], psa[:],
                                     ACTF.Copy)

            # ---- L1 table build: [x@W1 | x@W1@a_src] (full, replicated) ----
            SLAB = 8
            for s0 in range(0, nstr, SLAB):
                sn = min(SLAB, nstr - s0)
                xslab = xp.tile([128, SLAB * 128], BF16, tag="xt")
                nc.sync.dma_start(out=xslab[:, 0:sn * 128],
                                  in_=xt_d[:, s0 * 128:(s0 + sn) * 128])
                for i in range(sn):
                    s = s0 + i
                    psh = pst_pool.tile([128, TROW], F32, tag="pst")
                    nc.tensor.matmul(psh[:],
                                     lhsT=xslab[:, i * 128:(i + 1) * 128],
                                     rhs=w1_sb[:], start=True, stop=True)
                    nc.scalar.activation(table[:, s * TROW:(s + 1) * TROW],
                                         psh[:], ACTF.Copy)

            def agg_layer(layer):
                if layer == 1:
                    aloc, H, vcols = a1_loc, 2, 130
                else:
                    aloc, H, vcols = a2_loc, 1, 129
                hd = 128 // H
                tab_lo = table[:, 0:hstr * TROW]
                tab_hi = table[:, hstr * TROW:nstr * TROW]
                for j in range(nb):
                    kj = int(K[j])
                    cb = int(chunk_base[j])
                    o1 = pso_pool.tile([128, vcols], F32, tag="pso")
                    batches = []
                    done = 0
                    for rl in (int(K_lo[j]), int(K_hi[j])):
                        r0 = done
                        while done < r0 + rl:
                            gl = min(GBATCH, r0 + rl - done)
                            batches.append((done, gl, done >= int(K_lo[j])))
                            done += gl
                    for (b0, gl, in_hi) in batches:
                        k0 = cb + b0
                        gt = gp_pool.tile([128, GBATCH * TROW], BF16, tag="g")
                        _sbuf_gather_rows(
                            nc,
                            gt[:, 0:gl * TROW].rearrange(
                                "p (k c) -> p k c", c=TROW),
                            tab_hi if in_hi else tab_lo,
                            idx_t[:, k0 * 8:(k0 + gl) * 8],
                            gl * 128, qn[0])
                        qn[0] = (qn[0] + 1) % 4
                        gv = gt[:, 0:gl * TROW].rearrange(
                            "p (k c) -> p k c", c=TROW)
                        vt = vp.tile([128, GBATCH * vcols], BF16, tag="v")
                        vv = vt[:, 0:gl * vcols].rearrange(
                            "p (k c) -> p k c", c=vcols)
                        zt = wp.tile([128, GBATCH * H], F32, tag="z")
                        zv = zt[:, 0:gl * H].rearrange("p (k h) -> p k h", h=H)
                        for h in range(H):
                            nc.scalar.activation(
                                zv[:, :, h], gv[:, :, 128 + h],
                                ACTF.Prelu,
                                bias=aloc[:, j * H + h:j * H + h + 1],
                                alpha=NEG_SLOPE)
                        nc.scalar.activation(zv[:, :, :], zv[:, :, :],
                                             ACTF.Exp)
                        nc.vector.tensor_tensor(
                            out=vv[:, :, H * hd:vcols], in0=zv[:, :, :],
                            in1=msk_t[:, k0:k0 + gl].to_broadcast(
                                [128, gl, H]),
                            op=ALU.mult)
                        for h in range(H):
                            nc.vector.tensor_tensor(
                                out=vv[:, :, h * hd:(h + 1) * hd],
                                in0=gv[:, :, h * hd:(h + 1) * hd],
                                in1=vv[:, :, H * hd + h:H * hd + h + 1]
                                    .to_broadcast([128, gl, hd]),
                                op=ALU.mult)
                        for i in range(gl):
                            nc.tensor.matmul(
                                o1[:], lhsT=ident[:],
                                rhs=vt[:, i * vcols:(i + 1) * vcols],
                                start=(b0 + i == 0), stop=(b0 + i == kj - 1))

                    # ---- epilogue ----
                    dsafe = wp.tile([128, H], F32, tag="ds")
                    nc.vector.tensor_scalar_add(dsafe[:], o1[:, H * hd:vcols],
                                                1e-30)
                    rden = wp.tile([128, H], F32, tag="rd")
                    nc.vector.reciprocal(rden[:], dsafe[:])
                    if layer == 1:
                        pre = ep.tile([128, 128], F32, tag="pre")
                        for h in range(H):
                            nc.scalar.activation(
                                pre[:, h * hd:(h + 1) * hd],
                                o1[:, h * hd:(h + 1) * hd],
                                ACTF.Copy, scale=rden[:, h:h + 1])
                        nc.vector.tensor_tensor(out=pre[:], in0=pre[:],
                                                in1=b1_sb[:], op=ALU.add)
                        row = ep.tile([128, TROW], BF16, tag="row")
                        nc.scalar.activation(row[:, 0:128], pre[:], ACTF.Relu)
                        # alpha_src2 / alpha_dst2 from relu(out1)
                        tmp2 = ep.tile([128, 128], F32, tag="tmp2")
                        red = wp.tile([128, 2], F32, tag="red")
                        nc.vector.tensor_tensor(out=tmp2[:], in0=row[:, 0:128],
                                                in1=ws2[:], op=ALU.mult)
                        nc.vector.tensor_reduce(
                            red[:, 0:1],
                            tmp2[:].rearrange("p (o c) -> p o c", o=1),
                            axis=AXL.X, op=ALU.add)
                        nc.vector.tensor_tensor(out=tmp2[:], in0=row[:, 0:128],
                                                in1=wd2[:], op=ALU.mult)
                        nc.vector.tensor_reduce(
                            red[:, 1:2],
                            tmp2[:].rearrange("p (o c) -> p o c", o=1),
                            axis=AXL.X, op=ALU.add)
                        nc.scalar.activation(row[:, 128:129], red[:, 0:1],
                                             ACTF.Copy)
                        nc.scalar.activation(a2_loc[:, j:j + 1], red[:, 1:2],
                                             ACTF.Copy)
                        nc.scalar.activation(row[:, 129:132], red[:, 0:1]
                                             .to_broadcast([128, 3]),
                                             ACTF.Copy)
                        if j < nbh:
                            nc.sync.dma_start(
                                out=loc1[:, j * TROW:(j + 1) * TROW],
                                in_=row[:])
                        else:
                            nc.sync.dma_start(
                                out=loc2[:, (j - nbh) * TROW:
                                          (j - nbh + 1) * TROW],
                                in_=row[:])
                    else:
                        preb = ep.tile([128, 128], BF16, tag="preb")
                        nc.scalar.activation(preb[:], o1[:, 0:128], ACTF.Copy,
                                             scale=rden[:, 0:1])
                        pstr = pst_pool.tile([128, 128], F32, tag="pst")
                        nc.tensor.matmul(pstr[:], lhsT=preb[:], rhs=ident[:],
                                         start=True, stop=True)
                        preT = ep.tile([128, 128], BF16, tag="preT")
                        nc.scalar.activation(preT[:], pstr[:], ACTF.Copy)
                        ps2 = pst_pool.tile([128, 64], F32, tag="pst")
                        nc.tensor.matmul(ps2[:], lhsT=preT[:], rhs=w2_sb[:],
                                         start=True, stop=True)
                        ob = ep.tile([128, OUT], F32, tag="ob")
                        nc.vector.tensor_tensor(out=ob[:], in0=ps2[:],
                                                in1=b2_sb[:], op=ALU.add)
                        nc.scalar.activation(ob[:], ob[:], ACTF.Sigmoid)
                        nc.sync.dma_start(out=out_d[j * 128:(j + 1) * 128, :],
                                          in_=ob[:])

            agg_layer(1)

            # ---- AllGather [relu(out1) | asrc2] rows -> L2 table ----
            # Two collectives: the first covers blocks 0..nbh-1 and runs as
            # soon as those epilogues land, overlapping the L1 tail.
            nc.gpsimd.collective_compute(
                "AllGather", mybir.AluOpType.bypass,
                replica_groups=[list(range(NCORES))],
                ins=[loc1[:]], outs=[ag_out1[:]],
            )
            nc.gpsimd.collective_compute(
                "AllGather", mybir.AluOpType.bypass,
                replica_groups=[list(range(NCORES))],
                ins=[loc2[:]], outs=[ag_out2[:]],
            )
            for cc in range(NCORES):
                base = cc * nb * TROW
                nc.sync.dma_start(
                    out=table[:, base:base + nbh * TROW],
                    in_=ag_out1[cc * 128:(cc + 1) * 128, :])
                nc.sync.dma_start(
                    out=table[:, base + nbh * TROW:base + nb * TROW],
                    in_=ag_out2[cc * 128:(cc + 1) * 128, :])

            agg_layer(2)

    nc.finalize()
    return nc


def kernel(x, edge_index, W1, att_src1, att_dst1, b1, W2, att_src2, att_dst2,
           b2):
    from concourse import bass_utils

    x = np.asarray(x, np.float32)
    W1 = np.asarray(W1, np.float32)
    W2 = np.asarray(W2, np.float32)
    att_src1 = np.asarray(att_src1, np.float32)
    att_dst1 = np.asarray(att_dst1, np.float32)
    att_src2 = np.asarray(att_src2, np.float32)
    att_dst2 = np.asarray(att_dst2, np.float32)
    b1 = np.asarray(b1, np.float32)
    b2 = np.asarray(b2, np.float32)
    n_nodes = x.shape[0]

    sch = _build_schedule(edge_index, n_nodes)
    vpad, pc = sch["vpad"], sch["pc"]

    W1r = W1.reshape(F_IN, HEADS, HID)
    w1aug = np.zeros((128, TROW), np.float32)
    w1aug[:, 0:128] = W1
    w1aug[:, 128] = W1r[:, 0, :] @ att_src1[0]
    w1aug[:, 129] = W1r[:, 1, :] @ att_src1[1]
    wad1 = np.stack([W1r[:, h, :] @ att_dst1[h] for h in range(HEADS)], 1)
    ws2_t = np.broadcast_to(W2 @ att_src2[0], (128, 128)).copy()
    wd2_t = np.broadcast_to(W2 @ att_dst2[0], (128, 128)).copy()
    b1_rep = np.broadcast_to(b1, (128, HEADS * HID)).astype(np.float32).copy()
    b2_rep = np.broadcast_to(b2, (128, OUT)).astype(np.float32).copy()

    x_rho = np.zeros((vpad, F_IN), np.float32)
    x_rho[sch["row_of_node"]] = x
    xt_full = _bf16(np.ascontiguousarray(x_rho.T))          # [128, vpad]

    key = (vpad, sch["nch"], tuple(sch["K"].tolist()),
           tuple(sch["K_lo"].tolist()), DEBUG)
    if key not in _cache:
        _cache[key] = _build_program(vpad, pc, sch["nb"], sch["half"],
                                     sch["K"], sch["K_lo"], sch["K_hi"],
                                     sch["nch"], sch["chunk_base"])
    nc = _cache[key]

    in_maps = []
    for c in range(NCORES):
        in_maps.append({
            "xt": xt_full,
            "xtl": np.ascontiguousarray(xt_full[:, c * pc:(c + 1) * pc]),
            "idx": sch["idx_wrapped"][c],
            "msk": _bf16(sch["mask_stream"][c]),
            "ident": _bf16(np.eye(128, dtype=np.float32)),
            "w1aug": _bf16(w1aug),
            "wad1": _bf16(wad1),
            "w2": _bf16(W2),
            "ws2": _bf16(ws2_t),
            "wd2": _bf16(wd2_t),
            "b1rep": b1_rep,
            "b2rep": b2_rep,
        })
    res = bass_utils.run_bass_kernel_spmd(nc, in_maps,
                                          core_ids=list(range(NCORES)),
                                          trace=TRACE)
    kernel.last_exec_ns = res.exec_time_ns
    kernel.last_mean_ns = res.mean_exec_time_ns
    out_all = np.concatenate([res.results[c]["out"] for c in range(NCORES)], 0)
    return out_all[sch["row_of_node"][:n_nodes]]


# revision 11
# speedup vs baseline: 2.5157x; 1.1164x over previous
"""Two-layer GAT on 8 Trainium2 NeuronCores (Bass/Tile) — v3.

The per-edge gather tables live in SBUF as bf16 rows of 264B
([h (128) | alpha_src (1-2) | pad], 132 bf16) and are read with
non-transpose SBUF-source dma_gather on 4 rotating SWDGE queues: plain
SBUF->SBUF 264B descriptor pairs (no HBM latency, no XBAR), with descriptor
generation spread over all 8 Q7 cores.  The bass wrapper only allows
transpose=True for SBUF sources, so the instruction is constructed directly
(the non-transpose SBUF path exists in the ucode and is exact).  L1 table
rows are [x@W1 | x@W1@a_src] computed per-core from the replicated x (no
AllGather); L2 rows are [relu(out1) | relu(out1)@W2@a_src2] (W2 is applied
AFTER aggregation by linearity), so the only collective is a 1.7MB/core bf16
AllGather.  The slot-chunk schedule, lane-major V build, softmax by
reciprocal and identity-stationary segment-sum matmuls are from v1.
"""

import numpy as np

NCORES = 8
F_IN = 128
HID = 64
HEADS = 2
OUT = 64
NEG_SLOPE = 0.2
GBATCH = 16   # chunks per dma_gather call
TROW = 132    # table row: 132 bf16 = 264B

DEBUG = False
TRACE = False
_cache = {}


def _bf16(x):
    u = np.asarray(x, np.float32).view(np.uint32)
    r = ((u >> 16) & 1) + 0x7FFF
    return ((u + r) >> 16).astype(np.uint16)


def _build_schedule(edge_index, n_nodes):
    ei = np.asarray(edge_index).astype(np.int64)
    src = np.concatenate([ei[0], np.arange(n_nodes, dtype=np.int64)])
    dst = np.concatenate([ei[1], np.arange(n_nodes, dtype=np.int64)])
    deg = np.bincount(dst, minlength=n_nodes)

    stripe = NCORES * 128
    vpad = ((n_nodes + stripe - 1) // stripe) * stripe
    pc = vpad // NCORES
    nb = pc // 128
    half = vpad // 2
    assert half <= 32768

    degp = np.zeros(vpad, np.int64)
    degp[:n_nodes] = deg
    order = np.argsort(-degp, kind="stable")
    rank = np.empty(vpad, np.int64)
    rank[order] = np.arange(vpad)

    s = np.arange(vpad)
    g = s // 128
    lane = s % 128
    row_of_rank = (g % NCORES) * pc + (g // NCORES) * 128 + lane
    row_of_node = row_of_rank[rank[:n_nodes]]

    # Re-pack nodes within each core to minimize per-stripe slot maxima.
    # An edge's lo/hi class depends only on its source's CORE (cores 0-3 are
    # the lo half), which a within-core permutation preserves, so per-node
    # lo/hi counts are invariant under the re-packing.
    e_dstrow = row_of_node[dst]
    e_srcrow = row_of_node[src]
    is_hi = e_srcrow >= half
    lo_cnt = np.bincount(e_dstrow[~is_hi], minlength=vpad)
    hi_cnt = np.bincount(e_dstrow[is_hi], minlength=vpad)
    rfull = row_of_rank[rank]
    lo_n = lo_cnt[rfull]
    hi_n = hi_cnt[rfull]
    key = np.maximum(lo_n, hi_n) * 1000.0 + lo_n + hi_n
    core_of = rfull // pc
    new_row = np.empty(vpad, np.int64)
    for cc in range(NCORES):
        nodes = np.nonzero(core_of == cc)[0]
        o = nodes[np.argsort(-key[nodes], kind="stable")]
        new_row[o] = cc * pc + np.arange(pc)
    row_of_node = new_row[:n_nodes]

    e_dstrow = row_of_node[dst]
    e_srcrow = row_of_node[src]
    is_hi = e_srcrow >= half

    lo_cnt = np.bincount(e_dstrow[~is_hi], minlength=vpad)
    hi_cnt = np.bincount(e_dstrow[is_hi], minlength=vpad)

    jj = (np.arange(vpad) % pc) // 128
    K_lo = np.zeros(nb, np.int64)
    K_hi = np.zeros(nb, np.int64)
    np.maximum.at(K_lo, jj, lo_cnt)
    np.maximum.at(K_hi, jj, hi_cnt)
    K = K_lo + K_hi
    bump = K == 0
    K_lo[bump] += 1
    K[bump] += 1
    nch = int(K.sum())
    chunk_base = np.concatenate([[0], np.cumsum(K)])[:-1]

    key = e_dstrow * 2 + is_hi
    ord_e = np.argsort(key, kind="stable")
    ds = e_dstrow[ord_e]
    hs = is_hi[ord_e]
    first = np.r_[True, ds[1:] != ds[:-1]]
    grp_start = np.flatnonzero(first)
    grp_id = np.cumsum(first) - 1
    slot = np.arange(ds.shape[0]) - grp_start[grp_id]
    c = ds // pc
    j = (ds % pc) // 128
    ln = ds % 128
    pos = chunk_base[j] + np.where(hs, K_lo[j] + slot - lo_cnt[ds], slot)
    assert (pos >= chunk_base[j]).all() and (pos < chunk_base[j] + K[j]).all()

    idx_val = np.where(hs, e_srcrow[ord_e] - half, e_srcrow[ord_e])
    idx_stream = np.zeros((NCORES, 128, nch), np.int16)
    mask_stream = np.zeros((NCORES, 128, nch), np.float32)
    idx_stream[c, ln, pos] = idx_val.astype(np.int16)
    mask_stream[c, ln, pos] = 1.0

    # wrapped int16 layout: chunk k -> columns 8k:8k+8; within a chunk the
    # 128 lane-indices wrap as flat[i] -> [i % 16, i // 16], replicated over
    # the 8 16-partition groups (each SWDGE queue's core-pair reads its own).
    iw = idx_stream.transpose(0, 2, 1).reshape(NCORES, nch, 8, 16)
    iw = iw.transpose(0, 3, 1, 2).reshape(NCORES, 16, nch * 8)
    idx_wrapped = np.tile(iw, (1, 8, 1))

    return dict(vpad=vpad, pc=pc, nb=nb, half=half, K=K, K_lo=K_lo, K_hi=K_hi,
                nch=nch, chunk_base=chunk_base, row_of_node=row_of_node,
                idx_wrapped=np.ascontiguousarray(idx_wrapped),
                mask_stream=mask_stream)


def _sbuf_gather_rows(nc, out_ap, in_ap, idxs_ap, num_idxs, queue_num):
    """Non-transpose SBUF-source dma_gather (row layout, 264B elements).

    The bass wrapper restricts SBUF sources to transpose=True; the ucode's
    non-transpose src_is_sbuf path is complete and exact, so build the
    instruction directly.  Row r of the table sits at partition r%128,
    byte offset (r//128)*2*TROW of in_ap.
    """
    import concourse.mybir as mybir
    gp = nc.gpsimd
    return gp.add_instruction(
        mybir.InstDMAGatherAnt(
            name=nc.get_next_instruction_name(),
            ins=[
                gp.lower_ap(in_ap),
                gp.lower_ap(idxs_ap),
                gp.lower_val_access(gp.to_reg(num_idxs)),
            ],
            outs=[gp.lower_ap(out_ap)],
            transpose=False,
            num_idxs=num_idxs,
            elem_size=TROW,
            stride_bytes_256=0,
            gen_mode=0,
            single_packet=False,
            queue_num=queue_num,
            sbuf_tokens_per_rank=128,
            sbuf_free_dim_per_rank=2 * TROW,
            sbuf_free_dim_pad_per_rank=0,
            sbuf_byte_offset=0,
        )
    )


def _build_program(vpad, pc, nb, half, K, K_lo, K_hi, nch, chunk_base):
    import concourse.bacc as bacc
    import concourse.mybir as mybir
    import concourse.tile as tile

    F32 = mybir.dt.float32
    BF16 = mybir.dt.bfloat16
    I16 = mybir.dt.int16
    ACTF = mybir.ActivationFunctionType
    ALU = mybir.AluOpType
    AXL = mybir.AxisListType

    nstr = vpad // 128
    hstr = half // 128

    nc = bacc.Bacc("TRN2", target_bir_lowering=False, debug=False,
                   num_devices=NCORES, num_swdge_queues=4)

    xt_d = nc.dram_tensor("xt", [128, vpad], BF16, kind="ExternalInput")
    xtl_d = nc.dram_tensor("xtl", [128, pc], BF16, kind="ExternalInput")
    idx_d = nc.dram_tensor("idx", [128, nch * 8], I16, kind="ExternalInput")
    msk_d = nc.dram_tensor("msk", [128, nch], BF16, kind="ExternalInput")
    id_d = nc.dram_tensor("ident", [128, 128], BF16, kind="ExternalInput")
    w1_d = nc.dram_tensor("w1aug", [128, TROW], BF16, kind="ExternalInput")
    wad1_d = nc.dram_tensor("wad1", [128, 2], BF16, kind="ExternalInput")
    w2_d = nc.dram_tensor("w2", [128, 64], BF16, kind="ExternalInput")
    ws2_d = nc.dram_tensor("ws2", [128, 128], BF16, kind="ExternalInput")
    wd2_d = nc.dram_tensor("wd2", [128, 128], BF16, kind="ExternalInput")
    b1_d = nc.dram_tensor("b1rep", [128, 128], F32, kind="ExternalInput")
    b2_d = nc.dram_tensor("b2rep", [128, 64], F32, kind="ExternalInput")
    out_d = nc.dram_tensor("out", [pc, OUT], F32, kind="ExternalOutput")

    qn = [0]

    with tile.TileContext(nc) as tc:
        with (
            tc.tile_pool(name="const", bufs=1) as cp,
            tc.tile_pool(name="dram", bufs=1, space="DRAM") as dp,
            tc.tile_pool(name="xs", bufs=3) as xp,
            tc.tile_pool(name="pso", bufs=3, space="PSUM") as pso_pool,
            tc.tile_pool(name="pst", bufs=3, space="PSUM") as pst_pool,
            tc.tile_pool(name="g", bufs=3) as gp_pool,
            tc.tile_pool(name="v", bufs=3) as vp,
            tc.tile_pool(name="wz", bufs=4) as wp,
            tc.tile_pool(name="epi", bufs=3) as ep,
        ):
            table = cp.tile([128, nstr * TROW], BF16)
            idx_t = cp.tile([128, nch * 8], I16)
            msk_t = cp.tile([128, nch], BF16)
            ident = cp.tile([128, 128], BF16)
            w1_sb = cp.tile([128, TROW], BF16)
            wad1 = cp.tile([128, 2], BF16)
            w2_sb = cp.tile([128, 64], BF16)
            ws2 = cp.tile([128, 128], BF16)
            wd2 = cp.tile([128, 128], BF16)
            b1_sb = cp.tile([128, 128], F32)
            b2_sb = cp.tile([128, 64], F32)
            xtl = cp.tile([128, pc], BF16)
            a1_loc = cp.tile([128, nb * 2], F32)
            a2_loc = cp.tile([128, nb], F32)
            for t, d in ((idx_t, idx_d), (msk_t, msk_d), (ident, id_d),
                         (w1_sb, w1_d), (wad1, wad1_d), (w2_sb, w2_d),
                         (ws2, ws2_d), (wd2, wd2_d), (b1_sb, b1_d),
                         (b2_sb, b2_d), (xtl, xtl_d)):
                nc.sync.dma_start(out=t[:], in_=d[:])

            nbh = nb - 4               # first AG half: all but the last 4 blocks
            loc1 = dp.tile([128, nbh * TROW], BF16)
            loc2 = dp.tile([128, (nb - nbh) * TROW], BF16)
            ag_out1 = dp.tile([128 * NCORES, nbh * TROW], BF16)
            ag_out2 = dp.tile([128 * NCORES, (nb - nbh) * TROW], BF16)

            # ---- adst1 for local blocks ----
            for j in range(nb):
                psa = pst_pool.tile([128, 2], F32, tag="pst")
                nc.tensor.matmul(psa[:], lhsT=xtl[:, j * 128:(j + 1) * 128],
                                 rhs=wad1[:], start=True, stop=True)
                nc.scalar.activation(a1_loc[:, j * 2:(j + 1) * 2], psa[:],
                                     ACTF.Copy)

            # ---- L1 table build: [x@W1 | x@W1@a_src] (full, replicated) ----
            SLAB = 8
            for s0 in range(0, nstr, SLAB):
                sn = min(SLAB, nstr - s0)
                xslab = xp.tile([128, SLAB * 128], BF16, tag="xt")
                nc.sync.dma_start(out=xslab[:, 0:sn * 128],
                                  in_=xt_d[:, s0 * 128:(s0 + sn) * 128])
                for i in range(sn):
                    s = s0 + i
                    psh = pst_pool.tile([128, TROW], F32, tag="pst")
                    nc.tensor.matmul(psh[:],
                                     lhsT=xslab[:, i * 128:(i + 1) * 128],
                                     rhs=w1_sb[:], start=True, stop=True)
                    nc.scalar.activation(table[:, s * TROW:(s + 1) * TROW],
                                         psh[:], ACTF.Copy)

            def agg_layer(layer):
                if layer == 1:
                    aloc, H, vcols = a1_loc, 2, 130
                else:
                    aloc, H, vcols = a2_loc, 1, 129
                hd = 128 // H
                tab_lo = table[:, 0:hstr * TROW]
                tab_hi = table[:, hstr * TROW:nstr * TROW]
                for j in range(nb):
                    kj = int(K[j])
                    cb = int(chunk_base[j])
                    o1 = pso_pool.tile([128, vcols], F32, tag="pso")
                    batches = []
                    done = 0
                    for rl in (int(K_lo[j]), int(K_hi[j])):
                        r0 = done
                        while done < r0 + rl:
                            gl = min(GBATCH, r0 + rl - done)
                            batches.append((done, gl, done >= int(K_lo[j])))
                            done += gl
                    for (b0, gl, in_hi) in batches:
                        k0 = cb + b0
                        gt = gp_pool.tile([128, GBATCH * TROW], BF16, tag="g")
                        _sbuf_gather_rows(
                            nc,
                            gt[:, 0:gl * TROW].rearrange(
                                "p (k c) -> p k c", c=TROW),
                            tab_hi if in_hi else tab_lo,
                            idx_t[:, k0 * 8:(k0 + gl) * 8],
                            gl * 128, qn[0])
                        qn[0] = (qn[0] + 1) % 4
                        gv = gt[:, 0:gl * TROW].rearrange(
                            "p (k c) -> p k c", c=TROW)
                        vt = vp.tile([128, GBATCH * vcols], BF16, tag="v")
                        vv = vt[:, 0:gl * vcols].rearrange(
                            "p (k c) -> p k c", c=vcols)
                        zt = wp.tile([128, GBATCH * H], F32, tag="z")
                        zv = zt[:, 0:gl * H].rearrange("p (k h) -> p k h", h=H)
                        for h in range(H):
                            nc.scalar.activation(
                                zv[:, :, h], gv[:, :, 128 + h],
                                ACTF.Prelu,
                                bias=aloc[:, j * H + h:j * H + h + 1],
                                alpha=NEG_SLOPE)
                        nc.scalar.activation(zv[:, :, :], zv[:, :, :],
                                             ACTF.Exp)
                        nc.vector.tensor_tensor(
                            out=vv[:, :, H * hd:vcols], in0=zv[:, :, :],
                            in1=msk_t[:, k0:k0 + gl].to_broadcast(
                                [128, gl, H]),
                            op=ALU.mult)
                        for h in range(H):
                            nc.vector.tensor_tensor(
                                out=vv[:, :, h * hd:(h + 1) * hd],
                                in0=gv[:, :, h * hd:(h + 1) * hd],
                                in1=vv[:, :, H * hd + h:H * hd + h + 1]
                                    .to_broadcast([128, gl, hd]),
                                op=ALU.mult)
                        for i in range(gl):
                            nc.tensor.matmul(
                                o1[:], lhsT=ident[:],
                                rhs=vt[:, i * vcols:(i + 1) * vcols],
                                start=(b0 + i == 0), stop=(b0 + i == kj - 1))

                    # ---- epilogue ----
                    dsafe = wp.tile([128, H], F32, tag="ds")
                    nc.vector.tensor_scalar_add(dsafe[:], o1[:, H * hd:vcols],
                                                1e-30)
                    rden = wp.tile([128, H], F32, tag="rd")
                    nc.vector.reciprocal(rden[:], dsafe[:])
                    if layer == 1:
                        pre = ep.tile([128, 128], F32, tag="pre")
                        for h in range(H):
                            nc.scalar.activation(
                                pre[:, h * hd:(h + 1) * hd],
                                o1[:, h * hd:(h + 1) * hd],
                                ACTF.Copy, scale=rden[:, h:h + 1])
                        nc.vector.tensor_tensor(out=pre[:], in0=pre[:],
                                                in1=b1_sb[:], op=ALU.add)
                        row = ep.tile([128, TROW], BF16, tag="row")
                        nc.scalar.activation(row[:, 0:128], pre[:], ACTF.Relu)
                        # alpha_src2 / alpha_dst2 from relu(out1)
                        tmp2 = ep.tile([128, 128], F32, tag="tmp2")
                        red = wp.tile([128, 2], F32, tag="red")
                        nc.vector.tensor_tensor(out=tmp2[:], in0=row[:, 0:128],
                                                in1=ws2[:], op=ALU.mult)
                        nc.vector.tensor_reduce(
                            red[:, 0:1],
                            tmp2[:].rearrange("p (o c) -> p o c", o=1),
                            axis=AXL.X, op=ALU.add)
                        nc.vector.tensor_tensor(out=tmp2[:], in0=row[:, 0:128],
                                                in1=wd2[:], op=ALU.mult)
                        nc.vector.tensor_reduce(
                            red[:, 1:2],
                            tmp2[:].rearrange("p (o c) -> p o c", o=1),
                            axis=AXL.X, op=ALU.add)
                        nc.scalar.activation(row[:, 128:129], red[:, 0:1],
                                             ACTF.Copy)
                        nc.scalar.activation(a2_loc[:, j:j + 1], red[:, 1:2],
                                             ACTF.Copy)
                        nc.scalar.activation(row[:, 129:132], red[:, 0:1]
                                             .to_broadcast([128, 3]),
                                             ACTF.Copy)
                        if j < nbh:
                            nc.sync.dma_start(
                                out=loc1[:, j * TROW:(j + 1) * TROW],
                                in_=row[:])
                        else:
                            nc.sync.dma_start(
                                out=loc2[:, (j - nbh) * TROW:
                                          (j - nbh + 1) * TROW],
                                in_=row[:])
                    else:
                        preb = ep.tile([128, 128], BF16, tag="preb")
                        nc.scalar.activation(preb[:], o1[:, 0:128], ACTF.Copy,
                                             scale=rden[:, 0:1])
                        pstr = pst_pool.tile([128, 128], F32, tag="pst")
                        nc.tensor.matmul(pstr[:], lhsT=preb[:], rhs=ident[:],
                                         start=True, stop=True)
                        preT = ep.tile([128, 128], BF16, tag="preT")
                        nc.scalar.activation(preT[:], pstr[:], ACTF.Copy)
                        ps2 = pst_pool.tile([128, 64], F32, tag="pst")
                        nc.tensor.matmul(ps2[:], lhsT=preT[:], rhs=w2_sb[:],
                                         start=True, stop=True)
                        ob = ep.tile([128, OUT], F32, tag="ob")
                        nc.vector.tensor_tensor(out=ob[:], in0=ps2[:],
                                                in1=b2_sb[:], op=ALU.add)
                        nc.scalar.activation(ob[:], ob[:], ACTF.Sigmoid)
                        nc.sync.dma_start(out=out_d[j * 128:(j + 1) * 128, :],
                                          in_=ob[:])

            agg_layer(1)

            # ---- AllGather [relu(out1) | asrc2] rows -> L2 table ----
            # Two collectives: the first covers blocks 0..nbh-1 and runs as
            # soon as those epilogues land, overlapping the L1 tail.
            nc.gpsimd.collective_compute(
                "AllGather", mybir.AluOpType.bypass,
                replica_groups=[list(range(NCORES))],
                ins=[loc1[:]], outs=[ag_out1[:]],
            )
            nc.gpsimd.collective_compute(
                "AllGather", mybir.AluOpType.bypass,
                replica_groups=[list(range(NCORES))],
                ins=[loc2[:]], outs=[ag_out2[:]],
            )
            for cc in range(NCORES):
                base = cc * nb * TROW
                nc.sync.dma_start(
                    out=table[:, base:base + nbh * TROW],
                    in_=ag_out1[cc * 128:(cc + 1) * 128, :])
                nc.sync.dma_start(
                    out=table[:, base + nbh * TROW:base + nb * TROW],
                    in_=ag_out2[cc * 128:(cc + 1) * 128, :])

            agg_layer(2)

    nc.finalize()
    return nc


def kernel(x, edge_index, W1, att_src1, att_dst1, b1, W2, att_src2, att_dst2,
           b2):
    from concourse import bass_utils

    x = np.asarray(x, np.float32)
    W1 = np.asarray(W1, np.float32)
    W2 = np.asarray(W2, np.float32)
    att_src1 = np.asarray(att_src1, np.float32)
    att_dst1 = np.asarray(att_dst1, np.float32)
    att_src2 = np.asarray(att_src2, np.float32)
    att_dst2 = np.asarray(att_dst2, np.float32)
    b1 = np.asarray(b1, np.float32)
    b2 = np.asarray(b2, np.float32)
    n_nodes = x.shape[0]

    sch = _build_schedule(edge_index, n_nodes)
    vpad, pc = sch["vpad"], sch["pc"]

    W1r = W1.reshape(F_IN, HEADS, HID)
    w1aug = np.zeros((128, TROW), np.float32)
    w1aug[:, 0:128] = W1
    w1aug[:, 128] = W1r[:, 0, :] @ att_src1[0]
    w1aug[:, 129] = W1r[:, 1, :] @ att_src1[1]
    wad1 = np.stack([W1r[:, h, :] @ att_dst1[h] for h in range(HEADS)], 1)
    ws2_t = np.broadcast_to(W2 @ att_src2[0], (128, 128)).copy()
    wd2_t = np.broadcast_to(W2 @ att_dst2[0], (128, 128)).copy()
    b1_rep = np.broadcast_to(b1, (128, HEADS * HID)).astype(np.float32).copy()
    b2_rep = np.broadcast_to(b2, (128, OUT)).astype(np.float32).copy()

    x_rho = np.zeros((vpad, F_IN), np.float32)
    x_rho[sch["row_of_node"]] = x
    xt_full = _bf16(np.ascontiguousarray(x_rho.T))          # [128, vpad]

    key = (vpad, sch["nch"], tuple(sch["K"].tolist()),
           tuple(sch["K_lo"].tolist()), DEBUG)
    if key not in _cache:
        _cache[key] = _build_program(vpad, pc, sch["nb"], sch["half"],
                                     sch["K"], sch["K_lo"], sch["K_hi"],
                                     sch["nch"], sch["chunk_base"])
    nc = _cache[key]

    in_maps = []
    for c in range(NCORES):
        in_maps.append({
            "xt": xt_full,
            "xtl": np.ascontiguousarray(xt_full[:, c * pc:(c + 1) * pc]),
            "idx": sch["idx_wrapped"][c],
            "msk": _bf16(sch["mask_stream"][c]),
            "ident": _bf16(np.eye(128, dtype=np.float32)),
            "w1aug": _bf16(w1aug),
            "wad1": _bf16(wad1),
            "w2": _bf16(W2),
            "ws2": _bf16(ws2_t),
            "wd2": _bf16(wd2_t),
            "b1rep": b1_rep,
            "b2rep": b2_rep,
        })
    res = bass_utils.run_bass_kernel_spmd(nc, in_maps,
                                          core_ids=list(range(NCORES)),
                                          trace=TRACE)
    kernel.last_exec_ns = res.exec_time_ns
    kernel.last_mean_ns = res.mean_exec_time_ns
    out_all = np.concatenate([res.results[c]["out"] for c in range(NCORES)], 0)
    return out_all[sch["row_of_node"][:n_nodes]]


# revision 12
# speedup vs baseline: 2.5581x; 1.0168x over previous
"""Two-layer GAT on 8 Trainium2 NeuronCores (Bass/Tile) — v3.

The per-edge gather tables live in SBUF as bf16 rows of 264B
([h (128) | alpha_src (1-2) | pad], 132 bf16) and are read with
non-transpose SBUF-source dma_gather on 4 rotating SWDGE queues: plain
SBUF->SBUF 264B descriptor pairs (no HBM latency, no XBAR), with descriptor
generation spread over all 8 Q7 cores.  The bass wrapper only allows
transpose=True for SBUF sources, so the instruction is constructed directly
(the non-transpose SBUF path exists in the ucode and is exact).  L1 table
rows are [x@W1 | x@W1@a_src] computed per-core from the replicated x (no
AllGather); L2 rows are [relu(out1) | relu(out1)@W2@a_src2] (W2 is applied
AFTER aggregation by linearity), so the only collective is a 1.7MB/core bf16
AllGather.  The slot-chunk schedule, lane-major V build, softmax by
reciprocal and identity-stationary segment-sum matmuls are from v1.
"""

import numpy as np

NCORES = 8
F_IN = 128
HID = 64
HEADS = 2
OUT = 64
NEG_SLOPE = 0.2
GBATCH = 16   # chunks per dma_gather call
TROW = 132    # table row: 132 bf16 = 264B

DEBUG = False
TRACE = False
_cache = {}


def _bf16(x):
    u = np.asarray(x, np.float32).view(np.uint32)
    r = ((u >> 16) & 1) + 0x7FFF
    return ((u + r) >> 16).astype(np.uint16)


def _build_schedule(edge_index, n_nodes):
    ei = np.asarray(edge_index).astype(np.int64)
    src = np.concatenate([ei[0], np.arange(n_nodes, dtype=np.int64)])
    dst = np.concatenate([ei[1], np.arange(n_nodes, dtype=np.int64)])
    deg = np.bincount(dst, minlength=n_nodes)

    stripe = NCORES * 128
    vpad = ((n_nodes + stripe - 1) // stripe) * stripe
    pc = vpad // NCORES
    nb = pc // 128
    half = vpad // 2
    assert half <= 32768

    degp = np.zeros(vpad, np.int64)
    degp[:n_nodes] = deg
    order = np.argsort(-degp, kind="stable")
    rank = np.empty(vpad, np.int64)
    rank[order] = np.arange(vpad)

    s = np.arange(vpad)
    g = s // 128
    lane = s % 128
    row_of_rank = (g % NCORES) * pc + (g // NCORES) * 128 + lane
    row_of_node = row_of_rank[rank[:n_nodes]]

    # Re-pack nodes within each core to minimize per-stripe slot maxima.
    # An edge's lo/hi class depends only on its source's CORE (cores 0-3 are
    # the lo half), which a within-core permutation preserves, so per-node
    # lo/hi counts are invariant under the re-packing.
    e_dstrow = row_of_node[dst]
    e_srcrow = row_of_node[src]
    is_hi = e_srcrow >= half
    lo_cnt = np.bincount(e_dstrow[~is_hi], minlength=vpad)
    hi_cnt = np.bincount(e_dstrow[is_hi], minlength=vpad)
    rfull = row_of_rank[rank]
    lo_n = lo_cnt[rfull]
    hi_n = hi_cnt[rfull]
    key = np.maximum(lo_n, hi_n) * 1000.0 + lo_n + hi_n
    core_of = rfull // pc
    new_row = np.empty(vpad, np.int64)
    for cc in range(NCORES):
        nodes = np.nonzero(core_of == cc)[0]
        o = nodes[np.argsort(-key[nodes], kind="stable")]
        new_row[o] = cc * pc + np.arange(pc)
    row_of_node = new_row[:n_nodes]

    e_dstrow = row_of_node[dst]
    e_srcrow = row_of_node[src]
    is_hi = e_srcrow >= half

    lo_cnt = np.bincount(e_dstrow[~is_hi], minlength=vpad)
    hi_cnt = np.bincount(e_dstrow[is_hi], minlength=vpad)

    jj = (np.arange(vpad) % pc) // 128
    K_lo = np.zeros(nb, np.int64)
    K_hi = np.zeros(nb, np.int64)
    np.maximum.at(K_lo, jj, lo_cnt)
    np.maximum.at(K_hi, jj, hi_cnt)
    K = K_lo + K_hi
    bump = K == 0
    K_lo[bump] += 1
    K[bump] += 1
    nch = int(K.sum())
    chunk_base = np.concatenate([[0], np.cumsum(K)])[:-1]

    key = e_dstrow * 2 + is_hi
    ord_e = np.argsort(key, kind="stable")
    ds = e_dstrow[ord_e]
    hs = is_hi[ord_e]
    first = np.r_[True, ds[1:] != ds[:-1]]
    grp_start = np.flatnonzero(first)
    grp_id = np.cumsum(first) - 1
    slot = np.arange(ds.shape[0]) - grp_start[grp_id]
    c = ds // pc
    j = (ds % pc) // 128
    ln = ds % 128
    pos = chunk_base[j] + np.where(hs, K_lo[j] + slot - lo_cnt[ds], slot)
    assert (pos >= chunk_base[j]).all() and (pos < chunk_base[j] + K[j]).all()

    idx_val = np.where(hs, e_srcrow[ord_e] - half, e_srcrow[ord_e])
    idx_stream = np.zeros((NCORES, 128, nch), np.int16)
    mask_stream = np.zeros((NCORES, 128, nch), np.float32)
    idx_stream[c, ln, pos] = idx_val.astype(np.int16)
    mask_stream[c, ln, pos] = 1.0

    # wrapped int16 layout: chunk k -> columns 8k:8k+8; within a chunk the
    # 128 lane-indices wrap as flat[i] -> [i % 16, i // 16], replicated over
    # the 8 16-partition groups (each SWDGE queue's core-pair reads its own).
    iw = idx_stream.transpose(0, 2, 1).reshape(NCORES, nch, 8, 16)
    iw = iw.transpose(0, 3, 1, 2).reshape(NCORES, 16, nch * 8)
    idx_wrapped = np.tile(iw, (1, 8, 1))

    return dict(vpad=vpad, pc=pc, nb=nb, half=half, K=K, K_lo=K_lo, K_hi=K_hi,
                nch=nch, chunk_base=chunk_base, row_of_node=row_of_node,
                idx_wrapped=np.ascontiguousarray(idx_wrapped),
                mask_stream=mask_stream)


def _sbuf_gather_rows(nc, out_ap, in_ap, idxs_ap, num_idxs, queue_num):
    """Non-transpose SBUF-source dma_gather (row layout, 264B elements).

    The bass wrapper restricts SBUF sources to transpose=True; the ucode's
    non-transpose src_is_sbuf path is complete and exact, so build the
    instruction directly.  Row r of the table sits at partition r%128,
    byte offset (r//128)*2*TROW of in_ap.
    """
    import concourse.mybir as mybir
    gp = nc.gpsimd
    return gp.add_instruction(
        mybir.InstDMAGatherAnt(
            name=nc.get_next_instruction_name(),
            ins=[
                gp.lower_ap(in_ap),
                gp.lower_ap(idxs_ap),
                gp.lower_val_access(gp.to_reg(num_idxs)),
            ],
            outs=[gp.lower_ap(out_ap)],
            transpose=False,
            num_idxs=num_idxs,
            elem_size=TROW,
            stride_bytes_256=0,
            gen_mode=0,
            single_packet=False,
            queue_num=queue_num,
            sbuf_tokens_per_rank=128,
            sbuf_free_dim_per_rank=2 * TROW,
            sbuf_free_dim_pad_per_rank=0,
            sbuf_byte_offset=0,
        )
    )


def _build_program(vpad, pc, nb, half, K, K_lo, K_hi, nch, chunk_base):
    import concourse.bacc as bacc
    import concourse.mybir as mybir
    import concourse.tile as tile

    F32 = mybir.dt.float32
    BF16 = mybir.dt.bfloat16
    I16 = mybir.dt.int16
    ACTF = mybir.ActivationFunctionType
    ALU = mybir.AluOpType
    AXL = mybir.AxisListType

    nstr = vpad // 128
    hstr = half // 128

    nc = bacc.Bacc("TRN2", target_bir_lowering=False, debug=False,
                   num_devices=NCORES, num_swdge_queues=4)

    xt_d = nc.dram_tensor("xt", [128, vpad], BF16, kind="ExternalInput")
    xtl_d = nc.dram_tensor("xtl", [128, pc], BF16, kind="ExternalInput")
    idx_d = nc.dram_tensor("idx", [128, nch * 8], I16, kind="ExternalInput")
    msk_d = nc.dram_tensor("msk", [128, nch], BF16, kind="ExternalInput")
    id_d = nc.dram_tensor("ident", [128, 128], BF16, kind="ExternalInput")
    w1_d = nc.dram_tensor("w1aug", [128, TROW], BF16, kind="ExternalInput")
    wad1_d = nc.dram_tensor("wad1", [128, 2], BF16, kind="ExternalInput")
    w2_d = nc.dram_tensor("w2", [128, 64], BF16, kind="ExternalInput")
    ws2_d = nc.dram_tensor("ws2", [128, 128], BF16, kind="ExternalInput")
    wd2_d = nc.dram_tensor("wd2", [128, 128], BF16, kind="ExternalInput")
    b1_d = nc.dram_tensor("b1rep", [128, 128], F32, kind="ExternalInput")
    b2_d = nc.dram_tensor("b2rep", [128, 64], F32, kind="ExternalInput")
    out_d = nc.dram_tensor("out", [pc, OUT], F32, kind="ExternalOutput")

    qn = [0]

    with tile.TileContext(nc) as tc:
        with (
            tc.tile_pool(name="const", bufs=1) as cp,
            tc.tile_pool(name="dram", bufs=1, space="DRAM") as dp,
            tc.tile_pool(name="xs", bufs=3) as xp,
            tc.tile_pool(name="pso", bufs=3, space="PSUM") as pso_pool,
            tc.tile_pool(name="pst", bufs=3, space="PSUM") as pst_pool,
            tc.tile_pool(name="g", bufs=3) as gp_pool,
            tc.tile_pool(name="v", bufs=3) as vp,
            tc.tile_pool(name="wz", bufs=4) as wp,
            tc.tile_pool(name="epi", bufs=3) as ep,
        ):
            table = cp.tile([128, nstr * TROW], BF16)
            idx_t = cp.tile([128, nch * 8], I16)
            msk_t = cp.tile([128, nch], BF16)
            ident = cp.tile([128, 128], BF16)
            w1_sb = cp.tile([128, TROW], BF16)
            wad1 = cp.tile([128, 2], BF16)
            w2_sb = cp.tile([128, 64], BF16)
            ws2 = cp.tile([128, 128], BF16)
            wd2 = cp.tile([128, 128], BF16)
            b1_sb = cp.tile([128, 128], F32)
            b2_sb = cp.tile([128, 64], F32)
            xtl = cp.tile([128, pc], BF16)
            a1_loc = cp.tile([128, nb * 2], F32)
            a2_loc = cp.tile([128, nb], F32)
            for t, d in ((idx_t, idx_d), (msk_t, msk_d), (ident, id_d),
                         (w1_sb, w1_d), (wad1, wad1_d), (w2_sb, w2_d),
                         (ws2, ws2_d), (wd2, wd2_d), (b1_sb, b1_d),
                         (b2_sb, b2_d), (xtl, xtl_d)):
                nc.sync.dma_start(out=t[:], in_=d[:])

            nbh = nb - 4               # first AG half: all but the last 4 blocks
            loc1 = dp.tile([128, nbh * TROW], BF16)
            loc2 = dp.tile([128, (nb - nbh) * TROW], BF16)
            ag_out1 = dp.tile([128 * NCORES, nbh * TROW], BF16)
            ag_out2 = dp.tile([128 * NCORES, (nb - nbh) * TROW], BF16)

            # ---- adst1 for local blocks ----
            for j in range(nb):
                psa = pst_pool.tile([128, 2], F32, tag="pst")
                nc.tensor.matmul(psa[:], lhsT=xtl[:, j * 128:(j + 1) * 128],
                                 rhs=wad1[:], start=True, stop=True)
                nc.scalar.activation(a1_loc[:, j * 2:(j + 1) * 2], psa[:],
                                     ACTF.Copy)

            # ---- L1 table build: [x@W1 | x@W1@a_src] (full, replicated) ----
            SLAB = 8
            for s0 in range(0, nstr, SLAB):
                sn = min(SLAB, nstr - s0)
                xslab = xp.tile([128, SLAB * 128], BF16, tag="xt")
                nc.sync.dma_start(out=xslab[:, 0:sn * 128],
                                  in_=xt_d[:, s0 * 128:(s0 + sn) * 128])
                for i in range(sn):
                    s = s0 + i
                    psh = pst_pool.tile([128, TROW], F32, tag="pst")
                    nc.tensor.matmul(psh[:],
                                     lhsT=xslab[:, i * 128:(i + 1) * 128],
                                     rhs=w1_sb[:], start=True, stop=True)
                    nc.scalar.activation(table[:, s * TROW:(s + 1) * TROW],
                                         psh[:], ACTF.Copy)

            def agg_layer(layer, post_block=None):
                if layer == 1:
                    aloc, H, vcols = a1_loc, 2, 130
                else:
                    aloc, H, vcols = a2_loc, 1, 129
                hd = 128 // H
                tab_lo = table[:, 0:hstr * TROW]
                tab_hi = table[:, hstr * TROW:nstr * TROW]
                for j in range(nb):
                    kj = int(K[j])
                    cb = int(chunk_base[j])
                    o1 = pso_pool.tile([128, vcols], F32, tag="pso")
                    batches = []
                    done = 0
                    for rl in (int(K_lo[j]), int(K_hi[j])):
                        r0 = done
                        while done < r0 + rl:
                            gl = min(GBATCH, r0 + rl - done)
                            batches.append((done, gl, done >= int(K_lo[j])))
                            done += gl
                    for (b0, gl, in_hi) in batches:
                        k0 = cb + b0
                        gt = gp_pool.tile([128, GBATCH * TROW], BF16, tag="g")
                        _sbuf_gather_rows(
                            nc,
                            gt[:, 0:gl * TROW].rearrange(
                                "p (k c) -> p k c", c=TROW),
                            tab_hi if in_hi else tab_lo,
                            idx_t[:, k0 * 8:(k0 + gl) * 8],
                            gl * 128, qn[0])
                        qn[0] = (qn[0] + 1) % 4
                        gv = gt[:, 0:gl * TROW].rearrange(
                            "p (k c) -> p k c", c=TROW)
                        vt = vp.tile([128, GBATCH * vcols], BF16, tag="v")
                        vv = vt[:, 0:gl * vcols].rearrange(
                            "p (k c) -> p k c", c=vcols)
                        zt = wp.tile([128, GBATCH * H], F32, tag="z")
                        zv = zt[:, 0:gl * H].rearrange("p (k h) -> p k h", h=H)
                        for h in range(H):
                            nc.scalar.activation(
                                zv[:, :, h], gv[:, :, 128 + h],
                                ACTF.Prelu,
                                bias=aloc[:, j * H + h:j * H + h + 1],
                                alpha=NEG_SLOPE)
                        nc.scalar.activation(zv[:, :, :], zv[:, :, :],
                                             ACTF.Exp)
                        nc.vector.tensor_tensor(
                            out=vv[:, :, H * hd:vcols], in0=zv[:, :, :],
                            in1=msk_t[:, k0:k0 + gl].to_broadcast(
                                [128, gl, H]),
                            op=ALU.mult)
                        for h in range(H):
                            nc.vector.tensor_tensor(
                                out=vv[:, :, h * hd:(h + 1) * hd],
                                in0=gv[:, :, h * hd:(h + 1) * hd],
                                in1=vv[:, :, H * hd + h:H * hd + h + 1]
                                    .to_broadcast([128, gl, hd]),
                                op=ALU.mult)
                        for i in range(gl):
                            nc.tensor.matmul(
                                o1[:], lhsT=ident[:],
                                rhs=vt[:, i * vcols:(i + 1) * vcols],
                                start=(b0 + i == 0), stop=(b0 + i == kj - 1))

                    # ---- epilogue ----
                    dsafe = wp.tile([128, H], F32, tag="ds")
                    nc.vector.tensor_scalar_add(dsafe[:], o1[:, H * hd:vcols],
                                                1e-30)
                    rden = wp.tile([128, H], F32, tag="rd")
                    nc.vector.reciprocal(rden[:], dsafe[:])
                    if layer == 1:
                        pre = ep.tile([128, 128], F32, tag="pre")
                        for h in range(H):
                            nc.scalar.activation(
                                pre[:, h * hd:(h + 1) * hd],
                                o1[:, h * hd:(h + 1) * hd],
                                ACTF.Copy, scale=rden[:, h:h + 1])
                        nc.vector.tensor_tensor(out=pre[:], in0=pre[:],
                                                in1=b1_sb[:], op=ALU.add)
                        row = ep.tile([128, TROW], BF16, tag="row")
                        nc.scalar.activation(row[:, 0:128], pre[:], ACTF.Relu)
                        # alpha_src2 / alpha_dst2 from relu(out1)
                        tmp2 = ep.tile([128, 128], F32, tag="tmp2")
                        red = wp.tile([128, 2], F32, tag="red")
                        nc.vector.tensor_tensor(out=tmp2[:], in0=row[:, 0:128],
                                                in1=ws2[:], op=ALU.mult)
                        nc.vector.tensor_reduce(
                            red[:, 0:1],
                            tmp2[:].rearrange("p (o c) -> p o c", o=1),
                            axis=AXL.X, op=ALU.add)
                        nc.vector.tensor_tensor(out=tmp2[:], in0=row[:, 0:128],
                                                in1=wd2[:], op=ALU.mult)
                        nc.vector.tensor_reduce(
                            red[:, 1:2],
                            tmp2[:].rearrange("p (o c) -> p o c", o=1),
                            axis=AXL.X, op=ALU.add)
                        nc.scalar.activation(row[:, 128:129], red[:, 0:1],
                                             ACTF.Copy)
                        nc.scalar.activation(a2_loc[:, j:j + 1], red[:, 1:2],
                                             ACTF.Copy)
                        nc.scalar.activation(row[:, 129:132], red[:, 0:1]
                                             .to_broadcast([128, 3]),
                                             ACTF.Copy)
                        if j < nbh:
                            nc.sync.dma_start(
                                out=loc1[:, j * TROW:(j + 1) * TROW],
                                in_=row[:])
                        else:
                            nc.sync.dma_start(
                                out=loc2[:, (j - nbh) * TROW:
                                          (j - nbh + 1) * TROW],
                                in_=row[:])
                    else:
                        preb = ep.tile([128, 128], BF16, tag="preb")
                        nc.scalar.activation(preb[:], o1[:, 0:128], ACTF.Copy,
                                             scale=rden[:, 0:1])
                        pstr = pst_pool.tile([128, 128], F32, tag="pst")
                        nc.tensor.matmul(pstr[:], lhsT=preb[:], rhs=ident[:],
                                         start=True, stop=True)
                        preT = ep.tile([128, 128], BF16, tag="preT")
                        nc.scalar.activation(preT[:], pstr[:], ACTF.Copy)
                        ps2 = pst_pool.tile([128, 64], F32, tag="pst")
                        nc.tensor.matmul(ps2[:], lhsT=preT[:], rhs=w2_sb[:],
                                         start=True, stop=True)
                        ob = ep.tile([128, OUT], F32, tag="ob")
                        nc.vector.tensor_tensor(out=ob[:], in0=ps2[:],
                                                in1=b2_sb[:], op=ALU.add)
                        nc.scalar.activation(ob[:], ob[:], ACTF.Sigmoid)
                        nc.sync.dma_start(out=out_d[j * 128:(j + 1) * 128, :],
                                          in_=ob[:])
                    if post_block is not None:
                        post_block(j)

            def l1_post(j):
                # Fire each AllGather as soon as its source blocks are done,
                # so it overlaps the remaining L1 aggregation in program
                # order on the GpSimd engine.
                if j == nbh - 1:
                    nc.gpsimd.collective_compute(
                        "AllGather", mybir.AluOpType.bypass,
                        replica_groups=[list(range(NCORES))],
                        ins=[loc1[:]], outs=[ag_out1[:]],
                    )
                elif j == nb - 1:
                    nc.gpsimd.collective_compute(
                        "AllGather", mybir.AluOpType.bypass,
                        replica_groups=[list(range(NCORES))],
                        ins=[loc2[:]], outs=[ag_out2[:]],
                    )

            agg_layer(1, post_block=l1_post)

            # ---- load the AllGathered [relu(out1) | asrc2] rows as L2 table
            for cc in range(NCORES):
                base = cc * nb * TROW
                nc.sync.dma_start(
                    out=table[:, base:base + nbh * TROW],
                    in_=ag_out1[cc * 128:(cc + 1) * 128, :])
                nc.sync.dma_start(
                    out=table[:, base + nbh * TROW:base + nb * TROW],
                    in_=ag_out2[cc * 128:(cc + 1) * 128, :])

            agg_layer(2)

    nc.finalize()
    return nc


def kernel(x, edge_index, W1, att_src1, att_dst1, b1, W2, att_src2, att_dst2,
           b2):
    from concourse import bass_utils

    x = np.asarray(x, np.float32)
    W1 = np.asarray(W1, np.float32)
    W2 = np.asarray(W2, np.float32)
    att_src1 = np.asarray(att_src1, np.float32)
    att_dst1 = np.asarray(att_dst1, np.float32)
    att_src2 = np.asarray(att_src2, np.float32)
    att_dst2 = np.asarray(att_dst2, np.float32)
    b1 = np.asarray(b1, np.float32)
    b2 = np.asarray(b2, np.float32)
    n_nodes = x.shape[0]

    sch = _build_schedule(edge_index, n_nodes)
    vpad, pc = sch["vpad"], sch["pc"]

    W1r = W1.reshape(F_IN, HEADS, HID)
    w1aug = np.zeros((128, TROW), np.float32)
    w1aug[:, 0:128] = W1
    w1aug[:, 128] = W1r[:, 0, :] @ att_src1[0]
    w1aug[:, 129] = W1r[:, 1, :] @ att_src1[1]
    wad1 = np.stack([W1r[:, h, :] @ att_dst1[h] for h in range(HEADS)], 1)
    ws2_t = np.broadcast_to(W2 @ att_src2[0], (128, 128)).copy()
    wd2_t = np.broadcast_to(W2 @ att_dst2[0], (128, 128)).copy()
    b1_rep = np.broadcast_to(b1, (128, HEADS * HID)).astype(np.float32).copy()
    b2_rep = np.broadcast_to(b2, (128, OUT)).astype(np.float32).copy()

    x_rho = np.zeros((vpad, F_IN), np.float32)
    x_rho[sch["row_of_node"]] = x
    xt_full = _bf16(np.ascontiguousarray(x_rho.T))          # [128, vpad]

    key = (vpad, sch["nch"], tuple(sch["K"].tolist()),
           tuple(sch["K_lo"].tolist()), DEBUG)
    if key not in _cache:
        _cache[key] = _build_program(vpad, pc, sch["nb"], sch["half"],
                                     sch["K"], sch["K_lo"], sch["K_hi"],
                                     sch["nch"], sch["chunk_base"])
    nc = _cache[key]

    in_maps = []
    for c in range(NCORES):
        in_maps.append({
            "xt": xt_full,
            "xtl": np.ascontiguousarray(xt_full[:, c * pc:(c + 1) * pc]),
            "idx": sch["idx_wrapped"][c],
            "msk": _bf16(sch["mask_stream"][c]),
            "ident": _bf16(np.eye(128, dtype=np.float32)),
            "w1aug": _bf16(w1aug),
            "wad1": _bf16(wad1),
            "w2": _bf16(W2),
            "ws2": _bf16(ws2_t),
            "wd2": _bf16(wd2_t),
            "b1rep": b1_rep,
            "b2rep": b2_rep,
        })
    res = bass_utils.run_bass_kernel_spmd(nc, in_maps,
                                          core_ids=list(range(NCORES)),
                                          trace=TRACE)
    kernel.last_exec_ns = res.exec_time_ns
    kernel.last_mean_ns = res.mean_exec_time_ns
    out_all = np.concatenate([res.results[c]["out"] for c in range(NCORES)], 0)
    return out_all[sch["row_of_node"][:n_nodes]]


# revision 13
# speedup vs baseline: 3.0597x; 1.1961x over previous
"""Two-layer GAT on 8 Trainium2 NeuronCores (Bass/Tile) — v3.

The per-edge gather tables live in SBUF as bf16 rows of 264B
([h (128) | alpha_src (1-2) | pad], 132 bf16) and are read with
non-transpose SBUF-source dma_gather on 4 rotating SWDGE queues: plain
SBUF->SBUF 264B descriptor pairs (no HBM latency, no XBAR), with descriptor
generation spread over all 8 Q7 cores.  The bass wrapper only allows
transpose=True for SBUF sources, so the instruction is constructed directly
(the non-transpose SBUF path exists in the ucode and is exact).  L1 table
rows are [x@W1 | x@W1@a_src] computed per-core from the replicated x (no
AllGather); L2 rows are [relu(out1) | relu(out1)@W2@a_src2] (W2 is applied
AFTER aggregation by linearity), so the only collective is a 1.7MB/core bf16
AllGather.  The slot-chunk schedule, lane-major V build, softmax by
reciprocal and identity-stationary segment-sum matmuls are from v1.
"""

import numpy as np

NCORES = 8
F_IN = 128
HID = 64
HEADS = 2
OUT = 64
NEG_SLOPE = 0.2
GBATCH = 16   # chunks per dma_gather call
TROW = 132    # table row: 132 bf16 = 264B

DEBUG = False
TRACE = False
_cache = {}


def _bf16(x):
    u = np.asarray(x, np.float32).view(np.uint32)
    r = ((u >> 16) & 1) + 0x7FFF
    return ((u + r) >> 16).astype(np.uint16)


def _build_schedule(edge_index, n_nodes):
    ei = np.asarray(edge_index).astype(np.int64)
    src = np.concatenate([ei[0], np.arange(n_nodes, dtype=np.int64)])
    dst = np.concatenate([ei[1], np.arange(n_nodes, dtype=np.int64)])
    deg = np.bincount(dst, minlength=n_nodes)

    stripe = NCORES * 128
    vpad = ((n_nodes + stripe - 1) // stripe) * stripe
    pc = vpad // NCORES
    nb = pc // 128
    half = vpad // 2
    assert half <= 32768

    degp = np.zeros(vpad, np.int64)
    degp[:n_nodes] = deg
    order = np.argsort(-degp, kind="stable")
    rank = np.empty(vpad, np.int64)
    rank[order] = np.arange(vpad)

    s = np.arange(vpad)
    g = s // 128
    lane = s % 128
    row_of_rank = (g % NCORES) * pc + (g // NCORES) * 128 + lane
    row_of_node = row_of_rank[rank[:n_nodes]]

    # Re-pack nodes within each core to minimize per-stripe slot maxima.
    # An edge's lo/hi class depends only on its source's CORE (cores 0-3 are
    # the lo half), which a within-core permutation preserves, so per-node
    # lo/hi counts are invariant under the re-packing.
    e_dstrow = row_of_node[dst]
    e_srcrow = row_of_node[src]
    is_hi = e_srcrow >= half
    lo_cnt = np.bincount(e_dstrow[~is_hi], minlength=vpad)
    hi_cnt = np.bincount(e_dstrow[is_hi], minlength=vpad)
    rfull = row_of_rank[rank]
    lo_n = lo_cnt[rfull]
    hi_n = hi_cnt[rfull]
    key = np.maximum(lo_n, hi_n) * 1000.0 + lo_n + hi_n
    core_of = rfull // pc
    new_row = np.empty(vpad, np.int64)
    for cc in range(NCORES):
        nodes = np.nonzero(core_of == cc)[0]
        o = nodes[np.argsort(-key[nodes], kind="stable")]
        new_row[o] = cc * pc + np.arange(pc)
    row_of_node = new_row[:n_nodes]

    e_dstrow = row_of_node[dst]
    e_srcrow = row_of_node[src]
    is_hi = e_srcrow >= half

    lo_cnt = np.bincount(e_dstrow[~is_hi], minlength=vpad)
    hi_cnt = np.bincount(e_dstrow[is_hi], minlength=vpad)

    jj = (np.arange(vpad) % pc) // 128
    K_lo = np.zeros(nb, np.int64)
    K_hi = np.zeros(nb, np.int64)
    np.maximum.at(K_lo, jj, lo_cnt)
    np.maximum.at(K_hi, jj, hi_cnt)
    K = K_lo + K_hi
    bump = K == 0
    K_lo[bump] += 1
    K[bump] += 1
    nch = int(K.sum())
    chunk_base = np.concatenate([[0], np.cumsum(K)])[:-1]

    key = e_dstrow * 2 + is_hi
    ord_e = np.argsort(key, kind="stable")
    ds = e_dstrow[ord_e]
    hs = is_hi[ord_e]
    first = np.r_[True, ds[1:] != ds[:-1]]
    grp_start = np.flatnonzero(first)
    grp_id = np.cumsum(first) - 1
    slot = np.arange(ds.shape[0]) - grp_start[grp_id]
    c = ds // pc
    j = (ds % pc) // 128
    ln = ds % 128
    pos = chunk_base[j] + np.where(hs, K_lo[j] + slot - lo_cnt[ds], slot)
    assert (pos >= chunk_base[j]).all() and (pos < chunk_base[j] + K[j]).all()

    idx_val = np.where(hs, e_srcrow[ord_e] - half, e_srcrow[ord_e])
    idx_stream = np.zeros((NCORES, 128, nch), np.int16)
    mask_stream = np.zeros((NCORES, 128, nch), np.float32)
    idx_stream[c, ln, pos] = idx_val.astype(np.int16)
    mask_stream[c, ln, pos] = 1.0

    # wrapped int16 layout: chunk k -> columns 8k:8k+8; within a chunk the
    # 128 lane-indices wrap as flat[i] -> [i % 16, i // 16], replicated over
    # the 8 16-partition groups (each SWDGE queue's core-pair reads its own).
    iw = idx_stream.transpose(0, 2, 1).reshape(NCORES, nch, 8, 16)
    iw = iw.transpose(0, 3, 1, 2).reshape(NCORES, 16, nch * 8)
    idx_wrapped = np.tile(iw, (1, 8, 1))

    return dict(vpad=vpad, pc=pc, nb=nb, half=half, K=K, K_lo=K_lo, K_hi=K_hi,
                nch=nch, chunk_base=chunk_base, row_of_node=row_of_node,
                idx_wrapped=np.ascontiguousarray(idx_wrapped),
                mask_stream=mask_stream)


def _sbuf_gather_rows(nc, out_ap, in_ap, idxs_ap, num_idxs, queue_num):
    """Non-transpose SBUF-source dma_gather (row layout, 264B elements).

    The bass wrapper restricts SBUF sources to transpose=True; the ucode's
    non-transpose src_is_sbuf path is complete and exact, so build the
    instruction directly.  Row r of the table sits at partition r%128,
    byte offset (r//128)*2*TROW of in_ap.
    """
    import concourse.mybir as mybir
    gp = nc.gpsimd
    return gp.add_instruction(
        mybir.InstDMAGatherAnt(
            name=nc.get_next_instruction_name(),
            ins=[
                gp.lower_ap(in_ap),
                gp.lower_ap(idxs_ap),
                gp.lower_val_access(gp.to_reg(num_idxs)),
            ],
            outs=[gp.lower_ap(out_ap)],
            transpose=False,
            num_idxs=num_idxs,
            elem_size=TROW,
            stride_bytes_256=0,
            gen_mode=0,
            single_packet=False,
            queue_num=queue_num,
            sbuf_tokens_per_rank=128,
            sbuf_free_dim_per_rank=2 * TROW,
            sbuf_free_dim_pad_per_rank=0,
            sbuf_byte_offset=0,
        )
    )


def _build_program(vpad, pc, nb, half, K, K_lo, K_hi, nch, chunk_base):
    import concourse.bacc as bacc
    import concourse.mybir as mybir
    import concourse.tile as tile

    F32 = mybir.dt.float32
    BF16 = mybir.dt.bfloat16
    I16 = mybir.dt.int16
    ACTF = mybir.ActivationFunctionType
    ALU = mybir.AluOpType
    AXL = mybir.AxisListType

    nstr = vpad // 128
    hstr = half // 128

    nc = bacc.Bacc("TRN2", target_bir_lowering=False, debug=False,
                   num_devices=NCORES, num_swdge_queues=4)

    xt_d = nc.dram_tensor("xt", [128, vpad], BF16, kind="ExternalInput")
    xtl_d = nc.dram_tensor("xtl", [128, pc], BF16, kind="ExternalInput")
    idx_d = nc.dram_tensor("idx", [128, nch * 8], I16, kind="ExternalInput")
    msk_d = nc.dram_tensor("msk", [128, nch], BF16, kind="ExternalInput")
    id_d = nc.dram_tensor("ident", [128, 128], BF16, kind="ExternalInput")
    w1_d = nc.dram_tensor("w1aug", [128, TROW], BF16, kind="ExternalInput")
    wad1_d = nc.dram_tensor("wad1", [128, 2], BF16, kind="ExternalInput")
    w2_d = nc.dram_tensor("w2", [128, 64], BF16, kind="ExternalInput")
    ws2_d = nc.dram_tensor("ws2", [128, 128], BF16, kind="ExternalInput")
    wd2_d = nc.dram_tensor("wd2", [128, 128], BF16, kind="ExternalInput")
    b1_d = nc.dram_tensor("b1rep", [128, 128], F32, kind="ExternalInput")
    b2_d = nc.dram_tensor("b2rep", [128, 64], F32, kind="ExternalInput")
    out_d = nc.dram_tensor("out", [pc, OUT], F32, kind="ExternalOutput")

    qn = [0]

    with tile.TileContext(nc) as tc:
        with (
            tc.tile_pool(name="const", bufs=1) as cp,
            tc.tile_pool(name="dram", bufs=1, space="DRAM") as dp,
            tc.tile_pool(name="xs", bufs=4) as xp,
            tc.tile_pool(name="pso", bufs=3, space="PSUM") as pso_pool,
            tc.tile_pool(name="pst", bufs=3, space="PSUM") as pst_pool,
            tc.tile_pool(name="g", bufs=4) as gp_pool,
            tc.tile_pool(name="v", bufs=4) as vp,
            tc.tile_pool(name="wz", bufs=4) as wp,
            tc.tile_pool(name="epi", bufs=3) as ep,
        ):
            table = cp.tile([128, nstr * TROW], BF16)
            idx_t = cp.tile([128, nch * 8], I16)
            msk_t = cp.tile([128, nch], BF16)
            ident = cp.tile([128, 128], BF16)
            w1_sb = cp.tile([128, TROW], BF16)
            wad1 = cp.tile([128, 2], BF16)
            w2_sb = cp.tile([128, 64], BF16)
            ws2 = cp.tile([128, 128], BF16)
            wd2 = cp.tile([128, 128], BF16)
            b1_sb = cp.tile([128, 128], F32)
            b2_sb = cp.tile([128, 64], F32)
            xtl = cp.tile([128, pc], BF16)
            a1_loc = cp.tile([128, nb * 2], F32)
            a2_loc = cp.tile([128, nb], F32)
            for t, d in ((idx_t, idx_d), (msk_t, msk_d), (ident, id_d),
                         (w1_sb, w1_d), (wad1, wad1_d), (w2_sb, w2_d),
                         (ws2, ws2_d), (wd2, wd2_d), (b1_sb, b1_d),
                         (b2_sb, b2_d), (xtl, xtl_d)):
                nc.sync.dma_start(out=t[:], in_=d[:])

            nbh = nb - 4               # first AG half: all but the last 4 blocks
            loc1 = dp.tile([128, nbh * TROW], BF16)
            loc2 = dp.tile([128, (nb - nbh) * TROW], BF16)
            ag_out1 = dp.tile([128 * NCORES, nbh * TROW], BF16)
            ag_out2 = dp.tile([128 * NCORES, (nb - nbh) * TROW], BF16)

            # ---- L1 table build: [x@W1 | x@W1@a_src] (full, replicated) ----
            SLAB = 8
            for s0 in range(0, nstr, SLAB):
                sn = min(SLAB, nstr - s0)
                xslab = xp.tile([128, SLAB * 128], BF16, tag="xt")
                nc.sync.dma_start(out=xslab[:, 0:sn * 128],
                                  in_=xt_d[:, s0 * 128:(s0 + sn) * 128])
                for i in range(sn):
                    s = s0 + i
                    psh = pst_pool.tile([128, TROW], F32, tag="pst")
                    nc.tensor.matmul(psh[:],
                                     lhsT=xslab[:, i * 128:(i + 1) * 128],
                                     rhs=w1_sb[:], start=True, stop=True)
                    nc.scalar.activation(table[:, s * TROW:(s + 1) * TROW],
                                         psh[:], ACTF.Copy)

            # ---- adst1 for local blocks (after the table build: its ACT
            # copies must not delay the table copies gating the first gather)
            for j in range(nb):
                psa = pst_pool.tile([128, 2], F32, tag="pst")
                nc.tensor.matmul(psa[:], lhsT=xtl[:, j * 128:(j + 1) * 128],
                                 rhs=wad1[:], start=True, stop=True)
                nc.scalar.activation(a1_loc[:, j * 2:(j + 1) * 2], psa[:],
                                     ACTF.Copy)

            def agg_layer(layer, post_block=None):
                if layer == 1:
                    aloc, H, vcols = a1_loc, 2, 130
                else:
                    aloc, H, vcols = a2_loc, 1, 129
                hd = 128 // H
                tab_lo = table[:, 0:hstr * TROW]
                tab_hi = table[:, hstr * TROW:nstr * TROW]
                for j in range(nb):
                    kj = int(K[j])
                    cb = int(chunk_base[j])
                    o1 = pso_pool.tile([128, vcols], F32, tag="pso")
                    batches = []
                    done = 0
                    for rl in (int(K_lo[j]), int(K_hi[j])):
                        r0 = done
                        while done < r0 + rl:
                            gl = min(GBATCH, r0 + rl - done)
                            batches.append((done, gl, done >= int(K_lo[j])))
                            done += gl
                    for (b0, gl, in_hi) in batches:
                        k0 = cb + b0
                        gt = gp_pool.tile([128, GBATCH * TROW], BF16, tag="g")
                        _sbuf_gather_rows(
                            nc,
                            gt[:, 0:gl * TROW].rearrange(
                                "p (k c) -> p k c", c=TROW),
                            tab_hi if in_hi else tab_lo,
                            idx_t[:, k0 * 8:(k0 + gl) * 8],
                            gl * 128, qn[0])
                        qn[0] = (qn[0] + 1) % 4
                        gv = gt[:, 0:gl * TROW].rearrange(
                            "p (k c) -> p k c", c=TROW)
                        vt = vp.tile([128, GBATCH * vcols], BF16, tag="v")
                        vv = vt[:, 0:gl * vcols].rearrange(
                            "p (k c) -> p k c", c=vcols)
                        zt = wp.tile([128, GBATCH * H], F32, tag="z")
                        zv = zt[:, 0:gl * H].rearrange("p (k h) -> p k h", h=H)
                        for h in range(H):
                            nc.scalar.activation(
                                zv[:, :, h], gv[:, :, 128 + h],
                                ACTF.Prelu,
                                bias=aloc[:, j * H + h:j * H + h + 1],
                                alpha=NEG_SLOPE)
                        nc.scalar.activation(zv[:, :, :], zv[:, :, :],
                                             ACTF.Exp)
                        nc.vector.tensor_tensor(
                            out=vv[:, :, H * hd:vcols], in0=zv[:, :, :],
                            in1=msk_t[:, k0:k0 + gl].to_broadcast(
                                [128, gl, H]),
                            op=ALU.mult)
                        for h in range(H):
                            nc.vector.tensor_tensor(
                                out=vv[:, :, h * hd:(h + 1) * hd],
                                in0=gv[:, :, h * hd:(h + 1) * hd],
                                in1=vv[:, :, H * hd + h:H * hd + h + 1]
                                    .to_broadcast([128, gl, hd]),
                                op=ALU.mult)
                        for i in range(gl):
                            nc.tensor.matmul(
                                o1[:], lhsT=ident[:],
                                rhs=vt[:, i * vcols:(i + 1) * vcols],
                                start=(b0 + i == 0), stop=(b0 + i == kj - 1))

                    # ---- epilogue ----
                    dsafe = wp.tile([128, H], F32, tag="ds")
                    nc.vector.tensor_scalar_add(dsafe[:], o1[:, H * hd:vcols],
                                                1e-30)
                    rden = wp.tile([128, H], F32, tag="rd")
                    nc.vector.reciprocal(rden[:], dsafe[:])
                    if layer == 1:
                        pre = ep.tile([128, 128], F32, tag="pre")
                        for h in range(H):
                            nc.scalar.activation(
                                pre[:, h * hd:(h + 1) * hd],
                                o1[:, h * hd:(h + 1) * hd],
                                ACTF.Copy, scale=rden[:, h:h + 1])
                        nc.vector.tensor_tensor(out=pre[:], in0=pre[:],
                                                in1=b1_sb[:], op=ALU.add)
                        row = ep.tile([128, TROW], BF16, tag="row")
                        nc.scalar.activation(row[:, 0:128], pre[:], ACTF.Relu)
                        # alpha_src2 / alpha_dst2 from relu(out1)
                        tmp2 = ep.tile([128, 128], F32, tag="tmp2")
                        red = wp.tile([128, 2], F32, tag="red")
                        nc.vector.tensor_tensor(out=tmp2[:], in0=row[:, 0:128],
                                                in1=ws2[:], op=ALU.mult)
                        nc.vector.tensor_reduce(
                            red[:, 0:1],
                            tmp2[:].rearrange("p (o c) -> p o c", o=1),
                            axis=AXL.X, op=ALU.add)
                        nc.vector.tensor_tensor(out=tmp2[:], in0=row[:, 0:128],
                                                in1=wd2[:], op=ALU.mult)
                        nc.vector.tensor_reduce(
                            red[:, 1:2],
                            tmp2[:].rearrange("p (o c) -> p o c", o=1),
                            axis=AXL.X, op=ALU.add)
                        nc.scalar.activation(row[:, 128:129], red[:, 0:1],
                                             ACTF.Copy)
                        nc.scalar.activation(a2_loc[:, j:j + 1], red[:, 1:2],
                                             ACTF.Copy)
                        nc.scalar.activation(row[:, 129:132], red[:, 0:1]
                                             .to_broadcast([128, 3]),
                                             ACTF.Copy)
                        if j < nbh:
                            nc.sync.dma_start(
                                out=loc1[:, j * TROW:(j + 1) * TROW],
                                in_=row[:])
                        else:
                            nc.sync.dma_start(
                                out=loc2[:, (j - nbh) * TROW:
                                          (j - nbh + 1) * TROW],
                                in_=row[:])
                    else:
                        preb = ep.tile([128, 128], BF16, tag="preb")
                        nc.scalar.activation(preb[:], o1[:, 0:128], ACTF.Copy,
                                             scale=rden[:, 0:1])
                        pstr = pst_pool.tile([128, 128], F32, tag="pst")
                        nc.tensor.matmul(pstr[:], lhsT=preb[:], rhs=ident[:],
                                         start=True, stop=True)
                        preT = ep.tile([128, 128], BF16, tag="preT")
                        nc.scalar.activation(preT[:], pstr[:], ACTF.Copy)
                        ps2 = pst_pool.tile([128, 64], F32, tag="pst")
                        nc.tensor.matmul(ps2[:], lhsT=preT[:], rhs=w2_sb[:],
                                         start=True, stop=True)
                        ob = ep.tile([128, OUT], F32, tag="ob")
                        nc.vector.tensor_tensor(out=ob[:], in0=ps2[:],
                                                in1=b2_sb[:], op=ALU.add)
                        nc.scalar.activation(ob[:], ob[:], ACTF.Sigmoid)
                        nc.sync.dma_start(out=out_d[j * 128:(j + 1) * 128, :],
                                          in_=ob[:])
                    if post_block is not None:
                        post_block(j)

            def l1_post(j):
                # Fire each AllGather as soon as its source blocks are done,
                # so it overlaps the remaining L1 aggregation in program
                # order on the GpSimd engine.
                if j == nbh - 1:
                    nc.gpsimd.collective_compute(
                        "AllGather", mybir.AluOpType.bypass,
                        replica_groups=[list(range(NCORES))],
                        ins=[loc1[:]], outs=[ag_out1[:]],
                    )
                elif j == nb - 1:
                    nc.gpsimd.collective_compute(
                        "AllGather", mybir.AluOpType.bypass,
                        replica_groups=[list(range(NCORES))],
                        ins=[loc2[:]], outs=[ag_out2[:]],
                    )

            agg_layer(1, post_block=l1_post)

            # ---- load the AllGathered [relu(out1) | asrc2] rows as L2 table
            for cc in range(NCORES):
                base = cc * nb * TROW
                nc.sync.dma_start(
                    out=table[:, base:base + nbh * TROW],
                    in_=ag_out1[cc * 128:(cc + 1) * 128, :])
                nc.sync.dma_start(
                    out=table[:, base + nbh * TROW:base + nb * TROW],
                    in_=ag_out2[cc * 128:(cc + 1) * 128, :])

            agg_layer(2)

    nc.finalize()
    return nc


def kernel(x, edge_index, W1, att_src1, att_dst1, b1, W2, att_src2, att_dst2,
           b2):
    from concourse import bass_utils

    x = np.asarray(x, np.float32)
    W1 = np.asarray(W1, np.float32)
    W2 = np.asarray(W2, np.float32)
    att_src1 = np.asarray(att_src1, np.float32)
    att_dst1 = np.asarray(att_dst1, np.float32)
    att_src2 = np.asarray(att_src2, np.float32)
    att_dst2 = np.asarray(att_dst2, np.float32)
    b1 = np.asarray(b1, np.float32)
    b2 = np.asarray(b2, np.float32)
    n_nodes = x.shape[0]

    sch = _build_schedule(edge_index, n_nodes)
    vpad, pc = sch["vpad"], sch["pc"]

    W1r = W1.reshape(F_IN, HEADS, HID)
    w1aug = np.zeros((128, TROW), np.float32)
    w1aug[:, 0:128] = W1
    w1aug[:, 128] = W1r[:, 0, :] @ att_src1[0]
    w1aug[:, 129] = W1r[:, 1, :] @ att_src1[1]
    wad1 = np.stack([W1r[:, h, :] @ att_dst1[h] for h in range(HEADS)], 1)
    ws2_t = np.broadcast_to(W2 @ att_src2[0], (128, 128)).copy()
    wd2_t = np.broadcast_to(W2 @ att_dst2[0], (128, 128)).copy()
    b1_rep = np.broadcast_to(b1, (128, HEADS * HID)).astype(np.float32).copy()
    b2_rep = np.broadcast_to(b2, (128, OUT)).astype(np.float32).copy()

    x_rho = np.zeros((vpad, F_IN), np.float32)
    x_rho[sch["row_of_node"]] = x
    xt_full = _bf16(np.ascontiguousarray(x_rho.T))          # [128, vpad]

    key = (vpad, sch["nch"], tuple(sch["K"].tolist()),
           tuple(sch["K_lo"].tolist()), DEBUG)
    if key not in _cache:
        _cache[key] = _build_program(vpad, pc, sch["nb"], sch["half"],
                                     sch["K"], sch["K_lo"], sch["K_hi"],
                                     sch["nch"], sch["chunk_base"])
    nc = _cache[key]

    in_maps = []
    for c in range(NCORES):
        in_maps.append({
            "xt": xt_full,
            "xtl": np.ascontiguousarray(xt_full[:, c * pc:(c + 1) * pc]),
            "idx": sch["idx_wrapped"][c],
            "msk": _bf16(sch["mask_stream"][c]),
            "ident": _bf16(np.eye(128, dtype=np.float32)),
            "w1aug": _bf16(w1aug),
            "wad1": _bf16(wad1),
            "w2": _bf16(W2),
            "ws2": _bf16(ws2_t),
            "wd2": _bf16(wd2_t),
            "b1rep": b1_rep,
            "b2rep": b2_rep,
        })
    res = bass_utils.run_bass_kernel_spmd(nc, in_maps,
                                          core_ids=list(range(NCORES)),
                                          trace=TRACE)
    kernel.last_exec_ns = res.exec_time_ns
    kernel.last_mean_ns = res.mean_exec_time_ns
    out_all = np.concatenate([res.results[c]["out"] for c in range(NCORES)], 0)
    return out_all[sch["row_of_node"][:n_nodes]]
